# revision 69
# baseline (speedup 1.0000x reference)
"""Trainium2 Bass kernel for MultiHeadAttention (B=4, S=1024, E=1024, H=16, Dh=64).

Sharding: 8 cores = (batch b in 0..3) x (head-group hg in 0..1, 8 heads each).
The reference reshapes [B,H,S,Dh] -> [B,S,E] WITHOUT transposing heads back, so
head h's attention output occupies output rows t' = h*64 + s//16 - the final
projection is row-parallel across head groups: no cross-core communication.

Per-core pipeline (PE-roofline-bound: every matmul at 1.0 cycles/row):
  - inputs host-cast to fp16 (2.4e-4 exactness, half DMA bytes), outputs bf16
    (cast back on host); 2-ec-batched DMA ordered for early PE start
  - QK projections: 4 PSUM accumulators x 4 subphases (wq/wk x s-halves) so
    the 4-bank score pool coexists - no PSUM pool boundary before attention;
    xt streamed chunk-by-chunk; bias+fp16 cast on DVE into per-(dt,sh) tiles
    (fine-grained deps so scores never wait on unrelated copies)
  - V projection (shares the QK PSUM ring) into fp16 V' with 64 ones cols per
    head (one gpsimd memset): the z matmul emits z^T AND softmax denominators
  - scores^T[t,s] = K @ Q^T per (head-pair, segment-packed col group) into
    [128,2,512] PSUM tiles (1 bank/head): ONE activation exps both heads;
    small diagonal blocks share a tile to cut activation count; e tiles bf16
    (range-safe for exp with no max-subtraction); causal triangle = 0/1
    multiply on gpsimd (SBUF-only engine) for diagonal segments only
  - z/normalize: PSUM z + sums; reciprocal(sums)->SBUF then two PSUM*SBUF
    multiplies write the scrambled-reshape layout directly in fp16
  - out = X2 @ wo (fp16), PSUM->SBUF copy on DVE, bf16 DMA out per half
  - software pipelining: z/normalize/output work is queued as fine-grained
    units and drained one per score block so the in-order PE queue always has
    ready work while ACT exps; V-projection fills pair 0; each pair's output
    projection is HELD one pair and released into the next pair's normalize
    window (fills the DVE-bound endgame); final pair orders j=1 before j=0.

TimelineSim: 99161 ns (baseline 136254), rel err 2.8e-3 (gate 2e-2).
"""
import numpy as np

B, S, E, H, DH = 4, 1024, 1024, 16, 64
NCORES = 8
HPC = 8          # heads per core
EC = 8           # 128-row chunks of E
TT = 8           # 128-row t-tiles of S
NJ = 2           # 512-col s-blocks

_CACHE = {}


def _build_causal():
    import concourse.bacc as bacc
    import concourse.tile as tile
    import concourse.mybir as mybir

    f32 = mybir.dt.float32
    f32r = mybir.dt.float32r
    f16 = mybir.dt.float16
    bf16 = mybir.dt.bfloat16
    Exp = mybir.ActivationFunctionType.Exp
    Copy = mybir.ActivationFunctionType.Copy
    Ident = mybir.ActivationFunctionType.Identity
    Ln = mybir.ActivationFunctionType.Ln
    mult = mybir.AluOpType.mult
    div = mybir.AluOpType.divide

    nc = bacc.Bacc("TRN2")
    xt = nc.dram_tensor("xt", [128, EC, S], f16, kind="ExternalInput")
    wq = nc.dram_tensor("wq", [128, EC, 512], f16, kind="ExternalInput")
    wk = nc.dram_tensor("wk", [128, EC, 512], f16, kind="ExternalInput")
    wv = nc.dram_tensor("wv", [128, EC, 512], f16, kind="ExternalInput")
    wo = nc.dram_tensor("wo", [128, EC, 1024], f16, kind="ExternalInput")
    bq = nc.dram_tensor("bq", [128, 4], f32, kind="ExternalInput")
    bk = nc.dram_tensor("bk", [128, 4], f32, kind="ExternalInput")
    tri = nc.dram_tensor("tri", [128, 128], bf16, kind="ExternalInput")
    out = nc.dram_tensor("out", [4, 128, 1024], bf16, kind="ExternalOutput")

    with tile.TileContext(nc) as tc:
        pp = tc.alloc_tile_pool(name="pp", bufs=1)
        xt_sb = pp.tile([128, EC, S], f16)
        wq_sb = pp.tile([128, EC, 512], f16)
        wk_sb = pp.tile([128, EC, 512], f16)
        wv_sb = pp.tile([128, EC, 512], f16)
        wo_sb = pp.tile([128, EC, 1024], f16)
        qt_sb = [[pp.tile([128, 512], f16, name=f"qt_{d}_{s}")
                  for s in range(2)] for d in range(4)]
        kt_sb = [[pp.tile([128, 512], f16, name=f"kt_{d}_{s}")
                  for s in range(2)] for d in range(4)]
        vp_sb = pp.tile([128, TT, 1024], f16)
        x2t_sb = pp.tile([128, EC, 512], f16)
        bq_sb = pp.tile([128, 4], f32)
        bk_sb = pp.tile([128, 4], f32)
        tri_sb = pp.tile([128, 128], bf16)

        # ---- DMA preload: 2-ec batches (HWDGE/SP fixed costs are per-DMA),
        # ordered so the QK phase can start early ----
        groups = [(0, 1), (1, 3), (3, 5), (5, 7), (7, 8)]
        for a, b in groups:
            nc.sync.dma_start(out=wq_sb[:, a:b, :], in_=wq[:, a:b, :])
            nc.sync.dma_start(out=xt_sb[:, a:b, 0:512], in_=xt[:, a:b, 0:512])
            if a == 1:
                nc.sync.dma_start(out=bq_sb, in_=bq.ap())
        nc.sync.dma_start(out=bk_sb, in_=bk.ap())
        for k in range(0, EC, 2):
            nc.sync.dma_start(out=wk_sb[:, k:k + 2, :], in_=wk[:, k:k + 2, :])
        for k in range(0, EC, 2):
            nc.sync.dma_start(out=xt_sb[:, k:k + 2, 512:1024], in_=xt[:, k:k + 2, 512:1024])
        nc.sync.dma_start(out=tri_sb, in_=tri.ap())
        for k in range(0, EC, 2):
            nc.sync.dma_start(out=wv_sb[:, k:k + 2, :], in_=wv[:, k:k + 2, :])
        for k in range(0, EC, 2):
            nc.sync.dma_start(out=wo_sb[:, k:k + 2, :], in_=wo[:, k:k + 2, :])

        vview = vp_sb.rearrange("p t (h two d) -> p t h two d", two=2, d=DH)
        for tt in range(TT):
            nc.gpsimd.memset(vview[:, tt, :, 1, :], 1.0)

        # ---- Q^T/K^T projections: 4 PSUM accumulators x 4 subphases so
        # the score pool (4 banks) can be allocated upfront: no PSUM pool
        # boundary between projections and attention ----
        scp = tc.alloc_tile_pool(name="scp", bufs=2, space="PSUM")
        qkp = tc.alloc_tile_pool(name="qkp", bufs=4, space="PSUM")
        ep = tc.alloc_tile_pool(name="ep", bufs=16)
        osp = tc.alloc_tile_pool(name="osp", bufs=2)
        rcp = tc.alloc_tile_pool(name="rcp", bufs=3)
        for sh, wi, wsb, dest, bias in (
            (0, 0, wq_sb, qt_sb, bq_sb), (0, 1, wk_sb, kt_sb, bk_sb),
            (1, 0, wq_sb, qt_sb, bq_sb), (1, 1, wk_sb, kt_sb, bk_sb),
        ):
            sl = slice(512 * sh, 512 * sh + 512)
            pss = [qkp.tile([128, 512], f32, tag="qk", name=f"qk_{sh}_{wi}_{d}")
                   for d in range(4)]
            for ec in range(EC):
                for dt_ in range(4):
                    nc.tensor.matmul(
                        pss[dt_],
                        wsb[:, ec, 128 * dt_:128 * dt_ + 128],
                        xt_sb[:, ec, sl],
                        start=(ec == 0), stop=(ec == EC - 1),
                    )
            for dt_ in range(4):
                nc.vector.tensor_scalar_add(
                    out=dest[dt_][sh], in0=pss[dt_],
                    scalar1=bias[:, dt_:dt_ + 1],
                )

        pend = []
        held = []

        def drain(n):
            for _ in range(min(n, len(pend))):
                pend.pop(0)()

        def vjob(tt):
            ps = qkp.tile([128, 512], f32, tag="qk", name=f"v_{tt}")
            for ec in range(EC):
                nc.tensor.matmul(
                    ps, xt_sb[:, ec, 128 * tt:128 * tt + 128], wv_sb[:, ec, :],
                    start=(ec == 0), stop=(ec == EC - 1),
                )
            nc.vector.tensor_copy(
                vview[:, tt, :, 0, :], ps.rearrange("p (h d) -> p h d", d=DH)
            )

        for tt in range(TT):
            pend.append(lambda tt=tt: vjob(tt))

        ztp = opp = None
        for hp in range(4):
            pair = (2 * hp, 2 * hp + 1)
            et_ref = {}
            js = [1, 0] if hp == 3 else [0, 1]
            blocks = []
            for j in js:
                cur, used = [], 0
                for tt in range(TT):
                    if 512 * j + 511 < 128 * tt:
                        continue
                    w = 512 * j + 512 - max(512 * j, 128 * tt)
                    if used + w > 512:
                        if cur:
                            blocks.append(cur)
                        cur, used = [], 0
                    cur.append((tt, j))
                    used += w
                if cur:
                    blocks.append(cur)

            def sblock(group, hp=hp, pair=pair, et_ref=et_ref):
                # group: list of (tt, j) segments packed into one score tile
                # (the small diagonal blocks share a tile so one activation
                # instruction exps them together)
                segs = []
                col = 0
                for (tt, j) in group:
                    c0 = 128 * tt
                    lo = max(512 * j, c0)
                    w = 512 * j + 512 - lo
                    segs.append((tt, j, c0, lo, w, col))
                    col += w
                total = col
                gname = f"{hp}_{group[0][0]}_{group[0][1]}"
                sc = scp.tile([128, 2, 512], f32, tag="sc", name=f"sc_{gname}")
                for (tt, j, c0, lo, w, col) in segs:
                    kc = c0 % 512
                    for hh, h in enumerate(pair):
                        pb = 64 * (h % 2)
                        nc.tensor.matmul(
                            sc[:, hh, col:col + w],
                            kt_sb[hp][tt // 4][pb:pb + 64, kc:kc + 128],
                            qt_sb[hp][j][pb:pb + 64, lo - 512 * j:512],
                            start=True, stop=True,
                        )
                et = ep.tile([128, 2, 512], bf16, tag="e", name=f"e_{gname}")
                nc.scalar.activation(
                    et[:, :, 0:total], sc[:, :, 0:total], Exp, scale=0.5
                )
                for (tt, j, c0, lo, w, col) in segs:
                    if lo == c0:
                        for hh in range(2):
                            nc.gpsimd.tensor_tensor(
                                et[:, hh, col:col + 128],
                                et[:, hh, col:col + 128], tri_sb, op=mult,
                            )
                    et_ref[(tt, j)] = (et, col, lo - 512 * j, w)

            def push_z(h, hh, j, et_ref=et_ref):
                """Append z-chain units (split mm halves + parallel divides)."""
                box = {}
                tts = [tt for tt in range(TT) if (tt, j) in et_ref]
                halves = ([tts[:4], tts[4:]] if len(tts) > 5 else [tts])

                def mm_unit(sub, first, last, h=h, hh=hh, j=j, box=box,
                            et_ref=et_ref, ntot=len(tts)):
                    if first:
                        box["zt"] = ztp.tile([128, 512], f32, tag="zt",
                                             name=f"zt_{h}_{j}")
                    zt = box["zt"]
                    for i, tt in enumerate(sub):
                        et, col, rel0, w = et_ref[(tt, j)]
                        nc.tensor.matmul(
                            zt[:, rel0:rel0 + w],
                            vp_sb[:, tt, 128 * h:128 * h + 128],
                            et[:, hh, col:col + w],
                            start=(first and i == 0),
                            stop=(last and i == len(sub) - 1),
                        )

                def div_unit(h=h, j=j, hp=hp, box=box):
                    # x2t = z / sums; both live in PSUM and an ALU op may
                    # read only one PSUM operand, so 1/sums goes via SBUF.
                    # For the final pair ACT is idle (no more exps) while DVE
                    # gates the endgame: compute 1/sums as exp(-ln(sums))
                    # there instead.
                    zt = box["zt"]
                    rec = rcp.tile([64, 512], f32, tag="rec",
                                   name=f"rec_{h}_{j}")
                    nc.vector.reciprocal(rec, zt[64:128, :])
                    zv = zt[0:64].rearrange("p (m c par) -> par p c m",
                                            m=32, c=8, par=2)
                    rv = rec.rearrange("p (m c par) -> par p c m",
                                       m=32, c=8, par=2)
                    o0 = 64 * h + 32 * j
                    nc.vector.tensor_tensor(
                        x2t_sb[0:64, :, o0:o0 + 32], zv[0], rv[0], op=mult)
                    nc.vector.tensor_tensor(
                        x2t_sb[64:128, :, o0:o0 + 32], zv[1], rv[1], op=mult)

                for si, sub in enumerate(halves):
                    pend.append(lambda s=sub, f=mm_unit, first=(si == 0),
                                last=(si == len(halves) - 1): f(s, first, last))
                pend.append(div_unit)

            osb_box = {}

            def o_unit(eh, half, hp=hp, osb_box=osb_box):
                if "t" not in osb_box:
                    osb_box["t"] = osp.tile([128, 1024], bf16, tag="osb",
                                            name=f"osb_{hp}")
                t = osb_box["t"]
                if half == 0:
                    osb_box[eh] = opp.tile([128, 512], f32, tag="op",
                                           name=f"op_{hp}_{eh}")
                op_t = osb_box[eh]
                cs = range(0, 4) if half == 0 else range(4, EC)
                for c in cs:
                    nc.tensor.matmul(
                        op_t, x2t_sb[:, c, 128 * hp:128 * hp + 128],
                        wo_sb[:, c, 512 * eh:512 * eh + 512],
                        start=(c == 0), stop=(c == EC - 1),
                    )
                if half == 1:
                    nc.vector.tensor_copy(t[:, 512 * eh:512 * eh + 512], op_t)
                    # final pair: issue eh0's transfer from the (idle)
                    # ACT queue so the last transfer isn't serialized
                    eng = nc.scalar if (eh == 0 and hp == 3) else nc.sync
                    eng.dma_start(out=out[hp, :, 512 * eh:512 * eh + 512],
                                  in_=t[:, 512 * eh:512 * eh + 512])

            nj0 = sum(1 for g in blocks if g[0][1] == js[0])
            for bi, group in enumerate(blocks):
                sblock(group)
                drain(1)
                if bi == nj0 - 1 and hp > 0:
                    # first j-group e-tiles complete: its z can drain now
                    for hh, h in enumerate(pair):
                        push_z(h, hh, js[0])
            if hp == 0:
                while pend:
                    pend.pop(0)()
                qkp.release()
                ztp = tc.alloc_tile_pool(name="ztp", bufs=2, space="PSUM")
                opp = tc.alloc_tile_pool(name="opp", bufs=2, space="PSUM")
                for hh, h in enumerate(pair):
                    push_z(h, hh, js[0])
            for hh, h in enumerate(pair):
                push_z(h, hh, js[1])
                if held:
                    # previous pair's output projection was held back: its
                    # matmuls fill the PE idle window while DVE normalizes
                    pend.append(held.pop(0))
                    pend.append(held.pop(0))
            for eh in range(2):
                for half in range(2):
                    u = (lambda f=o_unit, eh=eh, half=half: f(eh, half))
                    (held if hp < 3 else pend).append(u)

        while pend:
            pend.pop(0)()
        for p_ in (rcp, osp, ep, opp, ztp, scp):
            p_.release()
        pp.release()
    nc.compile()
    return nc


def _build_generic():
    """Fallback for a non-causal mask: the original fp32r kernel."""
    import concourse.bacc as bacc
    import concourse.tile as tile
    import concourse.mybir as mybir

    f32 = mybir.dt.float32
    f32r = mybir.dt.float32r
    bf16 = mybir.dt.bfloat16
    Exp = mybir.ActivationFunctionType.Exp
    mult = mybir.AluOpType.mult

    nc = bacc.Bacc("TRN2")
    xt = nc.dram_tensor("xt", [128, EC, S], f32r, kind="ExternalInput")
    wq = nc.dram_tensor("wq", [128, EC, 512], f32r, kind="ExternalInput")
    wk = nc.dram_tensor("wk", [128, EC, 512], f32r, kind="ExternalInput")
    wv = nc.dram_tensor("wv", [128, EC, 512], f32r, kind="ExternalInput")
    wo = nc.dram_tensor("wo", [128, EC, 1024], f32r, kind="ExternalInput")
    bq = nc.dram_tensor("bq", [128, 4], f32, kind="ExternalInput")
    bk = nc.dram_tensor("bk", [128, 4], f32, kind="ExternalInput")
    mkt = nc.dram_tensor("mkt", [128, TT, S], bf16, kind="ExternalInput")
    out = nc.dram_tensor("out", [4, 128, 1024], f32, kind="ExternalOutput")

    with tile.TileContext(nc) as tc:
        with (
            tc.tile_pool(name="persist", bufs=1) as pp,
            tc.tile_pool(name="mm", bufs=3, space="PSUM") as mm,
            tc.tile_pool(name="ztp", bufs=2, space="PSUM") as ztp,
        ):
            p1 = tc.alloc_tile_pool(name="p1", bufs=1)
            xt_sb = p1.tile([128, EC, S], f32r)
            wq_sb = p1.tile([128, EC, 512], f32r)
            wk_sb = p1.tile([128, EC, 512], f32r)
            wv_sb = p1.tile([128, EC, 512], f32r)
            for k in range(0, EC, 2):
                nc.sync.dma_start(out=xt_sb[:, k:k + 2, :], in_=xt[:, k:k + 2, :])
                nc.sync.dma_start(out=wq_sb[:, k:k + 2, :], in_=wq[:, k:k + 2, :])
                nc.sync.dma_start(out=wk_sb[:, k:k + 2, :], in_=wk[:, k:k + 2, :])
                nc.sync.dma_start(out=wv_sb[:, k:k + 2, :], in_=wv[:, k:k + 2, :])
            qt_sb = pp.tile([128, 4, S], f32r)
            kt_sb = pp.tile([128, 4, S], f32r)
            vp_sb = pp.tile([128, TT, 1024], bf16)
            x2t_sb = pp.tile([128, EC, 512], f32r)
            bq_sb = pp.tile([128, 4], f32)
            bk_sb = pp.tile([128, 4], f32)
            mkt_sb = pp.tile([128, TT, S], bf16)
            nc.sync.dma_start(out=mkt_sb, in_=mkt.ap())
            nc.sync.dma_start(out=bq_sb, in_=bq.ap())
            nc.sync.dma_start(out=bk_sb, in_=bk.ap())

            vview = vp_sb.rearrange("p t (h two d) -> p t h two d", two=2, d=DH)
            ones_sb = pp.tile([128, 512], f32)
            nc.vector.memset(ones_sb, 1.0)
            ones_v = ones_sb.rearrange("p (h d) -> p h d", d=DH)
            for tt in range(TT):
                nc.vector.tensor_copy(vview[:, tt, :, 1, :], ones_v)

            for wsb, dest, bias in ((wq_sb, qt_sb, bq_sb), (wk_sb, kt_sb, bk_sb)):
                for dt_ in range(4):
                    for sh in range(2):
                        ps = mm.tile([128, 512], f32, tag="mm")
                        for ec in range(EC):
                            nc.tensor.matmul(
                                ps, wsb[:, ec, 128 * dt_:128 * dt_ + 128],
                                xt_sb[:, ec, 512 * sh:512 * sh + 512],
                                start=(ec == 0), stop=(ec == EC - 1),
                            )
                        nc.vector.tensor_scalar_add(
                            out=dest[:, dt_, 512 * sh:512 * sh + 512],
                            in0=ps, scalar1=bias[:, dt_:dt_ + 1],
                        )
            for tt in range(TT):
                ps = mm.tile([128, 512], f32, tag="mm")
                for ec in range(EC):
                    nc.tensor.matmul(
                        ps, xt_sb[:, ec, 128 * tt:128 * tt + 128],
                        wv_sb[:, ec, :],
                        start=(ec == 0), stop=(ec == EC - 1),
                    )
                nc.vector.tensor_copy(
                    vview[:, tt, :, 0, :], ps.rearrange("p (h d) -> p h d", d=DH)
                )
            p1.release()
            late = tc.alloc_tile_pool(name="late", bufs=1)
            expa = tc.alloc_tile_pool(name="expa", bufs=16)
            small = tc.alloc_tile_pool(name="small", bufs=2)
            outp = tc.alloc_tile_pool(name="outp", bufs=2)
            wo_sb = late.tile([128, EC, 1024], f32r)
            nc.sync.dma_start(out=wo_sb, in_=wo.ap())

            for hp in range(HPC // 2):
                pair = (2 * hp, 2 * hp + 1)
                et = {}
                for tt in range(TT):
                    pss = {}
                    for h in pair:
                        dt_ = h // 2
                        pb = 64 * (h % 2)
                        ps = mm.tile([128, 1024], f32, tag="mm", name=f"ps_{h}_{tt}")
                        pss[h] = ps
                        for j in range(NJ):
                            nc.tensor.matmul(
                                ps[:, 512 * j:512 * j + 512],
                                kt_sb[pb:pb + 64, dt_, 128 * tt:128 * tt + 128],
                                qt_sb[pb:pb + 64, dt_, 512 * j:512 * j + 512],
                                start=True, stop=True,
                            )
                            nc.vector.tensor_add(
                                ps[:, 512 * j:512 * j + 512],
                                ps[:, 512 * j:512 * j + 512],
                                mkt_sb[:, tt, 512 * j:512 * j + 512],
                            )
                    for h in pair:
                        ps = pss[h]
                        e = expa.tile([128, 1024], bf16, tag="expa",
                                      name=f"e_{h}_{tt}")
                        nc.scalar.activation(e[:, :], ps[:, :], Exp, scale=0.5)
                        for j in range(NJ):
                            et[(h, tt, j)] = e[:, 512 * j:512 * j + 512]
                for h in pair:
                    zt_f = small.tile([64, S], f32, tag="ztf", name=f"ztf_{h}")
                    rec = small.tile([64, S], f32, tag="rec", name=f"rec_{h}")
                    for j in range(NJ):
                        zt = ztp.tile([128, 512], f32, tag="zt", name=f"zt_{h}_{j}")
                        for i, tt in enumerate(range(TT)):
                            nc.tensor.matmul(
                                zt, vp_sb[:, tt, 128 * h:128 * h + 128],
                                et[(h, tt, j)],
                                start=(i == 0), stop=(i == TT - 1),
                            )
                        nc.vector.reciprocal(rec[:, 512 * j:512 * j + 512],
                                             zt[64:128, :])
                        nc.vector.tensor_copy(zt_f[:, 512 * j:512 * j + 512],
                                              zt[0:64, :])
                    zv = zt_f.rearrange("p (m c par) -> par p c m", m=64, c=8, par=2)
                    rv = rec.rearrange("p (m c par) -> par p c m", m=64, c=8, par=2)
                    for P in range(2):
                        nc.vector.tensor_tensor(
                            x2t_sb[64 * P:64 * P + 64, :, 64 * h:64 * h + 64],
                            zv[P], rv[P], op=mult,
                        )

            for tp in range(4):
                osb = outp.tile([128, 1024], f32, tag="osb")
                for eh in range(2):
                    ps = mm.tile([128, 512], f32, tag="mm")
                    for c in range(EC):
                        nc.tensor.matmul(
                            ps, x2t_sb[:, c, 128 * tp:128 * tp + 128],
                            wo_sb[:, c, 512 * eh:512 * eh + 512],
                            start=(c == 0), stop=(c == EC - 1),
                        )
                    nc.vector.tensor_copy(osb[:, 512 * eh:512 * eh + 512], ps)
                nc.sync.dma_start(out=out[tp], in_=osb)
            for p in (outp, small, expa, late):
                p.release()
    nc.compile()
    return nc


def kernel(inputs, mask, wq, bq, wk, bk, wv, bv, wo, bo):
    import ml_dtypes
    from concourse.bass_utils import run_bass_kernel_spmd

    x = np.asarray(inputs, dtype=np.float32)
    wq = np.asarray(wq, dtype=np.float32)
    wk = np.asarray(wk, dtype=np.float32)
    wv = np.asarray(wv, dtype=np.float32)
    wo = np.asarray(wo, dtype=np.float32)
    bq = np.asarray(bq, dtype=np.float32)
    bk = np.asarray(bk, dtype=np.float32)
    mask2d = np.asarray(mask, dtype=np.float32).reshape(S, S)
    causal_ref = 1.0 - np.tril(np.ones((S, S), dtype=np.float32))
    causal = bool(np.array_equal(mask2d, causal_ref))
    variant = "causal" if causal else "generic"
    if variant not in _CACHE:
        _CACHE[variant] = _build_causal() if causal else _build_generic()
    nc = _CACHE[variant]

    in_maps = []
    for c in range(NCORES):
        b, hg = c // 2, c % 2
        sl = slice(512 * hg, 512 * hg + 512)
        if causal:
            f16 = np.float16
            m = {
                "xt": np.ascontiguousarray(
                    x[b].T.reshape(EC, 128, S).transpose(1, 0, 2)).astype(f16),
                "wq": np.ascontiguousarray(
                    wq[:, sl].reshape(EC, 128, 512).transpose(1, 0, 2)).astype(f16),
                "wk": np.ascontiguousarray(
                    wk[:, sl].reshape(EC, 128, 512).transpose(1, 0, 2)).astype(f16),
                "wv": np.ascontiguousarray(
                    wv[:, sl].reshape(EC, 128, 512).transpose(1, 0, 2)).astype(f16),
                "wo": np.ascontiguousarray(
                    wo.reshape(EC, 128, 1024).transpose(1, 0, 2)).astype(f16),
                "bq": np.ascontiguousarray(bq[sl].reshape(4, 128).T),
                "bk": np.ascontiguousarray(bk[sl].reshape(4, 128).T),
                "tri": np.triu(np.ones((128, 128))).astype(ml_dtypes.bfloat16),
            }
        else:
            m = {
                "xt": np.ascontiguousarray(x[b].T.reshape(EC, 128, S).transpose(1, 0, 2)),
                "wq": np.ascontiguousarray(wq[:, sl].reshape(EC, 128, 512).transpose(1, 0, 2)),
                "wk": np.ascontiguousarray(wk[:, sl].reshape(EC, 128, 512).transpose(1, 0, 2)),
                "wv": np.ascontiguousarray(wv[:, sl].reshape(EC, 128, 512).transpose(1, 0, 2)),
                "wo": np.ascontiguousarray(wo.reshape(EC, 128, 1024).transpose(1, 0, 2)),
                "bq": np.ascontiguousarray(bq[sl].reshape(4, 128).T),
                "bk": np.ascontiguousarray(bk[sl].reshape(4, 128).T),
                "mkt": np.ascontiguousarray(
                    (mask2d.T * np.float32(-2e9)).reshape(TT, 128, S).transpose(1, 0, 2)
                ).astype(ml_dtypes.bfloat16),
            }
        in_maps.append(m)

    global _last_in_maps
    _last_in_maps = in_maps
    res = run_bass_kernel_spmd(nc, in_maps, core_ids=list(range(NCORES)))
    full = np.empty((B, S, E), dtype=np.float32)
    for c in range(NCORES):
        b, hg = c // 2, c % 2
        full[b, 512 * hg:512 * hg + 512, :] = np.asarray(
            res.results[c]["out"], dtype=np.float32).reshape(512, 1024)

    # biases bv/bo are zero in this problem; fold in exactly if ever nonzero.
    bv = np.asarray(bv, dtype=np.float32)
    bo = np.asarray(bo, dtype=np.float32)
    if np.any(bv != 0):
        bmat = np.zeros((S, E), dtype=np.float64)
        tpr = np.arange(S)
        e = np.arange(E)
        bmat[:, :] = bv[(64 * (tpr[:, None] // 64) + e[None, :] % 64)]
        full += (bmat @ np.asarray(wo, dtype=np.float64)).astype(np.float32)[None]
    if np.any(bo != 0):
        full += bo[None, None, :]
    return full


# revision 70
# speedup vs baseline: 1.0028x; 1.0028x over previous
"""Trainium2 Bass kernel for MultiHeadAttention (B=4, S=1024, E=1024, H=16, Dh=64).

Sharding: 8 cores = (batch b in 0..3) x (head-group hg in 0..1, 8 heads each).
The reference reshapes [B,H,S,Dh] -> [B,S,E] WITHOUT transposing heads back, so
head h's attention output occupies output rows t' = h*64 + s//16 - the final
projection is row-parallel across head groups: no cross-core communication.

Per-core pipeline (PE-roofline-bound: every matmul at 1.0 cycles/row):
  - inputs host-cast to fp16 (2.4e-4 exactness, half DMA bytes), outputs bf16
    (cast back on host); 2-ec-batched DMA ordered for early PE start
  - QK projections: 4 PSUM accumulators x 4 subphases (wq/wk x s-halves) so
    the 4-bank score pool coexists - no PSUM pool boundary before attention;
    xt streamed chunk-by-chunk; bias+fp16 cast on DVE into per-(dt,sh) tiles
    (fine-grained deps so scores never wait on unrelated copies)
  - V projection (shares the QK PSUM ring) into fp16 V' with 64 ones cols per
    head (one gpsimd memset): the z matmul emits z^T AND softmax denominators
  - scores^T[t,s] = K @ Q^T per (head-pair, segment-packed col group) into
    [128,2,512] PSUM tiles (1 bank/head): ONE activation exps both heads;
    small diagonal blocks share a tile to cut activation count; e tiles bf16
    (range-safe for exp with no max-subtraction); causal triangle = 0/1
    multiply on gpsimd (SBUF-only engine) for diagonal segments only
  - z/normalize: PSUM z + sums; reciprocal(sums)->SBUF then two PSUM*SBUF
    multiplies write the scrambled-reshape layout directly in fp16
  - out = X2 @ wo (fp16), PSUM->SBUF copy on DVE, bf16 DMA out per half
  - software pipelining: z/normalize/output work is queued as fine-grained
    units and drained one per score block so the in-order PE queue always has
    ready work while ACT exps; V-projection fills pair 0; each pair's output
    projection is HELD one pair and released into the next pair's normalize
    window (fills the DVE-bound endgame); final pair orders j=1 before j=0.

TimelineSim: 99161 ns (baseline 136254), rel err 2.8e-3 (gate 2e-2).
"""
import numpy as np

B, S, E, H, DH = 4, 1024, 1024, 16, 64
NCORES = 8
HPC = 8          # heads per core
EC = 8           # 128-row chunks of E
TT = 8           # 128-row t-tiles of S
NJ = 2           # 512-col s-blocks

_CACHE = {}


def _build_causal():
    import concourse.bacc as bacc
    import concourse.tile as tile
    import concourse.mybir as mybir

    f32 = mybir.dt.float32
    f32r = mybir.dt.float32r
    f16 = mybir.dt.float16
    bf16 = mybir.dt.bfloat16
    Exp = mybir.ActivationFunctionType.Exp
    Copy = mybir.ActivationFunctionType.Copy
    Ident = mybir.ActivationFunctionType.Identity
    Ln = mybir.ActivationFunctionType.Ln
    mult = mybir.AluOpType.mult
    div = mybir.AluOpType.divide

    nc = bacc.Bacc("TRN2")
    xt = nc.dram_tensor("xt", [128, EC, S], f16, kind="ExternalInput")
    wq = nc.dram_tensor("wq", [128, EC, 512], f16, kind="ExternalInput")
    wk = nc.dram_tensor("wk", [128, EC, 512], f16, kind="ExternalInput")
    wv = nc.dram_tensor("wv", [128, EC, 512], f16, kind="ExternalInput")
    wo = nc.dram_tensor("wo", [128, EC, 1024], f16, kind="ExternalInput")
    bq = nc.dram_tensor("bq", [128, 4], f32, kind="ExternalInput")
    bk = nc.dram_tensor("bk", [128, 4], f32, kind="ExternalInput")
    tri = nc.dram_tensor("tri", [128, 128], bf16, kind="ExternalInput")
    out = nc.dram_tensor("out", [4, 128, 1024], bf16, kind="ExternalOutput")

    with tile.TileContext(nc) as tc:
        pp = tc.alloc_tile_pool(name="pp", bufs=1)
        xt_sb = pp.tile([128, EC, S], f16)
        wq_sb = pp.tile([128, EC, 512], f16)
        wk_sb = pp.tile([128, EC, 512], f16)
        wv_sb = pp.tile([128, EC, 512], f16)
        wo_sb = pp.tile([128, EC, 1024], f16)
        qt_sb = [[pp.tile([128, 512], f16, name=f"qt_{d}_{s}")
                  for s in range(2)] for d in range(4)]
        kt_sb = [[pp.tile([128, 512], f16, name=f"kt_{d}_{s}")
                  for s in range(2)] for d in range(4)]
        vp_sb = pp.tile([128, TT, 1024], f16)
        x2t_sb = pp.tile([128, EC, 512], f16)
        bq_sb = pp.tile([128, 4], f32)
        bk_sb = pp.tile([128, 4], f32)
        tri_sb = pp.tile([128, 128], bf16)

        # ---- DMA preload: 2-ec batches (HWDGE/SP fixed costs are per-DMA),
        # ordered so the QK phase can start early ----
        groups = [(0, 1), (1, 3), (3, 5), (5, 7), (7, 8)]
        for a, b in groups:
            nc.sync.dma_start(out=wq_sb[:, a:b, :], in_=wq[:, a:b, :])
            nc.sync.dma_start(out=xt_sb[:, a:b, 0:512], in_=xt[:, a:b, 0:512])
            if a == 1:
                nc.sync.dma_start(out=bq_sb, in_=bq.ap())
        nc.sync.dma_start(out=bk_sb, in_=bk.ap())
        for k in range(0, EC, 2):
            nc.sync.dma_start(out=wk_sb[:, k:k + 2, :], in_=wk[:, k:k + 2, :])
        for k in range(0, EC, 2):
            nc.sync.dma_start(out=xt_sb[:, k:k + 2, 512:1024], in_=xt[:, k:k + 2, 512:1024])
        nc.sync.dma_start(out=tri_sb, in_=tri.ap())
        for k in range(0, EC, 2):
            nc.sync.dma_start(out=wv_sb[:, k:k + 2, :], in_=wv[:, k:k + 2, :])
        for k in range(0, EC, 2):
            nc.sync.dma_start(out=wo_sb[:, k:k + 2, :], in_=wo[:, k:k + 2, :])

        vview = vp_sb.rearrange("p t (h two d) -> p t h two d", two=2, d=DH)
        for tt in range(TT):
            nc.gpsimd.memset(vview[:, tt, :, 1, :], 1.0)

        # ---- Q^T/K^T projections: 4 PSUM accumulators x 4 subphases so
        # the score pool (4 banks) can be allocated upfront: no PSUM pool
        # boundary between projections and attention ----
        scp = tc.alloc_tile_pool(name="scp", bufs=2, space="PSUM")
        qkp = tc.alloc_tile_pool(name="qkp", bufs=4, space="PSUM")
        ep = tc.alloc_tile_pool(name="ep", bufs=16)
        osp = tc.alloc_tile_pool(name="osp", bufs=2)
        rcp = tc.alloc_tile_pool(name="rcp", bufs=3)
        for sh, wi, wsb, dest, bias in (
            (0, 0, wq_sb, qt_sb, bq_sb), (0, 1, wk_sb, kt_sb, bk_sb),
            (1, 0, wq_sb, qt_sb, bq_sb), (1, 1, wk_sb, kt_sb, bk_sb),
        ):
            sl = slice(512 * sh, 512 * sh + 512)
            pss = [qkp.tile([128, 512], f32, tag="qk", name=f"qk_{sh}_{wi}_{d}")
                   for d in range(4)]
            for ec in range(EC):
                for dt_ in range(4):
                    nc.tensor.matmul(
                        pss[dt_],
                        wsb[:, ec, 128 * dt_:128 * dt_ + 128],
                        xt_sb[:, ec, sl],
                        start=(ec == 0), stop=(ec == EC - 1),
                    )
            for dt_ in range(4):
                nc.vector.tensor_scalar_add(
                    out=dest[dt_][sh], in0=pss[dt_],
                    scalar1=bias[:, dt_:dt_ + 1],
                )

        pend = []
        held = []

        def drain(n):
            for _ in range(min(n, len(pend))):
                pend.pop(0)()

        def vjob(tt):
            ps = qkp.tile([128, 512], f32, tag="qk", name=f"v_{tt}")
            for ec in range(EC):
                nc.tensor.matmul(
                    ps, xt_sb[:, ec, 128 * tt:128 * tt + 128], wv_sb[:, ec, :],
                    start=(ec == 0), stop=(ec == EC - 1),
                )
            nc.vector.tensor_copy(
                vview[:, tt, :, 0, :], ps.rearrange("p (h d) -> p h d", d=DH)
            )

        for tt in range(TT):
            pend.append(lambda tt=tt: vjob(tt))

        ztp = opp = None
        for hp in range(4):
            pair = (2 * hp, 2 * hp + 1)
            et_ref = {}
            js = [0, 1]
            blocks = []
            for j in js:
                cur, used = [], 0
                for tt in range(TT):
                    if 512 * j + 511 < 128 * tt:
                        continue
                    w = 512 * j + 512 - max(512 * j, 128 * tt)
                    if used + w > 512:
                        if cur:
                            blocks.append(cur)
                        cur, used = [], 0
                    cur.append((tt, j))
                    used += w
                if cur:
                    blocks.append(cur)

            def sblock(group, hp=hp, pair=pair, et_ref=et_ref):
                # group: list of (tt, j) segments packed into one score tile
                # (the small diagonal blocks share a tile so one activation
                # instruction exps them together)
                segs = []
                col = 0
                for (tt, j) in group:
                    c0 = 128 * tt
                    lo = max(512 * j, c0)
                    w = 512 * j + 512 - lo
                    segs.append((tt, j, c0, lo, w, col))
                    col += w
                total = col
                gname = f"{hp}_{group[0][0]}_{group[0][1]}"
                sc = scp.tile([128, 2, 512], f32, tag="sc", name=f"sc_{gname}")
                for (tt, j, c0, lo, w, col) in segs:
                    kc = c0 % 512
                    for hh, h in enumerate(pair):
                        pb = 64 * (h % 2)
                        nc.tensor.matmul(
                            sc[:, hh, col:col + w],
                            kt_sb[hp][tt // 4][pb:pb + 64, kc:kc + 128],
                            qt_sb[hp][j][pb:pb + 64, lo - 512 * j:512],
                            start=True, stop=True,
                        )
                et = ep.tile([128, 2, 512], bf16, tag="e", name=f"e_{gname}")
                nc.scalar.activation(
                    et[:, :, 0:total], sc[:, :, 0:total], Exp, scale=0.5
                )
                for (tt, j, c0, lo, w, col) in segs:
                    if lo == c0:
                        for hh in range(2):
                            nc.gpsimd.tensor_tensor(
                                et[:, hh, col:col + 128],
                                et[:, hh, col:col + 128], tri_sb, op=mult,
                            )
                    et_ref[(tt, j)] = (et, col, lo - 512 * j, w)

            def push_z(h, hh, j, et_ref=et_ref):
                """Append z-chain units (split mm halves + parallel divides)."""
                box = {}
                tts = [tt for tt in range(TT) if (tt, j) in et_ref]
                halves = ([tts[:4], tts[4:]] if len(tts) > 5 else [tts])

                def mm_unit(sub, first, last, h=h, hh=hh, j=j, box=box,
                            et_ref=et_ref, ntot=len(tts)):
                    if first:
                        box["zt"] = ztp.tile([128, 512], f32, tag="zt",
                                             name=f"zt_{h}_{j}")
                    zt = box["zt"]
                    for i, tt in enumerate(sub):
                        et, col, rel0, w = et_ref[(tt, j)]
                        nc.tensor.matmul(
                            zt[:, rel0:rel0 + w],
                            vp_sb[:, tt, 128 * h:128 * h + 128],
                            et[:, hh, col:col + w],
                            start=(first and i == 0),
                            stop=(last and i == len(sub) - 1),
                        )

                def div_unit(h=h, j=j, hp=hp, box=box):
                    # x2t = z / sums; both live in PSUM and an ALU op may
                    # read only one PSUM operand, so 1/sums goes via SBUF.
                    # For the final pair ACT is idle (no more exps) while DVE
                    # gates the endgame: compute 1/sums as exp(-ln(sums))
                    # there instead.
                    zt = box["zt"]
                    rec = rcp.tile([64, 512], f32, tag="rec",
                                   name=f"rec_{h}_{j}")
                    nc.vector.reciprocal(rec, zt[64:128, :])
                    zv = zt[0:64].rearrange("p (m c par) -> par p c m",
                                            m=32, c=8, par=2)
                    rv = rec.rearrange("p (m c par) -> par p c m",
                                       m=32, c=8, par=2)
                    o0 = 64 * h + 32 * j
                    nc.vector.tensor_tensor(
                        x2t_sb[0:64, :, o0:o0 + 32], zv[0], rv[0], op=mult)
                    nc.vector.tensor_tensor(
                        x2t_sb[64:128, :, o0:o0 + 32], zv[1], rv[1], op=mult)

                for si, sub in enumerate(halves):
                    pend.append(lambda s=sub, f=mm_unit, first=(si == 0),
                                last=(si == len(halves) - 1): f(s, first, last))
                pend.append(div_unit)

            osb_box = {}

            def o_unit(eh, half, hp=hp, osb_box=osb_box):
                if "t" not in osb_box:
                    osb_box["t"] = osp.tile([128, 1024], bf16, tag="osb",
                                            name=f"osb_{hp}")
                t = osb_box["t"]
                if half == 0:
                    osb_box[eh] = opp.tile([128, 512], f32, tag="op",
                                           name=f"op_{hp}_{eh}")
                op_t = osb_box[eh]
                cs = range(0, 4) if half == 0 else range(4, EC)
                for c in cs:
                    nc.tensor.matmul(
                        op_t, x2t_sb[:, c, 128 * hp:128 * hp + 128],
                        wo_sb[:, c, 512 * eh:512 * eh + 512],
                        start=(c == 0), stop=(c == EC - 1),
                    )
                if half == 1:
                    nc.vector.tensor_copy(t[:, 512 * eh:512 * eh + 512], op_t)
                    # final pair: issue eh0's transfer from the (idle)
                    # ACT queue so the last transfer isn't serialized
                    eng = nc.scalar if (eh == 0 and hp == 3) else nc.sync
                    eng.dma_start(out=out[hp, :, 512 * eh:512 * eh + 512],
                                  in_=t[:, 512 * eh:512 * eh + 512])

            nj0 = sum(1 for g in blocks if g[0][1] == js[0])
            for bi, group in enumerate(blocks):
                sblock(group)
                drain(1)
                if bi == nj0 - 1 and hp > 0:
                    # first j-group e-tiles complete: its z can drain now
                    for hh, h in enumerate(pair):
                        push_z(h, hh, js[0])
            if hp == 0:
                while pend:
                    pend.pop(0)()
                qkp.release()
                ztp = tc.alloc_tile_pool(name="ztp", bufs=2, space="PSUM")
                opp = tc.alloc_tile_pool(name="opp", bufs=2, space="PSUM")
                for hh, h in enumerate(pair):
                    push_z(h, hh, js[0])
            for hh, h in enumerate(pair):
                push_z(h, hh, js[1])
                if held:
                    # previous pair's output projection was held back: its
                    # matmuls fill the PE idle window while DVE normalizes
                    pend.append(held.pop(0))
                    pend.append(held.pop(0))
            for eh in range(2):
                for half in range(2):
                    u = (lambda f=o_unit, eh=eh, half=half: f(eh, half))
                    (held if hp < 3 else pend).append(u)

        while pend:
            pend.pop(0)()
        for p_ in (rcp, osp, ep, opp, ztp, scp):
            p_.release()
        pp.release()
    nc.compile()
    return nc


def _build_generic():
    """Fallback for a non-causal mask: the original fp32r kernel."""
    import concourse.bacc as bacc
    import concourse.tile as tile
    import concourse.mybir as mybir

    f32 = mybir.dt.float32
    f32r = mybir.dt.float32r
    bf16 = mybir.dt.bfloat16
    Exp = mybir.ActivationFunctionType.Exp
    mult = mybir.AluOpType.mult

    nc = bacc.Bacc("TRN2")
    xt = nc.dram_tensor("xt", [128, EC, S], f32r, kind="ExternalInput")
    wq = nc.dram_tensor("wq", [128, EC, 512], f32r, kind="ExternalInput")
    wk = nc.dram_tensor("wk", [128, EC, 512], f32r, kind="ExternalInput")
    wv = nc.dram_tensor("wv", [128, EC, 512], f32r, kind="ExternalInput")
    wo = nc.dram_tensor("wo", [128, EC, 1024], f32r, kind="ExternalInput")
    bq = nc.dram_tensor("bq", [128, 4], f32, kind="ExternalInput")
    bk = nc.dram_tensor("bk", [128, 4], f32, kind="ExternalInput")
    mkt = nc.dram_tensor("mkt", [128, TT, S], bf16, kind="ExternalInput")
    out = nc.dram_tensor("out", [4, 128, 1024], f32, kind="ExternalOutput")

    with tile.TileContext(nc) as tc:
        with (
            tc.tile_pool(name="persist", bufs=1) as pp,
            tc.tile_pool(name="mm", bufs=3, space="PSUM") as mm,
            tc.tile_pool(name="ztp", bufs=2, space="PSUM") as ztp,
        ):
            p1 = tc.alloc_tile_pool(name="p1", bufs=1)
            xt_sb = p1.tile([128, EC, S], f32r)
            wq_sb = p1.tile([128, EC, 512], f32r)
            wk_sb = p1.tile([128, EC, 512], f32r)
            wv_sb = p1.tile([128, EC, 512], f32r)
            for k in range(0, EC, 2):
                nc.sync.dma_start(out=xt_sb[:, k:k + 2, :], in_=xt[:, k:k + 2, :])
                nc.sync.dma_start(out=wq_sb[:, k:k + 2, :], in_=wq[:, k:k + 2, :])
                nc.sync.dma_start(out=wk_sb[:, k:k + 2, :], in_=wk[:, k:k + 2, :])
                nc.sync.dma_start(out=wv_sb[:, k:k + 2, :], in_=wv[:, k:k + 2, :])
            qt_sb = pp.tile([128, 4, S], f32r)
            kt_sb = pp.tile([128, 4, S], f32r)
            vp_sb = pp.tile([128, TT, 1024], bf16)
            x2t_sb = pp.tile([128, EC, 512], f32r)
            bq_sb = pp.tile([128, 4], f32)
            bk_sb = pp.tile([128, 4], f32)
            mkt_sb = pp.tile([128, TT, S], bf16)
            nc.sync.dma_start(out=mkt_sb, in_=mkt.ap())
            nc.sync.dma_start(out=bq_sb, in_=bq.ap())
            nc.sync.dma_start(out=bk_sb, in_=bk.ap())

            vview = vp_sb.rearrange("p t (h two d) -> p t h two d", two=2, d=DH)
            ones_sb = pp.tile([128, 512], f32)
            nc.vector.memset(ones_sb, 1.0)
            ones_v = ones_sb.rearrange("p (h d) -> p h d", d=DH)
            for tt in range(TT):
                nc.vector.tensor_copy(vview[:, tt, :, 1, :], ones_v)

            for wsb, dest, bias in ((wq_sb, qt_sb, bq_sb), (wk_sb, kt_sb, bk_sb)):
                for dt_ in range(4):
                    for sh in range(2):
                        ps = mm.tile([128, 512], f32, tag="mm")
                        for ec in range(EC):
                            nc.tensor.matmul(
                                ps, wsb[:, ec, 128 * dt_:128 * dt_ + 128],
                                xt_sb[:, ec, 512 * sh:512 * sh + 512],
                                start=(ec == 0), stop=(ec == EC - 1),
                            )
                        nc.vector.tensor_scalar_add(
                            out=dest[:, dt_, 512 * sh:512 * sh + 512],
                            in0=ps, scalar1=bias[:, dt_:dt_ + 1],
                        )
            for tt in range(TT):
                ps = mm.tile([128, 512], f32, tag="mm")
                for ec in range(EC):
                    nc.tensor.matmul(
                        ps, xt_sb[:, ec, 128 * tt:128 * tt + 128],
                        wv_sb[:, ec, :],
                        start=(ec == 0), stop=(ec == EC - 1),
                    )
                nc.vector.tensor_copy(
                    vview[:, tt, :, 0, :], ps.rearrange("p (h d) -> p h d", d=DH)
                )
            p1.release()
            late = tc.alloc_tile_pool(name="late", bufs=1)
            expa = tc.alloc_tile_pool(name="expa", bufs=16)
            small = tc.alloc_tile_pool(name="small", bufs=2)
            outp = tc.alloc_tile_pool(name="outp", bufs=2)
            wo_sb = late.tile([128, EC, 1024], f32r)
            nc.sync.dma_start(out=wo_sb, in_=wo.ap())

            for hp in range(HPC // 2):
                pair = (2 * hp, 2 * hp + 1)
                et = {}
                for tt in range(TT):
                    pss = {}
                    for h in pair:
                        dt_ = h // 2
                        pb = 64 * (h % 2)
                        ps = mm.tile([128, 1024], f32, tag="mm", name=f"ps_{h}_{tt}")
                        pss[h] = ps
                        for j in range(NJ):
                            nc.tensor.matmul(
                                ps[:, 512 * j:512 * j + 512],
                                kt_sb[pb:pb + 64, dt_, 128 * tt:128 * tt + 128],
                                qt_sb[pb:pb + 64, dt_, 512 * j:512 * j + 512],
                                start=True, stop=True,
                            )
                            nc.vector.tensor_add(
                                ps[:, 512 * j:512 * j + 512],
                                ps[:, 512 * j:512 * j + 512],
                                mkt_sb[:, tt, 512 * j:512 * j + 512],
                            )
                    for h in pair:
                        ps = pss[h]
                        e = expa.tile([128, 1024], bf16, tag="expa",
                                      name=f"e_{h}_{tt}")
                        nc.scalar.activation(e[:, :], ps[:, :], Exp, scale=0.5)
                        for j in range(NJ):
                            et[(h, tt, j)] = e[:, 512 * j:512 * j + 512]
                for h in pair:
                    zt_f = small.tile([64, S], f32, tag="ztf", name=f"ztf_{h}")
                    rec = small.tile([64, S], f32, tag="rec", name=f"rec_{h}")
                    for j in range(NJ):
                        zt = ztp.tile([128, 512], f32, tag="zt", name=f"zt_{h}_{j}")
                        for i, tt in enumerate(range(TT)):
                            nc.tensor.matmul(
                                zt, vp_sb[:, tt, 128 * h:128 * h + 128],
                                et[(h, tt, j)],
                                start=(i == 0), stop=(i == TT - 1),
                            )
                        nc.vector.reciprocal(rec[:, 512 * j:512 * j + 512],
                                             zt[64:128, :])
                        nc.vector.tensor_copy(zt_f[:, 512 * j:512 * j + 512],
                                              zt[0:64, :])
                    zv = zt_f.rearrange("p (m c par) -> par p c m", m=64, c=8, par=2)
                    rv = rec.rearrange("p (m c par) -> par p c m", m=64, c=8, par=2)
                    for P in range(2):
                        nc.vector.tensor_tensor(
                            x2t_sb[64 * P:64 * P + 64, :, 64 * h:64 * h + 64],
                            zv[P], rv[P], op=mult,
                        )

            for tp in range(4):
                osb = outp.tile([128, 1024], f32, tag="osb")
                for eh in range(2):
                    ps = mm.tile([128, 512], f32, tag="mm")
                    for c in range(EC):
                        nc.tensor.matmul(
                            ps, x2t_sb[:, c, 128 * tp:128 * tp + 128],
                            wo_sb[:, c, 512 * eh:512 * eh + 512],
                            start=(c == 0), stop=(c == EC - 1),
                        )
                    nc.vector.tensor_copy(osb[:, 512 * eh:512 * eh + 512], ps)
                nc.sync.dma_start(out=out[tp], in_=osb)
            for p in (outp, small, expa, late):
                p.release()
    nc.compile()
    return nc


def kernel(inputs, mask, wq, bq, wk, bk, wv, bv, wo, bo):
    import ml_dtypes
    from concourse.bass_utils import run_bass_kernel_spmd

    x = np.asarray(inputs, dtype=np.float32)
    wq = np.asarray(wq, dtype=np.float32)
    wk = np.asarray(wk, dtype=np.float32)
    wv = np.asarray(wv, dtype=np.float32)
    wo = np.asarray(wo, dtype=np.float32)
    bq = np.asarray(bq, dtype=np.float32)
    bk = np.asarray(bk, dtype=np.float32)
    mask2d = np.asarray(mask, dtype=np.float32).reshape(S, S)
    causal_ref = 1.0 - np.tril(np.ones((S, S), dtype=np.float32))
    causal = bool(np.array_equal(mask2d, causal_ref))
    variant = "causal" if causal else "generic"
    if variant not in _CACHE:
        _CACHE[variant] = _build_causal() if causal else _build_generic()
    nc = _CACHE[variant]

    in_maps = []
    for c in range(NCORES):
        b, hg = c // 2, c % 2
        sl = slice(512 * hg, 512 * hg + 512)
        if causal:
            f16 = np.float16
            m = {
                "xt": np.ascontiguousarray(
                    x[b].T.reshape(EC, 128, S).transpose(1, 0, 2)).astype(f16),
                "wq": np.ascontiguousarray(
                    wq[:, sl].reshape(EC, 128, 512).transpose(1, 0, 2)).astype(f16),
                "wk": np.ascontiguousarray(
                    wk[:, sl].reshape(EC, 128, 512).transpose(1, 0, 2)).astype(f16),
                "wv": np.ascontiguousarray(
                    wv[:, sl].reshape(EC, 128, 512).transpose(1, 0, 2)).astype(f16),
                "wo": np.ascontiguousarray(
                    wo.reshape(EC, 128, 1024).transpose(1, 0, 2)).astype(f16),
                "bq": np.ascontiguousarray(bq[sl].reshape(4, 128).T),
                "bk": np.ascontiguousarray(bk[sl].reshape(4, 128).T),
                "tri": np.triu(np.ones((128, 128))).astype(ml_dtypes.bfloat16),
            }
        else:
            m = {
                "xt": np.ascontiguousarray(x[b].T.reshape(EC, 128, S).transpose(1, 0, 2)),
                "wq": np.ascontiguousarray(wq[:, sl].reshape(EC, 128, 512).transpose(1, 0, 2)),
                "wk": np.ascontiguousarray(wk[:, sl].reshape(EC, 128, 512).transpose(1, 0, 2)),
                "wv": np.ascontiguousarray(wv[:, sl].reshape(EC, 128, 512).transpose(1, 0, 2)),
                "wo": np.ascontiguousarray(wo.reshape(EC, 128, 1024).transpose(1, 0, 2)),
                "bq": np.ascontiguousarray(bq[sl].reshape(4, 128).T),
                "bk": np.ascontiguousarray(bk[sl].reshape(4, 128).T),
                "mkt": np.ascontiguousarray(
                    (mask2d.T * np.float32(-2e9)).reshape(TT, 128, S).transpose(1, 0, 2)
                ).astype(ml_dtypes.bfloat16),
            }
        in_maps.append(m)

    global _last_in_maps
    _last_in_maps = in_maps
    res = run_bass_kernel_spmd(nc, in_maps, core_ids=list(range(NCORES)))
    full = np.empty((B, S, E), dtype=np.float32)
    for c in range(NCORES):
        b, hg = c // 2, c % 2
        full[b, 512 * hg:512 * hg + 512, :] = np.asarray(
            res.results[c]["out"], dtype=np.float32).reshape(512, 1024)

    # biases bv/bo are zero in this problem; fold in exactly if ever nonzero.
    bv = np.asarray(bv, dtype=np.float32)
    bo = np.asarray(bo, dtype=np.float32)
    if np.any(bv != 0):
        bmat = np.zeros((S, E), dtype=np.float64)
        tpr = np.arange(S)
        e = np.arange(E)
        bmat[:, :] = bv[(64 * (tpr[:, None] // 64) + e[None, :] % 64)]
        full += (bmat @ np.asarray(wo, dtype=np.float64)).astype(np.float32)[None]
    if np.any(bo != 0):
        full += bo[None, None, :]
    return full


# revision 74
# speedup vs baseline: 1.0041x; 1.0013x over previous
"""Trainium2 Bass kernel for MultiHeadAttention (B=4, S=1024, E=1024, H=16, Dh=64).

Sharding: 8 cores = (batch b in 0..3) x (head-group hg in 0..1, 8 heads each).
The reference reshapes [B,H,S,Dh] -> [B,S,E] WITHOUT transposing heads back, so
head h's attention output occupies output rows t' = h*64 + s//16 - the final
projection is row-parallel across head groups: no cross-core communication.

Per-core pipeline (PE-roofline-bound: every matmul at 1.0 cycles/row):
  - inputs host-cast to fp16 (2.4e-4 exactness, half DMA bytes), outputs bf16
    (cast back on host); 2-ec-batched DMA ordered for early PE start
  - QK projections: 4 PSUM accumulators x 4 subphases (wq/wk x s-halves) so
    the 4-bank score pool coexists - no PSUM pool boundary before attention;
    xt streamed chunk-by-chunk; bias+fp16 cast on DVE into per-(dt,sh) tiles
    (fine-grained deps so scores never wait on unrelated copies)
  - V projection (shares the QK PSUM ring) into fp16 V' with 64 ones cols per
    head (one gpsimd memset): the z matmul emits z^T AND softmax denominators
  - scores^T[t,s] = K @ Q^T per (head-pair, segment-packed col group) into
    [128,2,512] PSUM tiles (1 bank/head): ONE activation exps both heads;
    small diagonal blocks share a tile to cut activation count; e tiles bf16
    (range-safe for exp with no max-subtraction); causal triangle = 0/1
    multiply on gpsimd (SBUF-only engine) for diagonal segments only
  - z/normalize: PSUM z + sums; reciprocal(sums)->SBUF then two PSUM*SBUF
    multiplies write the scrambled-reshape layout directly in fp16
  - out = X2 @ wo (fp16), PSUM->SBUF copy on DVE, bf16 DMA out per half
  - software pipelining: z/normalize/output work is queued as fine-grained
    units and drained one per score block so the in-order PE queue always has
    ready work while ACT exps; V-projection fills pair 0; each pair's output
    projection is HELD one pair and released into the next pair's normalize
    window (fills the DVE-bound endgame).

TimelineSim: 98682 ns (baseline 136254), rel err 2.8e-3 (gate 2e-2).
"""
import numpy as np

B, S, E, H, DH = 4, 1024, 1024, 16, 64
NCORES = 8
HPC = 8          # heads per core
EC = 8           # 128-row chunks of E
TT = 8           # 128-row t-tiles of S
NJ = 2           # 512-col s-blocks

_CACHE = {}


def _build_causal():
    import concourse.bacc as bacc
    import concourse.tile as tile
    import concourse.mybir as mybir

    f32 = mybir.dt.float32
    f32r = mybir.dt.float32r
    f16 = mybir.dt.float16
    bf16 = mybir.dt.bfloat16
    Exp = mybir.ActivationFunctionType.Exp
    Copy = mybir.ActivationFunctionType.Copy
    Ident = mybir.ActivationFunctionType.Identity
    Ln = mybir.ActivationFunctionType.Ln
    mult = mybir.AluOpType.mult
    div = mybir.AluOpType.divide

    nc = bacc.Bacc("TRN2")
    xt = nc.dram_tensor("xt", [128, EC, S], f16, kind="ExternalInput")
    wq = nc.dram_tensor("wq", [128, EC, 512], f16, kind="ExternalInput")
    wk = nc.dram_tensor("wk", [128, EC, 512], f16, kind="ExternalInput")
    wv = nc.dram_tensor("wv", [128, EC, 512], f16, kind="ExternalInput")
    wo = nc.dram_tensor("wo", [128, EC, 1024], f16, kind="ExternalInput")
    bq = nc.dram_tensor("bq", [128, 4], f32, kind="ExternalInput")
    bk = nc.dram_tensor("bk", [128, 4], f32, kind="ExternalInput")
    tri = nc.dram_tensor("tri", [128, 128], bf16, kind="ExternalInput")
    out = nc.dram_tensor("out", [4, 128, 1024], bf16, kind="ExternalOutput")

    with tile.TileContext(nc) as tc:
        pp = tc.alloc_tile_pool(name="pp", bufs=1)
        xt_sb = pp.tile([128, EC, S], f16)
        wq_sb = pp.tile([128, EC, 512], f16)
        wk_sb = pp.tile([128, EC, 512], f16)
        wv_sb = pp.tile([128, EC, 512], f16)
        wo_sb = pp.tile([128, EC, 1024], f16)
        qt_sb = [[pp.tile([128, 512], f16, name=f"qt_{d}_{s}")
                  for s in range(2)] for d in range(4)]
        kt_sb = [[pp.tile([128, 512], f16, name=f"kt_{d}_{s}")
                  for s in range(2)] for d in range(4)]
        vp_sb = pp.tile([128, TT, 1024], f16)
        x2t_sb = pp.tile([128, EC, 512], f16)
        bq_sb = pp.tile([128, 4], f32)
        bk_sb = pp.tile([128, 4], f32)
        tri_sb = pp.tile([128, 128], bf16)

        # ---- DMA preload: 2-ec batches (HWDGE/SP fixed costs are per-DMA),
        # ordered so the QK phase can start early ----
        groups = [(0, 1), (1, 3), (3, 5), (5, 7), (7, 8)]
        for a, b in groups:
            nc.sync.dma_start(out=wq_sb[:, a:b, :], in_=wq[:, a:b, :])
            nc.sync.dma_start(out=xt_sb[:, a:b, 0:512], in_=xt[:, a:b, 0:512])
            if a == 1:
                nc.sync.dma_start(out=bq_sb, in_=bq.ap())
        nc.sync.dma_start(out=bk_sb, in_=bk.ap())
        for k in range(0, EC, 2):
            nc.sync.dma_start(out=wk_sb[:, k:k + 2, :], in_=wk[:, k:k + 2, :])
        for k in range(0, EC, 2):
            nc.sync.dma_start(out=xt_sb[:, k:k + 2, 512:1024], in_=xt[:, k:k + 2, 512:1024])
        nc.sync.dma_start(out=tri_sb, in_=tri.ap())
        for k in range(0, EC, 2):
            nc.sync.dma_start(out=wv_sb[:, k:k + 2, :], in_=wv[:, k:k + 2, :])
        for k in range(0, EC, 2):
            nc.sync.dma_start(out=wo_sb[:, k:k + 2, :], in_=wo[:, k:k + 2, :])

        vview = vp_sb.rearrange("p t (h two d) -> p t h two d", two=2, d=DH)
        for tt in range(TT):
            nc.gpsimd.memset(vview[:, tt, :, 1, :], 1.0)

        # ---- Q^T/K^T projections: 4 PSUM accumulators x 4 subphases so
        # the score pool (4 banks) can be allocated upfront: no PSUM pool
        # boundary between projections and attention ----
        scp = tc.alloc_tile_pool(name="scp", bufs=2, space="PSUM")
        qkp = tc.alloc_tile_pool(name="qkp", bufs=4, space="PSUM")
        ep = tc.alloc_tile_pool(name="ep", bufs=16)
        osp = tc.alloc_tile_pool(name="osp", bufs=2)
        rcp = tc.alloc_tile_pool(name="rcp", bufs=4)
        for sh, wi, wsb, dest, bias in (
            (0, 0, wq_sb, qt_sb, bq_sb), (0, 1, wk_sb, kt_sb, bk_sb),
            (1, 0, wq_sb, qt_sb, bq_sb), (1, 1, wk_sb, kt_sb, bk_sb),
        ):
            sl = slice(512 * sh, 512 * sh + 512)
            pss = [qkp.tile([128, 512], f32, tag="qk", name=f"qk_{sh}_{wi}_{d}")
                   for d in range(4)]
            for ec in range(EC):
                for dt_ in range(4):
                    nc.tensor.matmul(
                        pss[dt_],
                        wsb[:, ec, 128 * dt_:128 * dt_ + 128],
                        xt_sb[:, ec, sl],
                        start=(ec == 0), stop=(ec == EC - 1),
                    )
            for dt_ in range(4):
                nc.vector.tensor_scalar_add(
                    out=dest[dt_][sh], in0=pss[dt_],
                    scalar1=bias[:, dt_:dt_ + 1],
                )

        pend = []
        held = []

        def drain(n):
            for _ in range(min(n, len(pend))):
                pend.pop(0)()

        def vjob(tt):
            ps = qkp.tile([128, 512], f32, tag="qk", name=f"v_{tt}")
            for ec in range(EC):
                nc.tensor.matmul(
                    ps, xt_sb[:, ec, 128 * tt:128 * tt + 128], wv_sb[:, ec, :],
                    start=(ec == 0), stop=(ec == EC - 1),
                )
            nc.vector.tensor_copy(
                vview[:, tt, :, 0, :], ps.rearrange("p (h d) -> p h d", d=DH)
            )

        for tt in range(TT):
            pend.append(lambda tt=tt: vjob(tt))

        ztp = opp = None
        for hp in range(4):
            pair = (2 * hp, 2 * hp + 1)
            et_ref = {}
            js = [0, 1]
            blocks = []
            for j in js:
                cur, used = [], 0
                for tt in range(TT):
                    if 512 * j + 511 < 128 * tt:
                        continue
                    w = 512 * j + 512 - max(512 * j, 128 * tt)
                    if used + w > 512:
                        if cur:
                            blocks.append(cur)
                        cur, used = [], 0
                    cur.append((tt, j))
                    used += w
                if cur:
                    blocks.append(cur)

            def sblock(group, hp=hp, pair=pair, et_ref=et_ref):
                # group: list of (tt, j) segments packed into one score tile
                # (the small diagonal blocks share a tile so one activation
                # instruction exps them together)
                segs = []
                col = 0
                for (tt, j) in group:
                    c0 = 128 * tt
                    lo = max(512 * j, c0)
                    w = 512 * j + 512 - lo
                    segs.append((tt, j, c0, lo, w, col))
                    col += w
                total = col
                gname = f"{hp}_{group[0][0]}_{group[0][1]}"
                sc = scp.tile([128, 2, 512], f32, tag="sc", name=f"sc_{gname}")
                for (tt, j, c0, lo, w, col) in segs:
                    kc = c0 % 512
                    for hh, h in enumerate(pair):
                        pb = 64 * (h % 2)
                        nc.tensor.matmul(
                            sc[:, hh, col:col + w],
                            kt_sb[hp][tt // 4][pb:pb + 64, kc:kc + 128],
                            qt_sb[hp][j][pb:pb + 64, lo - 512 * j:512],
                            start=True, stop=True,
                        )
                et = ep.tile([128, 2, 512], bf16, tag="e", name=f"e_{gname}")
                nc.scalar.activation(
                    et[:, :, 0:total], sc[:, :, 0:total], Exp, scale=0.5
                )
                for (tt, j, c0, lo, w, col) in segs:
                    if lo == c0:
                        for hh in range(2):
                            nc.gpsimd.tensor_tensor(
                                et[:, hh, col:col + 128],
                                et[:, hh, col:col + 128], tri_sb, op=mult,
                            )
                    et_ref[(tt, j)] = (et, col, lo - 512 * j, w)

            def push_z(h, hh, j, et_ref=et_ref):
                """Append z-chain units (split mm halves + parallel divides)."""
                box = {}
                tts = [tt for tt in range(TT) if (tt, j) in et_ref]
                halves = ([tts[:4], tts[4:]] if len(tts) > 5 else [tts])

                def mm_unit(sub, first, last, h=h, hh=hh, j=j, box=box,
                            et_ref=et_ref, ntot=len(tts)):
                    if first:
                        box["zt"] = ztp.tile([128, 512], f32, tag="zt",
                                             name=f"zt_{h}_{j}")
                    zt = box["zt"]
                    for i, tt in enumerate(sub):
                        et, col, rel0, w = et_ref[(tt, j)]
                        nc.tensor.matmul(
                            zt[:, rel0:rel0 + w],
                            vp_sb[:, tt, 128 * h:128 * h + 128],
                            et[:, hh, col:col + w],
                            start=(first and i == 0),
                            stop=(last and i == len(sub) - 1),
                        )

                def div_unit(h=h, j=j, hp=hp, box=box):
                    # x2t = z / sums; both live in PSUM and an ALU op may
                    # read only one PSUM operand, so 1/sums goes via SBUF.
                    # For the final pair ACT is idle (no more exps) while DVE
                    # gates the endgame: compute 1/sums as exp(-ln(sums))
                    # there instead.
                    zt = box["zt"]
                    rec = rcp.tile([64, 512], f32, tag="rec",
                                   name=f"rec_{h}_{j}")
                    nc.vector.reciprocal(rec, zt[64:128, :])
                    zv = zt[0:64].rearrange("p (m c par) -> par p c m",
                                            m=32, c=8, par=2)
                    rv = rec.rearrange("p (m c par) -> par p c m",
                                       m=32, c=8, par=2)
                    o0 = 64 * h + 32 * j
                    nc.vector.tensor_tensor(
                        x2t_sb[0:64, :, o0:o0 + 32], zv[0], rv[0], op=mult)
                    nc.vector.tensor_tensor(
                        x2t_sb[64:128, :, o0:o0 + 32], zv[1], rv[1], op=mult)

                for si, sub in enumerate(halves):
                    pend.append(lambda s=sub, f=mm_unit, first=(si == 0),
                                last=(si == len(halves) - 1): f(s, first, last))
                pend.append(div_unit)

            osb_box = {}

            def o_unit(eh, half, hp=hp, osb_box=osb_box):
                if "t" not in osb_box:
                    osb_box["t"] = osp.tile([128, 1024], bf16, tag="osb",
                                            name=f"osb_{hp}")
                t = osb_box["t"]
                if half == 0:
                    osb_box[eh] = opp.tile([128, 512], f32, tag="op",
                                           name=f"op_{hp}_{eh}")
                op_t = osb_box[eh]
                cs = range(0, 4) if half == 0 else range(4, EC)
                for c in cs:
                    nc.tensor.matmul(
                        op_t, x2t_sb[:, c, 128 * hp:128 * hp + 128],
                        wo_sb[:, c, 512 * eh:512 * eh + 512],
                        start=(c == 0), stop=(c == EC - 1),
                    )
                if half == 1:
                    nc.vector.tensor_copy(t[:, 512 * eh:512 * eh + 512], op_t)
                    # final pair: issue eh0's transfer from the (idle)
                    # ACT queue so the last transfer isn't serialized
                    eng = nc.scalar if (eh == 0 and hp == 3) else nc.sync
                    eng.dma_start(out=out[hp, :, 512 * eh:512 * eh + 512],
                                  in_=t[:, 512 * eh:512 * eh + 512])

            nj0 = sum(1 for g in blocks if g[0][1] == js[0])
            for bi, group in enumerate(blocks):
                sblock(group)
                drain(1)
                if bi == nj0 - 1 and hp > 0:
                    # first j-group e-tiles complete: its z can drain now
                    for hh, h in enumerate(pair):
                        push_z(h, hh, js[0])
            if hp == 0:
                while pend:
                    pend.pop(0)()
                qkp.release()
                ztp = tc.alloc_tile_pool(name="ztp", bufs=3, space="PSUM")
                opp = tc.alloc_tile_pool(name="opp", bufs=1, space="PSUM")
                for hh, h in enumerate(pair):
                    push_z(h, hh, js[0])
            for hh, h in enumerate(pair):
                push_z(h, hh, js[1])
                if held:
                    # previous pair's output projection was held back: its
                    # matmuls fill the PE idle window while DVE normalizes
                    pend.append(held.pop(0))
                    pend.append(held.pop(0))
            for eh in range(2):
                for half in range(2):
                    u = (lambda f=o_unit, eh=eh, half=half: f(eh, half))
                    (held if hp < 3 else pend).append(u)

        while pend:
            pend.pop(0)()
        for p_ in (rcp, osp, ep, opp, ztp, scp):
            p_.release()
        pp.release()
    nc.compile()
    return nc


def _build_generic():
    """Fallback for a non-causal mask: the original fp32r kernel."""
    import concourse.bacc as bacc
    import concourse.tile as tile
    import concourse.mybir as mybir

    f32 = mybir.dt.float32
    f32r = mybir.dt.float32r
    bf16 = mybir.dt.bfloat16
    Exp = mybir.ActivationFunctionType.Exp
    mult = mybir.AluOpType.mult

    nc = bacc.Bacc("TRN2")
    xt = nc.dram_tensor("xt", [128, EC, S], f32r, kind="ExternalInput")
    wq = nc.dram_tensor("wq", [128, EC, 512], f32r, kind="ExternalInput")
    wk = nc.dram_tensor("wk", [128, EC, 512], f32r, kind="ExternalInput")
    wv = nc.dram_tensor("wv", [128, EC, 512], f32r, kind="ExternalInput")
    wo = nc.dram_tensor("wo", [128, EC, 1024], f32r, kind="ExternalInput")
    bq = nc.dram_tensor("bq", [128, 4], f32, kind="ExternalInput")
    bk = nc.dram_tensor("bk", [128, 4], f32, kind="ExternalInput")
    mkt = nc.dram_tensor("mkt", [128, TT, S], bf16, kind="ExternalInput")
    out = nc.dram_tensor("out", [4, 128, 1024], f32, kind="ExternalOutput")

    with tile.TileContext(nc) as tc:
        with (
            tc.tile_pool(name="persist", bufs=1) as pp,
            tc.tile_pool(name="mm", bufs=3, space="PSUM") as mm,
            tc.tile_pool(name="ztp", bufs=2, space="PSUM") as ztp,
        ):
            p1 = tc.alloc_tile_pool(name="p1", bufs=1)
            xt_sb = p1.tile([128, EC, S], f32r)
            wq_sb = p1.tile([128, EC, 512], f32r)
            wk_sb = p1.tile([128, EC, 512], f32r)
            wv_sb = p1.tile([128, EC, 512], f32r)
            for k in range(0, EC, 2):
                nc.sync.dma_start(out=xt_sb[:, k:k + 2, :], in_=xt[:, k:k + 2, :])
                nc.sync.dma_start(out=wq_sb[:, k:k + 2, :], in_=wq[:, k:k + 2, :])
                nc.sync.dma_start(out=wk_sb[:, k:k + 2, :], in_=wk[:, k:k + 2, :])
                nc.sync.dma_start(out=wv_sb[:, k:k + 2, :], in_=wv[:, k:k + 2, :])
            qt_sb = pp.tile([128, 4, S], f32r)
            kt_sb = pp.tile([128, 4, S], f32r)
            vp_sb = pp.tile([128, TT, 1024], bf16)
            x2t_sb = pp.tile([128, EC, 512], f32r)
            bq_sb = pp.tile([128, 4], f32)
            bk_sb = pp.tile([128, 4], f32)
            mkt_sb = pp.tile([128, TT, S], bf16)
            nc.sync.dma_start(out=mkt_sb, in_=mkt.ap())
            nc.sync.dma_start(out=bq_sb, in_=bq.ap())
            nc.sync.dma_start(out=bk_sb, in_=bk.ap())

            vview = vp_sb.rearrange("p t (h two d) -> p t h two d", two=2, d=DH)
            ones_sb = pp.tile([128, 512], f32)
            nc.vector.memset(ones_sb, 1.0)
            ones_v = ones_sb.rearrange("p (h d) -> p h d", d=DH)
            for tt in range(TT):
                nc.vector.tensor_copy(vview[:, tt, :, 1, :], ones_v)

            for wsb, dest, bias in ((wq_sb, qt_sb, bq_sb), (wk_sb, kt_sb, bk_sb)):
                for dt_ in range(4):
                    for sh in range(2):
                        ps = mm.tile([128, 512], f32, tag="mm")
                        for ec in range(EC):
                            nc.tensor.matmul(
                                ps, wsb[:, ec, 128 * dt_:128 * dt_ + 128],
                                xt_sb[:, ec, 512 * sh:512 * sh + 512],
                                start=(ec == 0), stop=(ec == EC - 1),
                            )
                        nc.vector.tensor_scalar_add(
                            out=dest[:, dt_, 512 * sh:512 * sh + 512],
                            in0=ps, scalar1=bias[:, dt_:dt_ + 1],
                        )
            for tt in range(TT):
                ps = mm.tile([128, 512], f32, tag="mm")
                for ec in range(EC):
                    nc.tensor.matmul(
                        ps, xt_sb[:, ec, 128 * tt:128 * tt + 128],
                        wv_sb[:, ec, :],
                        start=(ec == 0), stop=(ec == EC - 1),
                    )
                nc.vector.tensor_copy(
                    vview[:, tt, :, 0, :], ps.rearrange("p (h d) -> p h d", d=DH)
                )
            p1.release()
            late = tc.alloc_tile_pool(name="late", bufs=1)
            expa = tc.alloc_tile_pool(name="expa", bufs=16)
            small = tc.alloc_tile_pool(name="small", bufs=2)
            outp = tc.alloc_tile_pool(name="outp", bufs=2)
            wo_sb = late.tile([128, EC, 1024], f32r)
            nc.sync.dma_start(out=wo_sb, in_=wo.ap())

            for hp in range(HPC // 2):
                pair = (2 * hp, 2 * hp + 1)
                et = {}
                for tt in range(TT):
                    pss = {}
                    for h in pair:
                        dt_ = h // 2
                        pb = 64 * (h % 2)
                        ps = mm.tile([128, 1024], f32, tag="mm", name=f"ps_{h}_{tt}")
                        pss[h] = ps
                        for j in range(NJ):
                            nc.tensor.matmul(
                                ps[:, 512 * j:512 * j + 512],
                                kt_sb[pb:pb + 64, dt_, 128 * tt:128 * tt + 128],
                                qt_sb[pb:pb + 64, dt_, 512 * j:512 * j + 512],
                                start=True, stop=True,
                            )
                            nc.vector.tensor_add(
                                ps[:, 512 * j:512 * j + 512],
                                ps[:, 512 * j:512 * j + 512],
                                mkt_sb[:, tt, 512 * j:512 * j + 512],
                            )
                    for h in pair:
                        ps = pss[h]
                        e = expa.tile([128, 1024], bf16, tag="expa",
                                      name=f"e_{h}_{tt}")
                        nc.scalar.activation(e[:, :], ps[:, :], Exp, scale=0.5)
                        for j in range(NJ):
                            et[(h, tt, j)] = e[:, 512 * j:512 * j + 512]
                for h in pair:
                    zt_f = small.tile([64, S], f32, tag="ztf", name=f"ztf_{h}")
                    rec = small.tile([64, S], f32, tag="rec", name=f"rec_{h}")
                    for j in range(NJ):
                        zt = ztp.tile([128, 512], f32, tag="zt", name=f"zt_{h}_{j}")
                        for i, tt in enumerate(range(TT)):
                            nc.tensor.matmul(
                                zt, vp_sb[:, tt, 128 * h:128 * h + 128],
                                et[(h, tt, j)],
                                start=(i == 0), stop=(i == TT - 1),
                            )
                        nc.vector.reciprocal(rec[:, 512 * j:512 * j + 512],
                                             zt[64:128, :])
                        nc.vector.tensor_copy(zt_f[:, 512 * j:512 * j + 512],
                                              zt[0:64, :])
                    zv = zt_f.rearrange("p (m c par) -> par p c m", m=64, c=8, par=2)
                    rv = rec.rearrange("p (m c par) -> par p c m", m=64, c=8, par=2)
                    for P in range(2):
                        nc.vector.tensor_tensor(
                            x2t_sb[64 * P:64 * P + 64, :, 64 * h:64 * h + 64],
                            zv[P], rv[P], op=mult,
                        )

            for tp in range(4):
                osb = outp.tile([128, 1024], f32, tag="osb")
                for eh in range(2):
                    ps = mm.tile([128, 512], f32, tag="mm")
                    for c in range(EC):
                        nc.tensor.matmul(
                            ps, x2t_sb[:, c, 128 * tp:128 * tp + 128],
                            wo_sb[:, c, 512 * eh:512 * eh + 512],
                            start=(c == 0), stop=(c == EC - 1),
                        )
                    nc.vector.tensor_copy(osb[:, 512 * eh:512 * eh + 512], ps)
                nc.sync.dma_start(out=out[tp], in_=osb)
            for p in (outp, small, expa, late):
                p.release()
    nc.compile()
    return nc


def kernel(inputs, mask, wq, bq, wk, bk, wv, bv, wo, bo):
    import ml_dtypes
    from concourse.bass_utils import run_bass_kernel_spmd

    x = np.asarray(inputs, dtype=np.float32)
    wq = np.asarray(wq, dtype=np.float32)
    wk = np.asarray(wk, dtype=np.float32)
    wv = np.asarray(wv, dtype=np.float32)
    wo = np.asarray(wo, dtype=np.float32)
    bq = np.asarray(bq, dtype=np.float32)
    bk = np.asarray(bk, dtype=np.float32)
    mask2d = np.asarray(mask, dtype=np.float32).reshape(S, S)
    causal_ref = 1.0 - np.tril(np.ones((S, S), dtype=np.float32))
    causal = bool(np.array_equal(mask2d, causal_ref))
    variant = "causal" if causal else "generic"
    if variant not in _CACHE:
        _CACHE[variant] = _build_causal() if causal else _build_generic()
    nc = _CACHE[variant]

    in_maps = []
    for c in range(NCORES):
        b, hg = c // 2, c % 2
        sl = slice(512 * hg, 512 * hg + 512)
        if causal:
            f16 = np.float16
            m = {
                "xt": np.ascontiguousarray(
                    x[b].T.reshape(EC, 128, S).transpose(1, 0, 2)).astype(f16),
                "wq": np.ascontiguousarray(
                    wq[:, sl].reshape(EC, 128, 512).transpose(1, 0, 2)).astype(f16),
                "wk": np.ascontiguousarray(
                    wk[:, sl].reshape(EC, 128, 512).transpose(1, 0, 2)).astype(f16),
                "wv": np.ascontiguousarray(
                    wv[:, sl].reshape(EC, 128, 512).transpose(1, 0, 2)).astype(f16),
                "wo": np.ascontiguousarray(
                    wo.reshape(EC, 128, 1024).transpose(1, 0, 2)).astype(f16),
                "bq": np.ascontiguousarray(bq[sl].reshape(4, 128).T),
                "bk": np.ascontiguousarray(bk[sl].reshape(4, 128).T),
                "tri": np.triu(np.ones((128, 128))).astype(ml_dtypes.bfloat16),
            }
        else:
            m = {
                "xt": np.ascontiguousarray(x[b].T.reshape(EC, 128, S).transpose(1, 0, 2)),
                "wq": np.ascontiguousarray(wq[:, sl].reshape(EC, 128, 512).transpose(1, 0, 2)),
                "wk": np.ascontiguousarray(wk[:, sl].reshape(EC, 128, 512).transpose(1, 0, 2)),
                "wv": np.ascontiguousarray(wv[:, sl].reshape(EC, 128, 512).transpose(1, 0, 2)),
                "wo": np.ascontiguousarray(wo.reshape(EC, 128, 1024).transpose(1, 0, 2)),
                "bq": np.ascontiguousarray(bq[sl].reshape(4, 128).T),
                "bk": np.ascontiguousarray(bk[sl].reshape(4, 128).T),
                "mkt": np.ascontiguousarray(
                    (mask2d.T * np.float32(-2e9)).reshape(TT, 128, S).transpose(1, 0, 2)
                ).astype(ml_dtypes.bfloat16),
            }
        in_maps.append(m)

    global _last_in_maps
    _last_in_maps = in_maps
    res = run_bass_kernel_spmd(nc, in_maps, core_ids=list(range(NCORES)))
    full = np.empty((B, S, E), dtype=np.float32)
    for c in range(NCORES):
        b, hg = c // 2, c % 2
        full[b, 512 * hg:512 * hg + 512, :] = np.asarray(
            res.results[c]["out"], dtype=np.float32).reshape(512, 1024)

    # biases bv/bo are zero in this problem; fold in exactly if ever nonzero.
    bv = np.asarray(bv, dtype=np.float32)
    bo = np.asarray(bo, dtype=np.float32)
    if np.any(bv != 0):
        bmat = np.zeros((S, E), dtype=np.float64)
        tpr = np.arange(S)
        e = np.arange(E)
        bmat[:, :] = bv[(64 * (tpr[:, None] // 64) + e[None, :] % 64)]
        full += (bmat @ np.asarray(wo, dtype=np.float64)).astype(np.float32)[None]
    if np.any(bo != 0):
        full += bo[None, None, :]
    return full


# revision 75
# speedup vs baseline: 1.0049x; 1.0008x over previous
"""Trainium2 Bass kernel for MultiHeadAttention (B=4, S=1024, E=1024, H=16, Dh=64).

Sharding: 8 cores = (batch b in 0..3) x (head-group hg in 0..1, 8 heads each).
The reference reshapes [B,H,S,Dh] -> [B,S,E] WITHOUT transposing heads back, so
head h's attention output occupies output rows t' = h*64 + s//16 - the final
projection is row-parallel across head groups: no cross-core communication.

Per-core pipeline (PE-roofline-bound: every matmul at 1.0 cycles/row):
  - inputs host-cast to fp16 (2.4e-4 exactness, half DMA bytes), outputs bf16
    (cast back on host); 2-ec-batched DMA ordered for early PE start
  - QK projections: 4 PSUM accumulators x 4 subphases (wq/wk x s-halves) so
    the 4-bank score pool coexists - no PSUM pool boundary before attention;
    xt streamed chunk-by-chunk; bias+fp16 cast on DVE into per-(dt,sh) tiles
    (fine-grained deps so scores never wait on unrelated copies)
  - V projection (shares the QK PSUM ring) into fp16 V' with 64 ones cols per
    head (one gpsimd memset): the z matmul emits z^T AND softmax denominators
  - scores^T[t,s] = K @ Q^T per (head-pair, segment-packed col group) into
    [128,2,512] PSUM tiles (1 bank/head): ONE activation exps both heads;
    small diagonal blocks share a tile to cut activation count; e tiles bf16
    (range-safe for exp with no max-subtraction); causal triangle = 0/1
    multiply on gpsimd (SBUF-only engine) for diagonal segments only
  - z/normalize: PSUM z + sums; reciprocal(sums)->SBUF then two PSUM*SBUF
    multiplies write the scrambled-reshape layout directly in fp16
  - out = X2 @ wo (fp16), PSUM->SBUF copy on DVE, bf16 DMA out per half
  - software pipelining: z/normalize/output work is queued as fine-grained
    units and drained one per score block so the in-order PE queue always has
    ready work while ACT exps; V-projection fills pair 0; each pair's output
    projection is HELD one pair and released into the next pair's normalize
    window (fills the DVE-bound endgame).

TimelineSim: 98682 ns (baseline 136254), rel err 2.8e-3 (gate 2e-2).
"""
import numpy as np

B, S, E, H, DH = 4, 1024, 1024, 16, 64
NCORES = 8
HPC = 8          # heads per core
EC = 8           # 128-row chunks of E
TT = 8           # 128-row t-tiles of S
NJ = 2           # 512-col s-blocks

_CACHE = {}


def _build_causal():
    import concourse.bacc as bacc
    import concourse.tile as tile
    import concourse.mybir as mybir

    f32 = mybir.dt.float32
    f32r = mybir.dt.float32r
    f16 = mybir.dt.float16
    bf16 = mybir.dt.bfloat16
    Exp = mybir.ActivationFunctionType.Exp
    Copy = mybir.ActivationFunctionType.Copy
    Ident = mybir.ActivationFunctionType.Identity
    Ln = mybir.ActivationFunctionType.Ln
    mult = mybir.AluOpType.mult
    div = mybir.AluOpType.divide

    nc = bacc.Bacc("TRN2")
    xt = nc.dram_tensor("xt", [128, EC, S], f16, kind="ExternalInput")
    wq = nc.dram_tensor("wq", [128, EC, 512], f16, kind="ExternalInput")
    wk = nc.dram_tensor("wk", [128, EC, 512], f16, kind="ExternalInput")
    wv = nc.dram_tensor("wv", [128, EC, 512], f16, kind="ExternalInput")
    wo = nc.dram_tensor("wo", [128, EC, 1024], f16, kind="ExternalInput")
    bq = nc.dram_tensor("bq", [128, 4], f32, kind="ExternalInput")
    bk = nc.dram_tensor("bk", [128, 4], f32, kind="ExternalInput")
    tri = nc.dram_tensor("tri", [128, 128], bf16, kind="ExternalInput")
    out = nc.dram_tensor("out", [4, 128, 1024], bf16, kind="ExternalOutput")

    with tile.TileContext(nc) as tc:
        pp = tc.alloc_tile_pool(name="pp", bufs=1)
        xt_sb = pp.tile([128, EC, S], f16)
        wq_sb = pp.tile([128, EC, 512], f16)
        wk_sb = pp.tile([128, EC, 512], f16)
        wv_sb = pp.tile([128, EC, 512], f16)
        wo_sb = pp.tile([128, EC, 1024], f16)
        qt_sb = [[pp.tile([128, 512], f16, name=f"qt_{d}_{s}")
                  for s in range(2)] for d in range(4)]
        kt_sb = [[pp.tile([128, 512], f16, name=f"kt_{d}_{s}")
                  for s in range(2)] for d in range(4)]
        vp_sb = pp.tile([128, TT, 1024], f16)
        x2t_sb = pp.tile([128, EC, 512], f16)
        bq_sb = pp.tile([128, 4], f32)
        bk_sb = pp.tile([128, 4], f32)
        tri_sb = pp.tile([128, 128], bf16)

        # ---- DMA preload: 2-ec batches (HWDGE/SP fixed costs are per-DMA),
        # ordered so the QK phase can start early ----
        groups = [(0, 1), (1, 3), (3, 5), (5, 7), (7, 8)]
        for a, b in groups:
            nc.sync.dma_start(out=wq_sb[:, a:b, :], in_=wq[:, a:b, :])
            nc.sync.dma_start(out=xt_sb[:, a:b, 0:512], in_=xt[:, a:b, 0:512])
            if a == 1:
                nc.sync.dma_start(out=bq_sb, in_=bq.ap())
        nc.sync.dma_start(out=bk_sb, in_=bk.ap())
        for k in range(0, EC, 2):
            nc.sync.dma_start(out=wk_sb[:, k:k + 2, :], in_=wk[:, k:k + 2, :])
        for k in range(0, EC, 2):
            nc.sync.dma_start(out=xt_sb[:, k:k + 2, 512:1024], in_=xt[:, k:k + 2, 512:1024])
        nc.sync.dma_start(out=tri_sb, in_=tri.ap())
        for k in range(0, EC, 2):
            nc.sync.dma_start(out=wv_sb[:, k:k + 2, :], in_=wv[:, k:k + 2, :])
        for k in range(0, EC, 2):
            nc.sync.dma_start(out=wo_sb[:, k:k + 2, :], in_=wo[:, k:k + 2, :])

        vview = vp_sb.rearrange("p t (h two d) -> p t h two d", two=2, d=DH)
        for tt in range(TT):
            nc.gpsimd.memset(vview[:, tt, :, 1, :], 1.0)

        # ---- Q^T/K^T projections: 4 PSUM accumulators x 4 subphases so
        # the score pool (4 banks) can be allocated upfront: no PSUM pool
        # boundary between projections and attention ----
        scp = tc.alloc_tile_pool(name="scp", bufs=2, space="PSUM")
        qkp = tc.alloc_tile_pool(name="qkp", bufs=4, space="PSUM")
        ep = tc.alloc_tile_pool(name="ep", bufs=24)
        osp = tc.alloc_tile_pool(name="osp", bufs=2)
        rcp = tc.alloc_tile_pool(name="rcp", bufs=3)
        for sh, wi, wsb, dest, bias in (
            (0, 0, wq_sb, qt_sb, bq_sb), (0, 1, wk_sb, kt_sb, bk_sb),
            (1, 0, wq_sb, qt_sb, bq_sb), (1, 1, wk_sb, kt_sb, bk_sb),
        ):
            sl = slice(512 * sh, 512 * sh + 512)
            pss = [qkp.tile([128, 512], f32, tag="qk", name=f"qk_{sh}_{wi}_{d}")
                   for d in range(4)]
            for ec in range(EC):
                for dt_ in range(4):
                    nc.tensor.matmul(
                        pss[dt_],
                        wsb[:, ec, 128 * dt_:128 * dt_ + 128],
                        xt_sb[:, ec, sl],
                        start=(ec == 0), stop=(ec == EC - 1),
                    )
            for dt_ in range(4):
                nc.vector.tensor_scalar_add(
                    out=dest[dt_][sh], in0=pss[dt_],
                    scalar1=bias[:, dt_:dt_ + 1],
                )

        pend = []
        held = []

        def drain(n):
            for _ in range(min(n, len(pend))):
                pend.pop(0)()

        def vjob(tt):
            ps = qkp.tile([128, 512], f32, tag="qk", name=f"v_{tt}")
            for ec in range(EC):
                nc.tensor.matmul(
                    ps, xt_sb[:, ec, 128 * tt:128 * tt + 128], wv_sb[:, ec, :],
                    start=(ec == 0), stop=(ec == EC - 1),
                )
            nc.vector.tensor_copy(
                vview[:, tt, :, 0, :], ps.rearrange("p (h d) -> p h d", d=DH)
            )

        for tt in range(TT):
            pend.append(lambda tt=tt: vjob(tt))

        ztp = opp = None
        for hp in range(4):
            pair = (2 * hp, 2 * hp + 1)
            et_ref = {}
            js = [0, 1]
            blocks = []
            for j in js:
                cur, used = [], 0
                for tt in range(TT):
                    if 512 * j + 511 < 128 * tt:
                        continue
                    w = 512 * j + 512 - max(512 * j, 128 * tt)
                    if used + w > 512:
                        if cur:
                            blocks.append(cur)
                        cur, used = [], 0
                    cur.append((tt, j))
                    used += w
                if cur:
                    blocks.append(cur)

            def sblock(group, hp=hp, pair=pair, et_ref=et_ref):
                # group: list of (tt, j) segments packed into one score tile
                # (the small diagonal blocks share a tile so one activation
                # instruction exps them together)
                segs = []
                col = 0
                for (tt, j) in group:
                    c0 = 128 * tt
                    lo = max(512 * j, c0)
                    w = 512 * j + 512 - lo
                    segs.append((tt, j, c0, lo, w, col))
                    col += w
                total = col
                gname = f"{hp}_{group[0][0]}_{group[0][1]}"
                sc = scp.tile([128, 2, 512], f32, tag="sc", name=f"sc_{gname}")
                for (tt, j, c0, lo, w, col) in segs:
                    kc = c0 % 512
                    for hh, h in enumerate(pair):
                        pb = 64 * (h % 2)
                        nc.tensor.matmul(
                            sc[:, hh, col:col + w],
                            kt_sb[hp][tt // 4][pb:pb + 64, kc:kc + 128],
                            qt_sb[hp][j][pb:pb + 64, lo - 512 * j:512],
                            start=True, stop=True,
                        )
                et = ep.tile([128, 2, 512], bf16, tag="e", name=f"e_{gname}")
                nc.scalar.activation(
                    et[:, :, 0:total], sc[:, :, 0:total], Exp, scale=0.5
                )
                for (tt, j, c0, lo, w, col) in segs:
                    if lo == c0:
                        for hh in range(2):
                            nc.gpsimd.tensor_tensor(
                                et[:, hh, col:col + 128],
                                et[:, hh, col:col + 128], tri_sb, op=mult,
                            )
                    et_ref[(tt, j)] = (et, col, lo - 512 * j, w)

            def push_z(h, hh, j, et_ref=et_ref):
                """Append z-chain units (split mm halves + parallel divides)."""
                box = {}
                tts = [tt for tt in range(TT) if (tt, j) in et_ref]
                halves = ([tts[:4], tts[4:]] if len(tts) > 5 else [tts])

                def mm_unit(sub, first, last, h=h, hh=hh, j=j, box=box,
                            et_ref=et_ref, ntot=len(tts)):
                    if first:
                        box["zt"] = ztp.tile([128, 512], f32, tag="zt",
                                             name=f"zt_{h}_{j}")
                    zt = box["zt"]
                    for i, tt in enumerate(sub):
                        et, col, rel0, w = et_ref[(tt, j)]
                        nc.tensor.matmul(
                            zt[:, rel0:rel0 + w],
                            vp_sb[:, tt, 128 * h:128 * h + 128],
                            et[:, hh, col:col + w],
                            start=(first and i == 0),
                            stop=(last and i == len(sub) - 1),
                        )

                def div_unit(h=h, j=j, hp=hp, box=box):
                    # x2t = z / sums; both live in PSUM and an ALU op may
                    # read only one PSUM operand, so 1/sums goes via SBUF.
                    # For the final pair ACT is idle (no more exps) while DVE
                    # gates the endgame: compute 1/sums as exp(-ln(sums))
                    # there instead.
                    zt = box["zt"]
                    rec = rcp.tile([64, 512], f32, tag="rec",
                                   name=f"rec_{h}_{j}")
                    nc.vector.reciprocal(rec, zt[64:128, :])
                    zv = zt[0:64].rearrange("p (m c par) -> par p c m",
                                            m=32, c=8, par=2)
                    rv = rec.rearrange("p (m c par) -> par p c m",
                                       m=32, c=8, par=2)
                    o0 = 64 * h + 32 * j
                    nc.vector.tensor_tensor(
                        x2t_sb[0:64, :, o0:o0 + 32], zv[0], rv[0], op=mult)
                    nc.vector.tensor_tensor(
                        x2t_sb[64:128, :, o0:o0 + 32], zv[1], rv[1], op=mult)

                for si, sub in enumerate(halves):
                    pend.append(lambda s=sub, f=mm_unit, first=(si == 0),
                                last=(si == len(halves) - 1): f(s, first, last))
                pend.append(div_unit)

            osb_box = {}

            def o_unit(eh, half, hp=hp, osb_box=osb_box):
                if "t" not in osb_box:
                    osb_box["t"] = osp.tile([128, 1024], bf16, tag="osb",
                                            name=f"osb_{hp}")
                t = osb_box["t"]
                if half == 0:
                    osb_box[eh] = opp.tile([128, 512], f32, tag="op",
                                           name=f"op_{hp}_{eh}")
                op_t = osb_box[eh]
                cs = range(0, 4) if half == 0 else range(4, EC)
                for c in cs:
                    nc.tensor.matmul(
                        op_t, x2t_sb[:, c, 128 * hp:128 * hp + 128],
                        wo_sb[:, c, 512 * eh:512 * eh + 512],
                        start=(c == 0), stop=(c == EC - 1),
                    )
                if half == 1:
                    nc.vector.tensor_copy(t[:, 512 * eh:512 * eh + 512], op_t)
                    # final pair: issue eh0's transfer from the (idle)
                    # ACT queue so the last transfer isn't serialized
                    eng = nc.scalar if (eh == 0 and hp == 3) else nc.sync
                    eng.dma_start(out=out[hp, :, 512 * eh:512 * eh + 512],
                                  in_=t[:, 512 * eh:512 * eh + 512])

            nj0 = sum(1 for g in blocks if g[0][1] == js[0])
            for bi, group in enumerate(blocks):
                sblock(group)
                drain(1)
                if bi == nj0 - 1 and hp > 0:
                    # first j-group e-tiles complete: its z can drain now
                    for hh, h in enumerate(pair):
                        push_z(h, hh, js[0])
            if hp == 0:
                while pend:
                    pend.pop(0)()
                qkp.release()
                ztp = tc.alloc_tile_pool(name="ztp", bufs=3, space="PSUM")
                opp = tc.alloc_tile_pool(name="opp", bufs=1, space="PSUM")
                for hh, h in enumerate(pair):
                    push_z(h, hh, js[0])
            for hh, h in enumerate(pair):
                push_z(h, hh, js[1])
                if held:
                    # previous pair's output projection was held back: its
                    # matmuls fill the PE idle window while DVE normalizes
                    pend.append(held.pop(0))
                    pend.append(held.pop(0))
            for eh in range(2):
                for half in range(2):
                    u = (lambda f=o_unit, eh=eh, half=half: f(eh, half))
                    (held if hp < 3 else pend).append(u)

        while pend:
            pend.pop(0)()
        for p_ in (rcp, osp, ep, opp, ztp, scp):
            p_.release()
        pp.release()
    nc.compile()
    return nc


def _build_generic():
    """Fallback for a non-causal mask: the original fp32r kernel."""
    import concourse.bacc as bacc
    import concourse.tile as tile
    import concourse.mybir as mybir

    f32 = mybir.dt.float32
    f32r = mybir.dt.float32r
    bf16 = mybir.dt.bfloat16
    Exp = mybir.ActivationFunctionType.Exp
    mult = mybir.AluOpType.mult

    nc = bacc.Bacc("TRN2")
    xt = nc.dram_tensor("xt", [128, EC, S], f32r, kind="ExternalInput")
    wq = nc.dram_tensor("wq", [128, EC, 512], f32r, kind="ExternalInput")
    wk = nc.dram_tensor("wk", [128, EC, 512], f32r, kind="ExternalInput")
    wv = nc.dram_tensor("wv", [128, EC, 512], f32r, kind="ExternalInput")
    wo = nc.dram_tensor("wo", [128, EC, 1024], f32r, kind="ExternalInput")
    bq = nc.dram_tensor("bq", [128, 4], f32, kind="ExternalInput")
    bk = nc.dram_tensor("bk", [128, 4], f32, kind="ExternalInput")
    mkt = nc.dram_tensor("mkt", [128, TT, S], bf16, kind="ExternalInput")
    out = nc.dram_tensor("out", [4, 128, 1024], f32, kind="ExternalOutput")

    with tile.TileContext(nc) as tc:
        with (
            tc.tile_pool(name="persist", bufs=1) as pp,
            tc.tile_pool(name="mm", bufs=3, space="PSUM") as mm,
            tc.tile_pool(name="ztp", bufs=2, space="PSUM") as ztp,
        ):
            p1 = tc.alloc_tile_pool(name="p1", bufs=1)
            xt_sb = p1.tile([128, EC, S], f32r)
            wq_sb = p1.tile([128, EC, 512], f32r)
            wk_sb = p1.tile([128, EC, 512], f32r)
            wv_sb = p1.tile([128, EC, 512], f32r)
            for k in range(0, EC, 2):
                nc.sync.dma_start(out=xt_sb[:, k:k + 2, :], in_=xt[:, k:k + 2, :])
                nc.sync.dma_start(out=wq_sb[:, k:k + 2, :], in_=wq[:, k:k + 2, :])
                nc.sync.dma_start(out=wk_sb[:, k:k + 2, :], in_=wk[:, k:k + 2, :])
                nc.sync.dma_start(out=wv_sb[:, k:k + 2, :], in_=wv[:, k:k + 2, :])
            qt_sb = pp.tile([128, 4, S], f32r)
            kt_sb = pp.tile([128, 4, S], f32r)
            vp_sb = pp.tile([128, TT, 1024], bf16)
            x2t_sb = pp.tile([128, EC, 512], f32r)
            bq_sb = pp.tile([128, 4], f32)
            bk_sb = pp.tile([128, 4], f32)
            mkt_sb = pp.tile([128, TT, S], bf16)
            nc.sync.dma_start(out=mkt_sb, in_=mkt.ap())
            nc.sync.dma_start(out=bq_sb, in_=bq.ap())
            nc.sync.dma_start(out=bk_sb, in_=bk.ap())

            vview = vp_sb.rearrange("p t (h two d) -> p t h two d", two=2, d=DH)
            ones_sb = pp.tile([128, 512], f32)
            nc.vector.memset(ones_sb, 1.0)
            ones_v = ones_sb.rearrange("p (h d) -> p h d", d=DH)
            for tt in range(TT):
                nc.vector.tensor_copy(vview[:, tt, :, 1, :], ones_v)

            for wsb, dest, bias in ((wq_sb, qt_sb, bq_sb), (wk_sb, kt_sb, bk_sb)):
                for dt_ in range(4):
                    for sh in range(2):
                        ps = mm.tile([128, 512], f32, tag="mm")
                        for ec in range(EC):
                            nc.tensor.matmul(
                                ps, wsb[:, ec, 128 * dt_:128 * dt_ + 128],
                                xt_sb[:, ec, 512 * sh:512 * sh + 512],
                                start=(ec == 0), stop=(ec == EC - 1),
                            )
                        nc.vector.tensor_scalar_add(
                            out=dest[:, dt_, 512 * sh:512 * sh + 512],
                            in0=ps, scalar1=bias[:, dt_:dt_ + 1],
                        )
            for tt in range(TT):
                ps = mm.tile([128, 512], f32, tag="mm")
                for ec in range(EC):
                    nc.tensor.matmul(
                        ps, xt_sb[:, ec, 128 * tt:128 * tt + 128],
                        wv_sb[:, ec, :],
                        start=(ec == 0), stop=(ec == EC - 1),
                    )
                nc.vector.tensor_copy(
                    vview[:, tt, :, 0, :], ps.rearrange("p (h d) -> p h d", d=DH)
                )
            p1.release()
            late = tc.alloc_tile_pool(name="late", bufs=1)
            expa = tc.alloc_tile_pool(name="expa", bufs=16)
            small = tc.alloc_tile_pool(name="small", bufs=2)
            outp = tc.alloc_tile_pool(name="outp", bufs=2)
            wo_sb = late.tile([128, EC, 1024], f32r)
            nc.sync.dma_start(out=wo_sb, in_=wo.ap())

            for hp in range(HPC // 2):
                pair = (2 * hp, 2 * hp + 1)
                et = {}
                for tt in range(TT):
                    pss = {}
                    for h in pair:
                        dt_ = h // 2
                        pb = 64 * (h % 2)
                        ps = mm.tile([128, 1024], f32, tag="mm", name=f"ps_{h}_{tt}")
                        pss[h] = ps
                        for j in range(NJ):
                            nc.tensor.matmul(
                                ps[:, 512 * j:512 * j + 512],
                                kt_sb[pb:pb + 64, dt_, 128 * tt:128 * tt + 128],
                                qt_sb[pb:pb + 64, dt_, 512 * j:512 * j + 512],
                                start=True, stop=True,
                            )
                            nc.vector.tensor_add(
                                ps[:, 512 * j:512 * j + 512],
                                ps[:, 512 * j:512 * j + 512],
                                mkt_sb[:, tt, 512 * j:512 * j + 512],
                            )
                    for h in pair:
                        ps = pss[h]
                        e = expa.tile([128, 1024], bf16, tag="expa",
                                      name=f"e_{h}_{tt}")
                        nc.scalar.activation(e[:, :], ps[:, :], Exp, scale=0.5)
                        for j in range(NJ):
                            et[(h, tt, j)] = e[:, 512 * j:512 * j + 512]
                for h in pair:
                    zt_f = small.tile([64, S], f32, tag="ztf", name=f"ztf_{h}")
                    rec = small.tile([64, S], f32, tag="rec", name=f"rec_{h}")
                    for j in range(NJ):
                        zt = ztp.tile([128, 512], f32, tag="zt", name=f"zt_{h}_{j}")
                        for i, tt in enumerate(range(TT)):
                            nc.tensor.matmul(
                                zt, vp_sb[:, tt, 128 * h:128 * h + 128],
                                et[(h, tt, j)],
                                start=(i == 0), stop=(i == TT - 1),
                            )
                        nc.vector.reciprocal(rec[:, 512 * j:512 * j + 512],
                                             zt[64:128, :])
                        nc.vector.tensor_copy(zt_f[:, 512 * j:512 * j + 512],
                                              zt[0:64, :])
                    zv = zt_f.rearrange("p (m c par) -> par p c m", m=64, c=8, par=2)
                    rv = rec.rearrange("p (m c par) -> par p c m", m=64, c=8, par=2)
                    for P in range(2):
                        nc.vector.tensor_tensor(
                            x2t_sb[64 * P:64 * P + 64, :, 64 * h:64 * h + 64],
                            zv[P], rv[P], op=mult,
                        )

            for tp in range(4):
                osb = outp.tile([128, 1024], f32, tag="osb")
                for eh in range(2):
                    ps = mm.tile([128, 512], f32, tag="mm")
                    for c in range(EC):
                        nc.tensor.matmul(
                            ps, x2t_sb[:, c, 128 * tp:128 * tp + 128],
                            wo_sb[:, c, 512 * eh:512 * eh + 512],
                            start=(c == 0), stop=(c == EC - 1),
                        )
                    nc.vector.tensor_copy(osb[:, 512 * eh:512 * eh + 512], ps)
                nc.sync.dma_start(out=out[tp], in_=osb)
            for p in (outp, small, expa, late):
                p.release()
    nc.compile()
    return nc


def kernel(inputs, mask, wq, bq, wk, bk, wv, bv, wo, bo):
    import ml_dtypes
    from concourse.bass_utils import run_bass_kernel_spmd

    x = np.asarray(inputs, dtype=np.float32)
    wq = np.asarray(wq, dtype=np.float32)
    wk = np.asarray(wk, dtype=np.float32)
    wv = np.asarray(wv, dtype=np.float32)
    wo = np.asarray(wo, dtype=np.float32)
    bq = np.asarray(bq, dtype=np.float32)
    bk = np.asarray(bk, dtype=np.float32)
    mask2d = np.asarray(mask, dtype=np.float32).reshape(S, S)
    causal_ref = 1.0 - np.tril(np.ones((S, S), dtype=np.float32))
    causal = bool(np.array_equal(mask2d, causal_ref))
    variant = "causal" if causal else "generic"
    if variant not in _CACHE:
        _CACHE[variant] = _build_causal() if causal else _build_generic()
    nc = _CACHE[variant]

    in_maps = []
    for c in range(NCORES):
        b, hg = c // 2, c % 2
        sl = slice(512 * hg, 512 * hg + 512)
        if causal:
            f16 = np.float16
            m = {
                "xt": np.ascontiguousarray(
                    x[b].T.reshape(EC, 128, S).transpose(1, 0, 2)).astype(f16),
                "wq": np.ascontiguousarray(
                    wq[:, sl].reshape(EC, 128, 512).transpose(1, 0, 2)).astype(f16),
                "wk": np.ascontiguousarray(
                    wk[:, sl].reshape(EC, 128, 512).transpose(1, 0, 2)).astype(f16),
                "wv": np.ascontiguousarray(
                    wv[:, sl].reshape(EC, 128, 512).transpose(1, 0, 2)).astype(f16),
                "wo": np.ascontiguousarray(
                    wo.reshape(EC, 128, 1024).transpose(1, 0, 2)).astype(f16),
                "bq": np.ascontiguousarray(bq[sl].reshape(4, 128).T),
                "bk": np.ascontiguousarray(bk[sl].reshape(4, 128).T),
                "tri": np.triu(np.ones((128, 128))).astype(ml_dtypes.bfloat16),
            }
        else:
            m = {
                "xt": np.ascontiguousarray(x[b].T.reshape(EC, 128, S).transpose(1, 0, 2)),
                "wq": np.ascontiguousarray(wq[:, sl].reshape(EC, 128, 512).transpose(1, 0, 2)),
                "wk": np.ascontiguousarray(wk[:, sl].reshape(EC, 128, 512).transpose(1, 0, 2)),
                "wv": np.ascontiguousarray(wv[:, sl].reshape(EC, 128, 512).transpose(1, 0, 2)),
                "wo": np.ascontiguousarray(wo.reshape(EC, 128, 1024).transpose(1, 0, 2)),
                "bq": np.ascontiguousarray(bq[sl].reshape(4, 128).T),
                "bk": np.ascontiguousarray(bk[sl].reshape(4, 128).T),
                "mkt": np.ascontiguousarray(
                    (mask2d.T * np.float32(-2e9)).reshape(TT, 128, S).transpose(1, 0, 2)
                ).astype(ml_dtypes.bfloat16),
            }
        in_maps.append(m)

    global _last_in_maps
    _last_in_maps = in_maps
    res = run_bass_kernel_spmd(nc, in_maps, core_ids=list(range(NCORES)))
    full = np.empty((B, S, E), dtype=np.float32)
    for c in range(NCORES):
        b, hg = c // 2, c % 2
        full[b, 512 * hg:512 * hg + 512, :] = np.asarray(
            res.results[c]["out"], dtype=np.float32).reshape(512, 1024)

    # biases bv/bo are zero in this problem; fold in exactly if ever nonzero.
    bv = np.asarray(bv, dtype=np.float32)
    bo = np.asarray(bo, dtype=np.float32)
    if np.any(bv != 0):
        bmat = np.zeros((S, E), dtype=np.float64)
        tpr = np.arange(S)
        e = np.arange(E)
        bmat[:, :] = bv[(64 * (tpr[:, None] // 64) + e[None, :] % 64)]
        full += (bmat @ np.asarray(wo, dtype=np.float64)).astype(np.float32)[None]
    if np.any(bo != 0):
        full += bo[None, None, :]
    return full


# revision 78
# speedup vs baseline: 1.0085x; 1.0036x over previous
"""Trainium2 Bass kernel for MultiHeadAttention (B=4, S=1024, E=1024, H=16, Dh=64).

Sharding: 8 cores = (batch b in 0..3) x (head-group hg in 0..1, 8 heads each).
The reference reshapes [B,H,S,Dh] -> [B,S,E] WITHOUT transposing heads back, so
head h's attention output occupies output rows t' = h*64 + s//16 - the final
projection is row-parallel across head groups: no cross-core communication.

Per-core pipeline (PE-roofline-bound: every matmul at 1.0 cycles/row):
  - inputs host-cast to fp16 (2.4e-4 exactness, half DMA bytes), outputs bf16
    (cast back on host); 2-ec-batched DMA ordered for early PE start
  - QK projections: 4 PSUM accumulators x 4 subphases (wq/wk x s-halves) so
    the 4-bank score pool coexists - no PSUM pool boundary before attention;
    xt streamed chunk-by-chunk; bias+fp16 cast on DVE into per-(dt,sh) tiles
    (fine-grained deps so scores never wait on unrelated copies)
  - V projection (shares the QK PSUM ring) into fp16 V' with 64 ones cols per
    head (one gpsimd memset): the z matmul emits z^T AND softmax denominators
  - scores^T[t,s] = K @ Q^T per (head-pair, segment-packed col group) into
    [128,2,512] PSUM tiles (1 bank/head): ONE activation exps both heads;
    small diagonal blocks share a tile to cut activation count; e tiles bf16
    (range-safe for exp with no max-subtraction); causal triangle = 0/1
    multiply on gpsimd (SBUF-only engine) for diagonal segments only
  - z/normalize: PSUM z + sums; reciprocal(sums)->SBUF then two PSUM*SBUF
    multiplies write the scrambled-reshape layout directly in fp16
  - out = X2 @ wo (fp16), PSUM->SBUF copy on DVE, bf16 DMA out per half
  - software pipelining: z/normalize/output work is queued as fine-grained
    units and drained one per score block so the in-order PE queue always has
    ready work while ACT exps; V-projection fills pair 0; each pair's output
    projection is HELD one pair and released into the next pair's normalize
    window (fills the DVE-bound endgame).

TimelineSim: 98682 ns (baseline 136254), rel err 2.8e-3 (gate 2e-2).
"""
import numpy as np

B, S, E, H, DH = 4, 1024, 1024, 16, 64
NCORES = 8
HPC = 8          # heads per core
EC = 8           # 128-row chunks of E
TT = 8           # 128-row t-tiles of S
NJ = 2           # 512-col s-blocks

_CACHE = {}


def _build_causal():
    import concourse.bacc as bacc
    import concourse.tile as tile
    import concourse.mybir as mybir

    f32 = mybir.dt.float32
    f32r = mybir.dt.float32r
    f16 = mybir.dt.float16
    bf16 = mybir.dt.bfloat16
    Exp = mybir.ActivationFunctionType.Exp
    Copy = mybir.ActivationFunctionType.Copy
    Ident = mybir.ActivationFunctionType.Identity
    Ln = mybir.ActivationFunctionType.Ln
    mult = mybir.AluOpType.mult
    div = mybir.AluOpType.divide

    nc = bacc.Bacc("TRN2")
    xt = nc.dram_tensor("xt", [128, EC, S], f16, kind="ExternalInput")
    wq = nc.dram_tensor("wq", [128, EC, 512], f16, kind="ExternalInput")
    wk = nc.dram_tensor("wk", [128, EC, 512], f16, kind="ExternalInput")
    wv = nc.dram_tensor("wv", [128, EC, 512], f16, kind="ExternalInput")
    wo = nc.dram_tensor("wo", [128, EC, 1024], f16, kind="ExternalInput")
    bq = nc.dram_tensor("bq", [128, 4], f32, kind="ExternalInput")
    bk = nc.dram_tensor("bk", [128, 4], f32, kind="ExternalInput")
    tri = nc.dram_tensor("tri", [128, 128], bf16, kind="ExternalInput")
    out = nc.dram_tensor("out", [4, 128, 1024], bf16, kind="ExternalOutput")

    with tile.TileContext(nc) as tc:
        pp = tc.alloc_tile_pool(name="pp", bufs=1)
        xt_sb = pp.tile([128, EC, S], f16)
        wq_sb = pp.tile([128, EC, 512], f16)
        wk_sb = pp.tile([128, EC, 512], f16)
        wv_sb = pp.tile([128, EC, 512], f16)
        wo_sb = pp.tile([128, EC, 1024], f16)
        qt_sb = [[pp.tile([128, 512], f16, name=f"qt_{d}_{s}")
                  for s in range(2)] for d in range(4)]
        kt_sb = [[pp.tile([128, 512], f16, name=f"kt_{d}_{s}")
                  for s in range(2)] for d in range(4)]
        vp_sb = pp.tile([128, TT, 1024], f16)
        x2t_sb = pp.tile([128, EC, 512], f16)
        bq_sb = pp.tile([128, 4], f32)
        bk_sb = pp.tile([128, 4], f32)
        tri_sb = pp.tile([128, 128], bf16)

        # ---- DMA preload: 2-ec batches (HWDGE/SP fixed costs are per-DMA),
        # ordered so the QK phase can start early ----
        groups = [(0, 1), (1, 3), (3, 5), (5, 7), (7, 8)]
        for a, b in groups:
            nc.sync.dma_start(out=wq_sb[:, a:b, :], in_=wq[:, a:b, :])
            nc.sync.dma_start(out=xt_sb[:, a:b, 0:512], in_=xt[:, a:b, 0:512])
            if a == 1:
                nc.sync.dma_start(out=bq_sb, in_=bq.ap())
        nc.sync.dma_start(out=bk_sb, in_=bk.ap())
        for k in range(0, EC, 2):
            nc.sync.dma_start(out=wk_sb[:, k:k + 2, :], in_=wk[:, k:k + 2, :])
        for k in range(0, EC, 2):
            nc.sync.dma_start(out=xt_sb[:, k:k + 2, 512:1024], in_=xt[:, k:k + 2, 512:1024])
        nc.sync.dma_start(out=tri_sb, in_=tri.ap())
        for k in range(0, EC, 2):
            nc.sync.dma_start(out=wv_sb[:, k:k + 2, :], in_=wv[:, k:k + 2, :])
        for k in range(0, EC, 2):
            nc.sync.dma_start(out=wo_sb[:, k:k + 2, :], in_=wo[:, k:k + 2, :])

        vview = vp_sb.rearrange("p t (h two d) -> p t h two d", two=2, d=DH)
        for tt in range(TT):
            nc.gpsimd.memset(vview[:, tt, :, 1, :], 1.0)

        # ---- Q^T/K^T projections: 4 PSUM accumulators x 4 subphases so
        # the score pool (4 banks) can be allocated upfront: no PSUM pool
        # boundary between projections and attention ----
        scp = tc.alloc_tile_pool(name="scp", bufs=2, space="PSUM")
        qkp = tc.alloc_tile_pool(name="qkp", bufs=4, space="PSUM")
        ep = tc.alloc_tile_pool(name="ep", bufs=24)
        osp = tc.alloc_tile_pool(name="osp", bufs=2)
        rcp = tc.alloc_tile_pool(name="rcp", bufs=3)
        for sh, wi, wsb, dest, bias in (
            (0, 0, wq_sb, qt_sb, bq_sb), (0, 1, wk_sb, kt_sb, bk_sb),
            (1, 0, wq_sb, qt_sb, bq_sb), (1, 1, wk_sb, kt_sb, bk_sb),
        ):
            sl = slice(512 * sh, 512 * sh + 512)
            pss = [qkp.tile([128, 512], f32, tag="qk", name=f"qk_{sh}_{wi}_{d}")
                   for d in range(4)]
            for ec in range(EC):
                for dt_ in range(4):
                    nc.tensor.matmul(
                        pss[dt_],
                        wsb[:, ec, 128 * dt_:128 * dt_ + 128],
                        xt_sb[:, ec, sl],
                        start=(ec == 0), stop=(ec == EC - 1),
                    )
            for dt_ in range(4):
                nc.vector.tensor_scalar_add(
                    out=dest[dt_][sh], in0=pss[dt_],
                    scalar1=bias[:, dt_:dt_ + 1],
                )

        pend = []
        held = []

        def drain(n):
            for _ in range(min(n, len(pend))):
                pend.pop(0)()

        def vjob(tt):
            ps = qkp.tile([128, 512], f32, tag="qk", name=f"v_{tt}")
            for ec in range(EC):
                nc.tensor.matmul(
                    ps, xt_sb[:, ec, 128 * tt:128 * tt + 128], wv_sb[:, ec, :],
                    start=(ec == 0), stop=(ec == EC - 1),
                )
            nc.vector.tensor_copy(
                vview[:, tt, :, 0, :], ps.rearrange("p (h d) -> p h d", d=DH)
            )

        for tt in range(TT):
            pend.append(lambda tt=tt: vjob(tt))

        ztp = opp = None
        for hp in range(4):
            pair = (2 * hp, 2 * hp + 1)
            et_ref = {}
            js = [1, 0] if hp in (1, 2) else [0, 1]
            blocks = []
            for j in js:
                cur, used = [], 0
                for tt in range(TT):
                    if 512 * j + 511 < 128 * tt:
                        continue
                    w = 512 * j + 512 - max(512 * j, 128 * tt)
                    if used + w > 512:
                        if cur:
                            blocks.append(cur)
                        cur, used = [], 0
                    cur.append((tt, j))
                    used += w
                if cur:
                    blocks.append(cur)

            def sblock(group, hp=hp, pair=pair, et_ref=et_ref):
                # group: list of (tt, j) segments packed into one score tile
                # (the small diagonal blocks share a tile so one activation
                # instruction exps them together)
                segs = []
                col = 0
                for (tt, j) in group:
                    c0 = 128 * tt
                    lo = max(512 * j, c0)
                    w = 512 * j + 512 - lo
                    segs.append((tt, j, c0, lo, w, col))
                    col += w
                total = col
                gname = f"{hp}_{group[0][0]}_{group[0][1]}"
                sc = scp.tile([128, 2, 512], f32, tag="sc", name=f"sc_{gname}")
                for (tt, j, c0, lo, w, col) in segs:
                    kc = c0 % 512
                    for hh, h in enumerate(pair):
                        pb = 64 * (h % 2)
                        nc.tensor.matmul(
                            sc[:, hh, col:col + w],
                            kt_sb[hp][tt // 4][pb:pb + 64, kc:kc + 128],
                            qt_sb[hp][j][pb:pb + 64, lo - 512 * j:512],
                            start=True, stop=True,
                        )
                et = ep.tile([128, 2, 512], bf16, tag="e", name=f"e_{gname}")
                nc.scalar.activation(
                    et[:, :, 0:total], sc[:, :, 0:total], Exp, scale=0.5
                )
                for (tt, j, c0, lo, w, col) in segs:
                    if lo == c0:
                        for hh in range(2):
                            nc.gpsimd.tensor_tensor(
                                et[:, hh, col:col + 128],
                                et[:, hh, col:col + 128], tri_sb, op=mult,
                            )
                    et_ref[(tt, j)] = (et, col, lo - 512 * j, w)

            def push_z(h, hh, j, et_ref=et_ref):
                """Append z-chain units (split mm halves + parallel divides)."""
                box = {}
                tts = [tt for tt in range(TT) if (tt, j) in et_ref]
                halves = [tts]

                def mm_unit(sub, first, last, h=h, hh=hh, j=j, box=box,
                            et_ref=et_ref, ntot=len(tts)):
                    if first:
                        box["zt"] = ztp.tile([128, 512], f32, tag="zt",
                                             name=f"zt_{h}_{j}")
                    zt = box["zt"]
                    for i, tt in enumerate(sub):
                        et, col, rel0, w = et_ref[(tt, j)]
                        nc.tensor.matmul(
                            zt[:, rel0:rel0 + w],
                            vp_sb[:, tt, 128 * h:128 * h + 128],
                            et[:, hh, col:col + w],
                            start=(first and i == 0),
                            stop=(last and i == len(sub) - 1),
                        )

                def div_unit(h=h, j=j, hp=hp, box=box):
                    # x2t = z / sums; both live in PSUM and an ALU op may
                    # read only one PSUM operand, so 1/sums goes via SBUF.
                    # For the final pair ACT is idle (no more exps) while DVE
                    # gates the endgame: compute 1/sums as exp(-ln(sums))
                    # there instead.
                    zt = box["zt"]
                    rec = rcp.tile([64, 512], f32, tag="rec",
                                   name=f"rec_{h}_{j}")
                    nc.vector.reciprocal(rec, zt[64:128, :])
                    zv = zt[0:64].rearrange("p (m c par) -> par p c m",
                                            m=32, c=8, par=2)
                    rv = rec.rearrange("p (m c par) -> par p c m",
                                       m=32, c=8, par=2)
                    o0 = 64 * h + 32 * j
                    nc.vector.tensor_tensor(
                        x2t_sb[0:64, :, o0:o0 + 32], zv[0], rv[0], op=mult)
                    nc.vector.tensor_tensor(
                        x2t_sb[64:128, :, o0:o0 + 32], zv[1], rv[1], op=mult)

                for si, sub in enumerate(halves):
                    pend.append(lambda s=sub, f=mm_unit, first=(si == 0),
                                last=(si == len(halves) - 1): f(s, first, last))
                pend.append(div_unit)

            osb_box = {}

            def o_unit(eh, half, hp=hp, osb_box=osb_box):
                if "t" not in osb_box:
                    osb_box["t"] = osp.tile([128, 1024], bf16, tag="osb",
                                            name=f"osb_{hp}")
                t = osb_box["t"]
                if half == 0:
                    osb_box[eh] = opp.tile([128, 512], f32, tag="op",
                                           name=f"op_{hp}_{eh}")
                op_t = osb_box[eh]
                cs = range(0, 4) if half == 0 else range(4, EC)
                for c in cs:
                    nc.tensor.matmul(
                        op_t, x2t_sb[:, c, 128 * hp:128 * hp + 128],
                        wo_sb[:, c, 512 * eh:512 * eh + 512],
                        start=(c == 0), stop=(c == EC - 1),
                    )
                if half == 1:
                    nc.vector.tensor_copy(t[:, 512 * eh:512 * eh + 512], op_t)
                    # final pair: issue eh0's transfer from the (idle)
                    # ACT queue so the last transfer isn't serialized
                    eng = nc.scalar if (eh == 0 and hp == 3) else nc.sync
                    eng.dma_start(out=out[hp, :, 512 * eh:512 * eh + 512],
                                  in_=t[:, 512 * eh:512 * eh + 512])

            nj0 = sum(1 for g in blocks if g[0][1] == js[0])
            for bi, group in enumerate(blocks):
                sblock(group)
                drain(1)
                if bi == nj0 - 1 and hp > 0:
                    # first j-group e-tiles complete: its z can drain now
                    for hh, h in enumerate(pair):
                        push_z(h, hh, js[0])
            if hp == 0:
                while pend:
                    pend.pop(0)()
                qkp.release()
                ztp = tc.alloc_tile_pool(name="ztp", bufs=3, space="PSUM")
                opp = tc.alloc_tile_pool(name="opp", bufs=1, space="PSUM")
                for hh, h in enumerate(pair):
                    push_z(h, hh, js[0])
            for hh, h in enumerate(pair):
                push_z(h, hh, js[1])
                if held:
                    # previous pair's output projection was held back: its
                    # matmuls fill the PE idle window while DVE normalizes
                    pend.append(held.pop(0))
                    pend.append(held.pop(0))
            for eh in range(2):
                for half in range(2):
                    u = (lambda f=o_unit, eh=eh, half=half: f(eh, half))
                    (held if hp < 3 else pend).append(u)

        while pend:
            pend.pop(0)()
        for p_ in (rcp, osp, ep, opp, ztp, scp):
            p_.release()
        pp.release()
    nc.compile()
    return nc


def _build_generic():
    """Fallback for a non-causal mask: the original fp32r kernel."""
    import concourse.bacc as bacc
    import concourse.tile as tile
    import concourse.mybir as mybir

    f32 = mybir.dt.float32
    f32r = mybir.dt.float32r
    bf16 = mybir.dt.bfloat16
    Exp = mybir.ActivationFunctionType.Exp
    mult = mybir.AluOpType.mult

    nc = bacc.Bacc("TRN2")
    xt = nc.dram_tensor("xt", [128, EC, S], f32r, kind="ExternalInput")
    wq = nc.dram_tensor("wq", [128, EC, 512], f32r, kind="ExternalInput")
    wk = nc.dram_tensor("wk", [128, EC, 512], f32r, kind="ExternalInput")
    wv = nc.dram_tensor("wv", [128, EC, 512], f32r, kind="ExternalInput")
    wo = nc.dram_tensor("wo", [128, EC, 1024], f32r, kind="ExternalInput")
    bq = nc.dram_tensor("bq", [128, 4], f32, kind="ExternalInput")
    bk = nc.dram_tensor("bk", [128, 4], f32, kind="ExternalInput")
    mkt = nc.dram_tensor("mkt", [128, TT, S], bf16, kind="ExternalInput")
    out = nc.dram_tensor("out", [4, 128, 1024], f32, kind="ExternalOutput")

    with tile.TileContext(nc) as tc:
        with (
            tc.tile_pool(name="persist", bufs=1) as pp,
            tc.tile_pool(name="mm", bufs=3, space="PSUM") as mm,
            tc.tile_pool(name="ztp", bufs=2, space="PSUM") as ztp,
        ):
            p1 = tc.alloc_tile_pool(name="p1", bufs=1)
            xt_sb = p1.tile([128, EC, S], f32r)
            wq_sb = p1.tile([128, EC, 512], f32r)
            wk_sb = p1.tile([128, EC, 512], f32r)
            wv_sb = p1.tile([128, EC, 512], f32r)
            for k in range(0, EC, 2):
                nc.sync.dma_start(out=xt_sb[:, k:k + 2, :], in_=xt[:, k:k + 2, :])
                nc.sync.dma_start(out=wq_sb[:, k:k + 2, :], in_=wq[:, k:k + 2, :])
                nc.sync.dma_start(out=wk_sb[:, k:k + 2, :], in_=wk[:, k:k + 2, :])
                nc.sync.dma_start(out=wv_sb[:, k:k + 2, :], in_=wv[:, k:k + 2, :])
            qt_sb = pp.tile([128, 4, S], f32r)
            kt_sb = pp.tile([128, 4, S], f32r)
            vp_sb = pp.tile([128, TT, 1024], bf16)
            x2t_sb = pp.tile([128, EC, 512], f32r)
            bq_sb = pp.tile([128, 4], f32)
            bk_sb = pp.tile([128, 4], f32)
            mkt_sb = pp.tile([128, TT, S], bf16)
            nc.sync.dma_start(out=mkt_sb, in_=mkt.ap())
            nc.sync.dma_start(out=bq_sb, in_=bq.ap())
            nc.sync.dma_start(out=bk_sb, in_=bk.ap())

            vview = vp_sb.rearrange("p t (h two d) -> p t h two d", two=2, d=DH)
            ones_sb = pp.tile([128, 512], f32)
            nc.vector.memset(ones_sb, 1.0)
            ones_v = ones_sb.rearrange("p (h d) -> p h d", d=DH)
            for tt in range(TT):
                nc.vector.tensor_copy(vview[:, tt, :, 1, :], ones_v)

            for wsb, dest, bias in ((wq_sb, qt_sb, bq_sb), (wk_sb, kt_sb, bk_sb)):
                for dt_ in range(4):
                    for sh in range(2):
                        ps = mm.tile([128, 512], f32, tag="mm")
                        for ec in range(EC):
                            nc.tensor.matmul(
                                ps, wsb[:, ec, 128 * dt_:128 * dt_ + 128],
                                xt_sb[:, ec, 512 * sh:512 * sh + 512],
                                start=(ec == 0), stop=(ec == EC - 1),
                            )
                        nc.vector.tensor_scalar_add(
                            out=dest[:, dt_, 512 * sh:512 * sh + 512],
                            in0=ps, scalar1=bias[:, dt_:dt_ + 1],
                        )
            for tt in range(TT):
                ps = mm.tile([128, 512], f32, tag="mm")
                for ec in range(EC):
                    nc.tensor.matmul(
                        ps, xt_sb[:, ec, 128 * tt:128 * tt + 128],
                        wv_sb[:, ec, :],
                        start=(ec == 0), stop=(ec == EC - 1),
                    )
                nc.vector.tensor_copy(
                    vview[:, tt, :, 0, :], ps.rearrange("p (h d) -> p h d", d=DH)
                )
            p1.release()
            late = tc.alloc_tile_pool(name="late", bufs=1)
            expa = tc.alloc_tile_pool(name="expa", bufs=16)
            small = tc.alloc_tile_pool(name="small", bufs=2)
            outp = tc.alloc_tile_pool(name="outp", bufs=2)
            wo_sb = late.tile([128, EC, 1024], f32r)
            nc.sync.dma_start(out=wo_sb, in_=wo.ap())

            for hp in range(HPC // 2):
                pair = (2 * hp, 2 * hp + 1)
                et = {}
                for tt in range(TT):
                    pss = {}
                    for h in pair:
                        dt_ = h // 2
                        pb = 64 * (h % 2)
                        ps = mm.tile([128, 1024], f32, tag="mm", name=f"ps_{h}_{tt}")
                        pss[h] = ps
                        for j in range(NJ):
                            nc.tensor.matmul(
                                ps[:, 512 * j:512 * j + 512],
                                kt_sb[pb:pb + 64, dt_, 128 * tt:128 * tt + 128],
                                qt_sb[pb:pb + 64, dt_, 512 * j:512 * j + 512],
                                start=True, stop=True,
                            )
                            nc.vector.tensor_add(
                                ps[:, 512 * j:512 * j + 512],
                                ps[:, 512 * j:512 * j + 512],
                                mkt_sb[:, tt, 512 * j:512 * j + 512],
                            )
                    for h in pair:
                        ps = pss[h]
                        e = expa.tile([128, 1024], bf16, tag="expa",
                                      name=f"e_{h}_{tt}")
                        nc.scalar.activation(e[:, :], ps[:, :], Exp, scale=0.5)
                        for j in range(NJ):
                            et[(h, tt, j)] = e[:, 512 * j:512 * j + 512]
                for h in pair:
                    zt_f = small.tile([64, S], f32, tag="ztf", name=f"ztf_{h}")
                    rec = small.tile([64, S], f32, tag="rec", name=f"rec_{h}")
                    for j in range(NJ):
                        zt = ztp.tile([128, 512], f32, tag="zt", name=f"zt_{h}_{j}")
                        for i, tt in enumerate(range(TT)):
                            nc.tensor.matmul(
                                zt, vp_sb[:, tt, 128 * h:128 * h + 128],
                                et[(h, tt, j)],
                                start=(i == 0), stop=(i == TT - 1),
                            )
                        nc.vector.reciprocal(rec[:, 512 * j:512 * j + 512],
                                             zt[64:128, :])
                        nc.vector.tensor_copy(zt_f[:, 512 * j:512 * j + 512],
                                              zt[0:64, :])
                    zv = zt_f.rearrange("p (m c par) -> par p c m", m=64, c=8, par=2)
                    rv = rec.rearrange("p (m c par) -> par p c m", m=64, c=8, par=2)
                    for P in range(2):
                        nc.vector.tensor_tensor(
                            x2t_sb[64 * P:64 * P + 64, :, 64 * h:64 * h + 64],
                            zv[P], rv[P], op=mult,
                        )

            for tp in range(4):
                osb = outp.tile([128, 1024], f32, tag="osb")
                for eh in range(2):
                    ps = mm.tile([128, 512], f32, tag="mm")
                    for c in range(EC):
                        nc.tensor.matmul(
                            ps, x2t_sb[:, c, 128 * tp:128 * tp + 128],
                            wo_sb[:, c, 512 * eh:512 * eh + 512],
                            start=(c == 0), stop=(c == EC - 1),
                        )
                    nc.vector.tensor_copy(osb[:, 512 * eh:512 * eh + 512], ps)
                nc.sync.dma_start(out=out[tp], in_=osb)
            for p in (outp, small, expa, late):
                p.release()
    nc.compile()
    return nc


def kernel(inputs, mask, wq, bq, wk, bk, wv, bv, wo, bo):
    import ml_dtypes
    from concourse.bass_utils import run_bass_kernel_spmd

    x = np.asarray(inputs, dtype=np.float32)
    wq = np.asarray(wq, dtype=np.float32)
    wk = np.asarray(wk, dtype=np.float32)
    wv = np.asarray(wv, dtype=np.float32)
    wo = np.asarray(wo, dtype=np.float32)
    bq = np.asarray(bq, dtype=np.float32)
    bk = np.asarray(bk, dtype=np.float32)
    mask2d = np.asarray(mask, dtype=np.float32).reshape(S, S)
    causal_ref = 1.0 - np.tril(np.ones((S, S), dtype=np.float32))
    causal = bool(np.array_equal(mask2d, causal_ref))
    variant = "causal" if causal else "generic"
    if variant not in _CACHE:
        _CACHE[variant] = _build_causal() if causal else _build_generic()
    nc = _CACHE[variant]

    in_maps = []
    for c in range(NCORES):
        b, hg = c // 2, c % 2
        sl = slice(512 * hg, 512 * hg + 512)
        if causal:
            f16 = np.float16
            m = {
                "xt": np.ascontiguousarray(
                    x[b].T.reshape(EC, 128, S).transpose(1, 0, 2)).astype(f16),
                "wq": np.ascontiguousarray(
                    wq[:, sl].reshape(EC, 128, 512).transpose(1, 0, 2)).astype(f16),
                "wk": np.ascontiguousarray(
                    wk[:, sl].reshape(EC, 128, 512).transpose(1, 0, 2)).astype(f16),
                "wv": np.ascontiguousarray(
                    wv[:, sl].reshape(EC, 128, 512).transpose(1, 0, 2)).astype(f16),
                "wo": np.ascontiguousarray(
                    wo.reshape(EC, 128, 1024).transpose(1, 0, 2)).astype(f16),
                "bq": np.ascontiguousarray(bq[sl].reshape(4, 128).T),
                "bk": np.ascontiguousarray(bk[sl].reshape(4, 128).T),
                "tri": np.triu(np.ones((128, 128))).astype(ml_dtypes.bfloat16),
            }
        else:
            m = {
                "xt": np.ascontiguousarray(x[b].T.reshape(EC, 128, S).transpose(1, 0, 2)),
                "wq": np.ascontiguousarray(wq[:, sl].reshape(EC, 128, 512).transpose(1, 0, 2)),
                "wk": np.ascontiguousarray(wk[:, sl].reshape(EC, 128, 512).transpose(1, 0, 2)),
                "wv": np.ascontiguousarray(wv[:, sl].reshape(EC, 128, 512).transpose(1, 0, 2)),
                "wo": np.ascontiguousarray(wo.reshape(EC, 128, 1024).transpose(1, 0, 2)),
                "bq": np.ascontiguousarray(bq[sl].reshape(4, 128).T),
                "bk": np.ascontiguousarray(bk[sl].reshape(4, 128).T),
                "mkt": np.ascontiguousarray(
                    (mask2d.T * np.float32(-2e9)).reshape(TT, 128, S).transpose(1, 0, 2)
                ).astype(ml_dtypes.bfloat16),
            }
        in_maps.append(m)

    global _last_in_maps
    _last_in_maps = in_maps
    res = run_bass_kernel_spmd(nc, in_maps, core_ids=list(range(NCORES)))
    full = np.empty((B, S, E), dtype=np.float32)
    for c in range(NCORES):
        b, hg = c // 2, c % 2
        full[b, 512 * hg:512 * hg + 512, :] = np.asarray(
            res.results[c]["out"], dtype=np.float32).reshape(512, 1024)

    # biases bv/bo are zero in this problem; fold in exactly if ever nonzero.
    bv = np.asarray(bv, dtype=np.float32)
    bo = np.asarray(bo, dtype=np.float32)
    if np.any(bv != 0):
        bmat = np.zeros((S, E), dtype=np.float64)
        tpr = np.arange(S)
        e = np.arange(E)
        bmat[:, :] = bv[(64 * (tpr[:, None] // 64) + e[None, :] % 64)]
        full += (bmat @ np.asarray(wo, dtype=np.float64)).astype(np.float32)[None]
    if np.any(bo != 0):
        full += bo[None, None, :]
    return full


# revision 79
# speedup vs baseline: 1.0130x; 1.0045x over previous
"""Trainium2 Bass kernel for MultiHeadAttention (B=4, S=1024, E=1024, H=16, Dh=64).

Sharding: 8 cores = (batch b in 0..3) x (head-group hg in 0..1, 8 heads each).
The reference reshapes [B,H,S,Dh] -> [B,S,E] WITHOUT transposing heads back, so
head h's attention output occupies output rows t' = h*64 + s//16 - the final
projection is row-parallel across head groups: no cross-core communication.

Per-core pipeline (PE-roofline-bound: every matmul at 1.0 cycles/row):
  - inputs host-cast to fp16 (2.4e-4 exactness, half DMA bytes), outputs bf16
    (cast back on host); 2-ec-batched DMA ordered for early PE start
  - QK projections: 4 PSUM accumulators x 4 subphases (wq/wk x s-halves) so
    the 4-bank score pool coexists - no PSUM pool boundary before attention;
    xt streamed chunk-by-chunk; bias+fp16 cast on DVE into per-(dt,sh) tiles
    (fine-grained deps so scores never wait on unrelated copies)
  - V projection (shares the QK PSUM ring) into fp16 V' with 64 ones cols per
    head (one gpsimd memset): the z matmul emits z^T AND softmax denominators
  - scores^T[t,s] = K @ Q^T per (head-pair, segment-packed col group) into
    [128,2,512] PSUM tiles (1 bank/head): ONE activation exps both heads;
    small diagonal blocks share a tile to cut activation count; e tiles bf16
    (range-safe for exp with no max-subtraction); causal triangle = 0/1
    multiply on gpsimd (SBUF-only engine) for diagonal segments only
  - z/normalize: PSUM z + sums; reciprocal(sums)->SBUF then two PSUM*SBUF
    multiplies write the scrambled-reshape layout directly in fp16
  - out = X2 @ wo (fp16), PSUM->SBUF copy on DVE, bf16 DMA out per half
  - software pipelining: z/normalize/output work is queued as fine-grained
    units and drained one per score block so the in-order PE queue always has
    ready work while ACT exps; V-projection fills pair 0; each pair's output
    projection is HELD one pair and released into the next pair's normalize
    window (fills the DVE-bound endgame).

TimelineSim: 98682 ns (baseline 136254), rel err 2.8e-3 (gate 2e-2).
"""
import numpy as np

B, S, E, H, DH = 4, 1024, 1024, 16, 64
NCORES = 8
HPC = 8          # heads per core
EC = 8           # 128-row chunks of E
TT = 8           # 128-row t-tiles of S
NJ = 2           # 512-col s-blocks

_CACHE = {}


def _build_causal():
    import concourse.bacc as bacc
    import concourse.tile as tile
    import concourse.mybir as mybir

    f32 = mybir.dt.float32
    f32r = mybir.dt.float32r
    f16 = mybir.dt.float16
    bf16 = mybir.dt.bfloat16
    Exp = mybir.ActivationFunctionType.Exp
    Copy = mybir.ActivationFunctionType.Copy
    Ident = mybir.ActivationFunctionType.Identity
    Ln = mybir.ActivationFunctionType.Ln
    mult = mybir.AluOpType.mult
    div = mybir.AluOpType.divide

    nc = bacc.Bacc("TRN2")
    xt = nc.dram_tensor("xt", [128, EC, S], f16, kind="ExternalInput")
    wq = nc.dram_tensor("wq", [128, EC, 512], f16, kind="ExternalInput")
    wk = nc.dram_tensor("wk", [128, EC, 512], f16, kind="ExternalInput")
    wv = nc.dram_tensor("wv", [128, EC, 512], f16, kind="ExternalInput")
    wo = nc.dram_tensor("wo", [128, EC, 1024], f16, kind="ExternalInput")
    bq = nc.dram_tensor("bq", [128, 4], f32, kind="ExternalInput")
    bk = nc.dram_tensor("bk", [128, 4], f32, kind="ExternalInput")
    tri = nc.dram_tensor("tri", [128, 128], bf16, kind="ExternalInput")
    out = nc.dram_tensor("out", [4, 128, 1024], bf16, kind="ExternalOutput")

    with tile.TileContext(nc) as tc:
        pp = tc.alloc_tile_pool(name="pp", bufs=1)
        xt_sb = pp.tile([128, EC, S], f16)
        wq_sb = pp.tile([128, EC, 512], f16)
        wk_sb = pp.tile([128, EC, 512], f16)
        wv_sb = pp.tile([128, EC, 512], f16)
        wo_sb = pp.tile([128, EC, 1024], f16)
        qt_sb = [[pp.tile([128, 512], f16, name=f"qt_{d}_{s}")
                  for s in range(2)] for d in range(4)]
        kt_sb = [[pp.tile([128, 512], f16, name=f"kt_{d}_{s}")
                  for s in range(2)] for d in range(4)]
        vp_sb = pp.tile([128, TT, 1024], f16)
        x2t_sb = pp.tile([128, EC, 512], f16)
        bq_sb = pp.tile([128, 4], f32)
        bk_sb = pp.tile([128, 4], f32)
        tri_sb = pp.tile([128, 128], bf16)

        # ---- DMA preload: 2-ec batches (HWDGE/SP fixed costs are per-DMA),
        # ordered so the QK phase can start early ----
        groups = [(0, 1), (1, 3), (3, 5), (5, 7), (7, 8)]
        for a, b in groups:
            nc.sync.dma_start(out=wq_sb[:, a:b, :], in_=wq[:, a:b, :])
            nc.sync.dma_start(out=xt_sb[:, a:b, 0:512], in_=xt[:, a:b, 0:512])
            if a == 1:
                nc.sync.dma_start(out=bq_sb, in_=bq.ap())
        nc.sync.dma_start(out=bk_sb, in_=bk.ap())
        for k in range(0, EC, 2):
            nc.sync.dma_start(out=wk_sb[:, k:k + 2, :], in_=wk[:, k:k + 2, :])
        for k in range(0, EC, 2):
            nc.sync.dma_start(out=xt_sb[:, k:k + 2, 512:1024], in_=xt[:, k:k + 2, 512:1024])
        nc.sync.dma_start(out=tri_sb, in_=tri.ap())
        for k in range(0, EC, 2):
            nc.sync.dma_start(out=wv_sb[:, k:k + 2, :], in_=wv[:, k:k + 2, :])
        for k in range(0, EC, 2):
            nc.sync.dma_start(out=wo_sb[:, k:k + 2, :], in_=wo[:, k:k + 2, :])

        vview = vp_sb.rearrange("p t (h two d) -> p t h two d", two=2, d=DH)
        for tt in range(TT):
            nc.gpsimd.memset(vview[:, tt, :, 1, :], 1.0)

        # ---- Q^T/K^T projections: 4 PSUM accumulators x 4 subphases so
        # the score pool (4 banks) can be allocated upfront: no PSUM pool
        # boundary between projections and attention ----
        scp = tc.alloc_tile_pool(name="scp", bufs=2, space="PSUM")
        qkp = tc.alloc_tile_pool(name="qkp", bufs=4, space="PSUM")
        ep = tc.alloc_tile_pool(name="ep", bufs=24)
        osp = tc.alloc_tile_pool(name="osp", bufs=2)
        rcp = tc.alloc_tile_pool(name="rcp", bufs=3)
        for sh, wi, wsb, dest, bias in (
            (0, 0, wq_sb, qt_sb, bq_sb), (0, 1, wk_sb, kt_sb, bk_sb),
            (1, 0, wq_sb, qt_sb, bq_sb), (1, 1, wk_sb, kt_sb, bk_sb),
        ):
            sl = slice(512 * sh, 512 * sh + 512)
            pss = [qkp.tile([128, 512], f32, tag="qk", name=f"qk_{sh}_{wi}_{d}")
                   for d in range(4)]
            for ec in range(EC):
                for dt_ in range(4):
                    nc.tensor.matmul(
                        pss[dt_],
                        wsb[:, ec, 128 * dt_:128 * dt_ + 128],
                        xt_sb[:, ec, sl],
                        start=(ec == 0), stop=(ec == EC - 1),
                    )
            for dt_ in range(4):
                nc.vector.tensor_scalar_add(
                    out=dest[dt_][sh], in0=pss[dt_],
                    scalar1=bias[:, dt_:dt_ + 1],
                )

        pend = []
        held = []

        def drain(n):
            for _ in range(min(n, len(pend))):
                pend.pop(0)()

        def vjob(tt):
            ps = qkp.tile([128, 512], f32, tag="qk", name=f"v_{tt}")
            for ec in range(EC):
                nc.tensor.matmul(
                    ps, xt_sb[:, ec, 128 * tt:128 * tt + 128], wv_sb[:, ec, :],
                    start=(ec == 0), stop=(ec == EC - 1),
                )
            nc.vector.tensor_copy(
                vview[:, tt, :, 0, :], ps.rearrange("p (h d) -> p h d", d=DH)
            )

        for tt in range(TT):
            pend.append(lambda tt=tt: vjob(tt))

        ztp = opp = None
        for hp in range(4):
            pair = (2 * hp, 2 * hp + 1)
            et_ref = {}
            js = [1, 0] if hp in (1, 2) else [0, 1]
            blocks = []
            for j in js:
                cur, used = [], 0
                for tt in range(TT):
                    if 512 * j + 511 < 128 * tt:
                        continue
                    w = 512 * j + 512 - max(512 * j, 128 * tt)
                    if used + w > 512:
                        if cur:
                            blocks.append(cur)
                        cur, used = [], 0
                    cur.append((tt, j))
                    used += w
                if cur:
                    blocks.append(cur)

            def sblock(group, hp=hp, pair=pair, et_ref=et_ref):
                # group: list of (tt, j) segments packed into one score tile
                # (the small diagonal blocks share a tile so one activation
                # instruction exps them together)
                segs = []
                col = 0
                for (tt, j) in group:
                    c0 = 128 * tt
                    lo = max(512 * j, c0)
                    w = 512 * j + 512 - lo
                    segs.append((tt, j, c0, lo, w, col))
                    col += w
                total = col
                gname = f"{hp}_{group[0][0]}_{group[0][1]}"
                sc = scp.tile([128, 2, 512], f32, tag="sc", name=f"sc_{gname}")
                for (tt, j, c0, lo, w, col) in segs:
                    kc = c0 % 512
                    for hh, h in enumerate(pair):
                        pb = 64 * (h % 2)
                        nc.tensor.matmul(
                            sc[:, hh, col:col + w],
                            kt_sb[hp][tt // 4][pb:pb + 64, kc:kc + 128],
                            qt_sb[hp][j][pb:pb + 64, lo - 512 * j:512],
                            start=True, stop=True,
                        )
                et = ep.tile([128, 2, 512], bf16, tag="e", name=f"e_{gname}")
                nc.scalar.activation(
                    et[:, :, 0:total], sc[:, :, 0:total], Exp, scale=0.5
                )
                for (tt, j, c0, lo, w, col) in segs:
                    if lo == c0:
                        for hh in range(2):
                            nc.gpsimd.tensor_tensor(
                                et[:, hh, col:col + 128],
                                et[:, hh, col:col + 128], tri_sb, op=mult,
                            )
                    et_ref[(tt, j)] = (et, col, lo - 512 * j, w)

            def push_z(h, hh, j, et_ref=et_ref):
                """Append z-chain units (split mm halves + parallel divides)."""
                box = {}
                tts = [tt for tt in range(TT) if (tt, j) in et_ref]
                halves = [tts]

                def mm_unit(sub, first, last, h=h, hh=hh, j=j, box=box,
                            et_ref=et_ref, ntot=len(tts)):
                    if first:
                        box["zt"] = ztp.tile([128, 512], f32, tag="zt",
                                             name=f"zt_{h}_{j}")
                    zt = box["zt"]
                    for i, tt in enumerate(sub):
                        et, col, rel0, w = et_ref[(tt, j)]
                        nc.tensor.matmul(
                            zt[:, rel0:rel0 + w],
                            vp_sb[:, tt, 128 * h:128 * h + 128],
                            et[:, hh, col:col + w],
                            start=(first and i == 0),
                            stop=(last and i == len(sub) - 1),
                        )

                def div_unit(h=h, j=j, hp=hp, box=box):
                    # x2t = z / sums; both live in PSUM and an ALU op may
                    # read only one PSUM operand, so 1/sums goes via SBUF.
                    # For the final pair ACT is idle (no more exps) while DVE
                    # gates the endgame: compute 1/sums as exp(-ln(sums))
                    # there instead.
                    zt = box["zt"]
                    rec = rcp.tile([64, 512], f32, tag="rec",
                                   name=f"rec_{h}_{j}")
                    nc.vector.reciprocal(rec, zt[64:128, :])
                    zv = zt[0:64].rearrange("p (m c par) -> par p c m",
                                            m=32, c=8, par=2)
                    rv = rec.rearrange("p (m c par) -> par p c m",
                                       m=32, c=8, par=2)
                    o0 = 64 * h + 32 * j
                    nc.vector.tensor_tensor(
                        x2t_sb[0:64, :, o0:o0 + 32], zv[0], rv[0], op=mult)
                    nc.vector.tensor_tensor(
                        x2t_sb[64:128, :, o0:o0 + 32], zv[1], rv[1], op=mult)

                for si, sub in enumerate(halves):
                    pend.append(lambda s=sub, f=mm_unit, first=(si == 0),
                                last=(si == len(halves) - 1): f(s, first, last))
                pend.append(div_unit)

            osb_box = {}

            def o_unit(eh, half, hp=hp, osb_box=osb_box):
                if "t" not in osb_box:
                    osb_box["t"] = osp.tile([128, 1024], bf16, tag="osb",
                                            name=f"osb_{hp}")
                t = osb_box["t"]
                if half == 0:
                    osb_box[eh] = opp.tile([128, 512], f32, tag="op",
                                           name=f"op_{hp}_{eh}")
                op_t = osb_box[eh]
                cs = range(0, 4) if half == 0 else range(4, EC)
                for c in cs:
                    nc.tensor.matmul(
                        op_t, x2t_sb[:, c, 128 * hp:128 * hp + 128],
                        wo_sb[:, c, 512 * eh:512 * eh + 512],
                        start=(c == 0), stop=(c == EC - 1),
                    )
                if half == 1:
                    nc.vector.tensor_copy(t[:, 512 * eh:512 * eh + 512], op_t)
                    # final pair: issue eh0's transfer from the (idle)
                    # ACT queue so the last transfer isn't serialized
                    eng = nc.scalar if (eh == 0 and hp == 3) else nc.sync
                    eng.dma_start(out=out[hp, :, 512 * eh:512 * eh + 512],
                                  in_=t[:, 512 * eh:512 * eh + 512])

            nj0 = sum(1 for g in blocks if g[0][1] == js[0])
            for bi, group in enumerate(blocks):
                sblock(group)
                drain(1)
                if bi == nj0 - 1 and hp > 0:
                    # first j-group e-tiles complete: its z can drain now
                    for hh, h in enumerate(pair):
                        push_z(h, hh, js[0])
            if hp == 0:
                while pend:
                    pend.pop(0)()
                qkp.release()
                ztp = tc.alloc_tile_pool(name="ztp", bufs=2, space="PSUM")
                opp = tc.alloc_tile_pool(name="opp", bufs=2, space="PSUM")
                for hh, h in enumerate(pair):
                    push_z(h, hh, js[0])
            for hh, h in enumerate(pair):
                push_z(h, hh, js[1])
                if held:
                    # previous pair's output projection was held back: its
                    # matmuls fill the PE idle window while DVE normalizes
                    pend.append(held.pop(0))
                    pend.append(held.pop(0))
            for eh in range(2):
                for half in range(2):
                    u = (lambda f=o_unit, eh=eh, half=half: f(eh, half))
                    (held if hp < 3 else pend).append(u)

        while pend:
            pend.pop(0)()
        for p_ in (rcp, osp, ep, opp, ztp, scp):
            p_.release()
        pp.release()
    nc.compile()
    return nc


def _build_generic():
    """Fallback for a non-causal mask: the original fp32r kernel."""
    import concourse.bacc as bacc
    import concourse.tile as tile
    import concourse.mybir as mybir

    f32 = mybir.dt.float32
    f32r = mybir.dt.float32r
    bf16 = mybir.dt.bfloat16
    Exp = mybir.ActivationFunctionType.Exp
    mult = mybir.AluOpType.mult

    nc = bacc.Bacc("TRN2")
    xt = nc.dram_tensor("xt", [128, EC, S], f32r, kind="ExternalInput")
    wq = nc.dram_tensor("wq", [128, EC, 512], f32r, kind="ExternalInput")
    wk = nc.dram_tensor("wk", [128, EC, 512], f32r, kind="ExternalInput")
    wv = nc.dram_tensor("wv", [128, EC, 512], f32r, kind="ExternalInput")
    wo = nc.dram_tensor("wo", [128, EC, 1024], f32r, kind="ExternalInput")
    bq = nc.dram_tensor("bq", [128, 4], f32, kind="ExternalInput")
    bk = nc.dram_tensor("bk", [128, 4], f32, kind="ExternalInput")
    mkt = nc.dram_tensor("mkt", [128, TT, S], bf16, kind="ExternalInput")
    out = nc.dram_tensor("out", [4, 128, 1024], f32, kind="ExternalOutput")

    with tile.TileContext(nc) as tc:
        with (
            tc.tile_pool(name="persist", bufs=1) as pp,
            tc.tile_pool(name="mm", bufs=3, space="PSUM") as mm,
            tc.tile_pool(name="ztp", bufs=2, space="PSUM") as ztp,
        ):
            p1 = tc.alloc_tile_pool(name="p1", bufs=1)
            xt_sb = p1.tile([128, EC, S], f32r)
            wq_sb = p1.tile([128, EC, 512], f32r)
            wk_sb = p1.tile([128, EC, 512], f32r)
            wv_sb = p1.tile([128, EC, 512], f32r)
            for k in range(0, EC, 2):
                nc.sync.dma_start(out=xt_sb[:, k:k + 2, :], in_=xt[:, k:k + 2, :])
                nc.sync.dma_start(out=wq_sb[:, k:k + 2, :], in_=wq[:, k:k + 2, :])
                nc.sync.dma_start(out=wk_sb[:, k:k + 2, :], in_=wk[:, k:k + 2, :])
                nc.sync.dma_start(out=wv_sb[:, k:k + 2, :], in_=wv[:, k:k + 2, :])
            qt_sb = pp.tile([128, 4, S], f32r)
            kt_sb = pp.tile([128, 4, S], f32r)
            vp_sb = pp.tile([128, TT, 1024], bf16)
            x2t_sb = pp.tile([128, EC, 512], f32r)
            bq_sb = pp.tile([128, 4], f32)
            bk_sb = pp.tile([128, 4], f32)
            mkt_sb = pp.tile([128, TT, S], bf16)
            nc.sync.dma_start(out=mkt_sb, in_=mkt.ap())
            nc.sync.dma_start(out=bq_sb, in_=bq.ap())
            nc.sync.dma_start(out=bk_sb, in_=bk.ap())

            vview = vp_sb.rearrange("p t (h two d) -> p t h two d", two=2, d=DH)
            ones_sb = pp.tile([128, 512], f32)
            nc.vector.memset(ones_sb, 1.0)
            ones_v = ones_sb.rearrange("p (h d) -> p h d", d=DH)
            for tt in range(TT):
                nc.vector.tensor_copy(vview[:, tt, :, 1, :], ones_v)

            for wsb, dest, bias in ((wq_sb, qt_sb, bq_sb), (wk_sb, kt_sb, bk_sb)):
                for dt_ in range(4):
                    for sh in range(2):
                        ps = mm.tile([128, 512], f32, tag="mm")
                        for ec in range(EC):
                            nc.tensor.matmul(
                                ps, wsb[:, ec, 128 * dt_:128 * dt_ + 128],
                                xt_sb[:, ec, 512 * sh:512 * sh + 512],
                                start=(ec == 0), stop=(ec == EC - 1),
                            )
                        nc.vector.tensor_scalar_add(
                            out=dest[:, dt_, 512 * sh:512 * sh + 512],
                            in0=ps, scalar1=bias[:, dt_:dt_ + 1],
                        )
            for tt in range(TT):
                ps = mm.tile([128, 512], f32, tag="mm")
                for ec in range(EC):
                    nc.tensor.matmul(
                        ps, xt_sb[:, ec, 128 * tt:128 * tt + 128],
                        wv_sb[:, ec, :],
                        start=(ec == 0), stop=(ec == EC - 1),
                    )
                nc.vector.tensor_copy(
                    vview[:, tt, :, 0, :], ps.rearrange("p (h d) -> p h d", d=DH)
                )
            p1.release()
            late = tc.alloc_tile_pool(name="late", bufs=1)
            expa = tc.alloc_tile_pool(name="expa", bufs=16)
            small = tc.alloc_tile_pool(name="small", bufs=2)
            outp = tc.alloc_tile_pool(name="outp", bufs=2)
            wo_sb = late.tile([128, EC, 1024], f32r)
            nc.sync.dma_start(out=wo_sb, in_=wo.ap())

            for hp in range(HPC // 2):
                pair = (2 * hp, 2 * hp + 1)
                et = {}
                for tt in range(TT):
                    pss = {}
                    for h in pair:
                        dt_ = h // 2
                        pb = 64 * (h % 2)
                        ps = mm.tile([128, 1024], f32, tag="mm", name=f"ps_{h}_{tt}")
                        pss[h] = ps
                        for j in range(NJ):
                            nc.tensor.matmul(
                                ps[:, 512 * j:512 * j + 512],
                                kt_sb[pb:pb + 64, dt_, 128 * tt:128 * tt + 128],
                                qt_sb[pb:pb + 64, dt_, 512 * j:512 * j + 512],
                                start=True, stop=True,
                            )
                            nc.vector.tensor_add(
                                ps[:, 512 * j:512 * j + 512],
                                ps[:, 512 * j:512 * j + 512],
                                mkt_sb[:, tt, 512 * j:512 * j + 512],
                            )
                    for h in pair:
                        ps = pss[h]
                        e = expa.tile([128, 1024], bf16, tag="expa",
                                      name=f"e_{h}_{tt}")
                        nc.scalar.activation(e[:, :], ps[:, :], Exp, scale=0.5)
                        for j in range(NJ):
                            et[(h, tt, j)] = e[:, 512 * j:512 * j + 512]
                for h in pair:
                    zt_f = small.tile([64, S], f32, tag="ztf", name=f"ztf_{h}")
                    rec = small.tile([64, S], f32, tag="rec", name=f"rec_{h}")
                    for j in range(NJ):
                        zt = ztp.tile([128, 512], f32, tag="zt", name=f"zt_{h}_{j}")
                        for i, tt in enumerate(range(TT)):
                            nc.tensor.matmul(
                                zt, vp_sb[:, tt, 128 * h:128 * h + 128],
                                et[(h, tt, j)],
                                start=(i == 0), stop=(i == TT - 1),
                            )
                        nc.vector.reciprocal(rec[:, 512 * j:512 * j + 512],
                                             zt[64:128, :])
                        nc.vector.tensor_copy(zt_f[:, 512 * j:512 * j + 512],
                                              zt[0:64, :])
                    zv = zt_f.rearrange("p (m c par) -> par p c m", m=64, c=8, par=2)
                    rv = rec.rearrange("p (m c par) -> par p c m", m=64, c=8, par=2)
                    for P in range(2):
                        nc.vector.tensor_tensor(
                            x2t_sb[64 * P:64 * P + 64, :, 64 * h:64 * h + 64],
                            zv[P], rv[P], op=mult,
                        )

            for tp in range(4):
                osb = outp.tile([128, 1024], f32, tag="osb")
                for eh in range(2):
                    ps = mm.tile([128, 512], f32, tag="mm")
                    for c in range(EC):
                        nc.tensor.matmul(
                            ps, x2t_sb[:, c, 128 * tp:128 * tp + 128],
                            wo_sb[:, c, 512 * eh:512 * eh + 512],
                            start=(c == 0), stop=(c == EC - 1),
                        )
                    nc.vector.tensor_copy(osb[:, 512 * eh:512 * eh + 512], ps)
                nc.sync.dma_start(out=out[tp], in_=osb)
            for p in (outp, small, expa, late):
                p.release()
    nc.compile()
    return nc


def kernel(inputs, mask, wq, bq, wk, bk, wv, bv, wo, bo):
    import ml_dtypes
    from concourse.bass_utils import run_bass_kernel_spmd

    x = np.asarray(inputs, dtype=np.float32)
    wq = np.asarray(wq, dtype=np.float32)
    wk = np.asarray(wk, dtype=np.float32)
    wv = np.asarray(wv, dtype=np.float32)
    wo = np.asarray(wo, dtype=np.float32)
    bq = np.asarray(bq, dtype=np.float32)
    bk = np.asarray(bk, dtype=np.float32)
    mask2d = np.asarray(mask, dtype=np.float32).reshape(S, S)
    causal_ref = 1.0 - np.tril(np.ones((S, S), dtype=np.float32))
    causal = bool(np.array_equal(mask2d, causal_ref))
    variant = "causal" if causal else "generic"
    if variant not in _CACHE:
        _CACHE[variant] = _build_causal() if causal else _build_generic()
    nc = _CACHE[variant]

    in_maps = []
    for c in range(NCORES):
        b, hg = c // 2, c % 2
        sl = slice(512 * hg, 512 * hg + 512)
        if causal:
            f16 = np.float16
            m = {
                "xt": np.ascontiguousarray(
                    x[b].T.reshape(EC, 128, S).transpose(1, 0, 2)).astype(f16),
                "wq": np.ascontiguousarray(
                    wq[:, sl].reshape(EC, 128, 512).transpose(1, 0, 2)).astype(f16),
                "wk": np.ascontiguousarray(
                    wk[:, sl].reshape(EC, 128, 512).transpose(1, 0, 2)).astype(f16),
                "wv": np.ascontiguousarray(
                    wv[:, sl].reshape(EC, 128, 512).transpose(1, 0, 2)).astype(f16),
                "wo": np.ascontiguousarray(
                    wo.reshape(EC, 128, 1024).transpose(1, 0, 2)).astype(f16),
                "bq": np.ascontiguousarray(bq[sl].reshape(4, 128).T),
                "bk": np.ascontiguousarray(bk[sl].reshape(4, 128).T),
                "tri": np.triu(np.ones((128, 128))).astype(ml_dtypes.bfloat16),
            }
        else:
            m = {
                "xt": np.ascontiguousarray(x[b].T.reshape(EC, 128, S).transpose(1, 0, 2)),
                "wq": np.ascontiguousarray(wq[:, sl].reshape(EC, 128, 512).transpose(1, 0, 2)),
                "wk": np.ascontiguousarray(wk[:, sl].reshape(EC, 128, 512).transpose(1, 0, 2)),
                "wv": np.ascontiguousarray(wv[:, sl].reshape(EC, 128, 512).transpose(1, 0, 2)),
                "wo": np.ascontiguousarray(wo.reshape(EC, 128, 1024).transpose(1, 0, 2)),
                "bq": np.ascontiguousarray(bq[sl].reshape(4, 128).T),
                "bk": np.ascontiguousarray(bk[sl].reshape(4, 128).T),
                "mkt": np.ascontiguousarray(
                    (mask2d.T * np.float32(-2e9)).reshape(TT, 128, S).transpose(1, 0, 2)
                ).astype(ml_dtypes.bfloat16),
            }
        in_maps.append(m)

    global _last_in_maps
    _last_in_maps = in_maps
    res = run_bass_kernel_spmd(nc, in_maps, core_ids=list(range(NCORES)))
    full = np.empty((B, S, E), dtype=np.float32)
    for c in range(NCORES):
        b, hg = c // 2, c % 2
        full[b, 512 * hg:512 * hg + 512, :] = np.asarray(
            res.results[c]["out"], dtype=np.float32).reshape(512, 1024)

    # biases bv/bo are zero in this problem; fold in exactly if ever nonzero.
    bv = np.asarray(bv, dtype=np.float32)
    bo = np.asarray(bo, dtype=np.float32)
    if np.any(bv != 0):
        bmat = np.zeros((S, E), dtype=np.float64)
        tpr = np.arange(S)
        e = np.arange(E)
        bmat[:, :] = bv[(64 * (tpr[:, None] // 64) + e[None, :] % 64)]
        full += (bmat @ np.asarray(wo, dtype=np.float64)).astype(np.float32)[None]
    if np.any(bo != 0):
        full += bo[None, None, :]
    return full


# revision 85
# speedup vs baseline: 1.0146x; 1.0016x over previous
"""Trainium2 Bass kernel for MultiHeadAttention (B=4, S=1024, E=1024, H=16, Dh=64).

Sharding: 8 cores = (batch b in 0..3) x (head-group hg in 0..1, 8 heads each).
The reference reshapes [B,H,S,Dh] -> [B,S,E] WITHOUT transposing heads back, so
head h's attention output occupies output rows t' = h*64 + s//16 - the final
projection is row-parallel across head groups: no cross-core communication.

Per-core pipeline (PE-roofline-bound: every matmul at 1.0 cycles/row):
  - inputs host-cast to fp16 (2.4e-4 exactness, half DMA bytes), outputs bf16
    (cast back on host); 2-ec-batched DMA ordered for early PE start
  - QK projections: 4 PSUM accumulators x 4 subphases (wq/wk x s-halves) so
    the 4-bank score pool coexists - no PSUM pool boundary before attention;
    xt streamed chunk-by-chunk; bias+fp16 cast on DVE into per-(dt,sh) tiles
    (fine-grained deps so scores never wait on unrelated copies)
  - V projection (shares the QK PSUM ring) into fp16 V' with 64 ones cols per
    head (one gpsimd memset): the z matmul emits z^T AND softmax denominators
  - scores^T[t,s] = K @ Q^T per (head-pair, segment-packed col group) into
    [128,2,512] PSUM tiles (1 bank/head): ONE activation exps both heads;
    small diagonal blocks share a tile to cut activation count; e tiles bf16
    (range-safe for exp with no max-subtraction); causal triangle = 0/1
    multiply on gpsimd (SBUF-only engine) for diagonal segments only
  - z/normalize: PSUM z + sums; reciprocal(sums)->SBUF then two PSUM*SBUF
    multiplies write the scrambled-reshape layout directly in fp16
  - out = X2 @ wo (fp16), PSUM->SBUF copy on DVE, bf16 DMA out per half
  - software pipelining: z/normalize/output work is queued as fine-grained
    units and drained one per score block so the in-order PE queue always has
    ready work while ACT exps; V-projection fills pair 0; each pair's output
    projection is HELD one pair and released into the next pair's normalize
    window (fills the DVE-bound endgame).

TimelineSim: 97732 ns (baseline 136254), rel err 2.8e-3 (gate 2e-2).
"""
import numpy as np

B, S, E, H, DH = 4, 1024, 1024, 16, 64
NCORES = 8
HPC = 8          # heads per core
EC = 8           # 128-row chunks of E
TT = 8           # 128-row t-tiles of S
NJ = 2           # 512-col s-blocks

_CACHE = {}


def _build_causal():
    import concourse.bacc as bacc
    import concourse.tile as tile
    import concourse.mybir as mybir

    f32 = mybir.dt.float32
    f32r = mybir.dt.float32r
    f16 = mybir.dt.float16
    bf16 = mybir.dt.bfloat16
    Exp = mybir.ActivationFunctionType.Exp
    Copy = mybir.ActivationFunctionType.Copy
    Ident = mybir.ActivationFunctionType.Identity
    Ln = mybir.ActivationFunctionType.Ln
    mult = mybir.AluOpType.mult
    div = mybir.AluOpType.divide

    nc = bacc.Bacc("TRN2")
    xt = nc.dram_tensor("xt", [128, EC, S], f16, kind="ExternalInput")
    wq = nc.dram_tensor("wq", [128, EC, 512], f16, kind="ExternalInput")
    wk = nc.dram_tensor("wk", [128, EC, 512], f16, kind="ExternalInput")
    wv = nc.dram_tensor("wv", [128, EC, 512], f16, kind="ExternalInput")
    wo = nc.dram_tensor("wo", [128, EC, 1024], f16, kind="ExternalInput")
    bq = nc.dram_tensor("bq", [128, 4], f32, kind="ExternalInput")
    bk = nc.dram_tensor("bk", [128, 4], f32, kind="ExternalInput")
    tri = nc.dram_tensor("tri", [128, 128], bf16, kind="ExternalInput")
    out = nc.dram_tensor("out", [4, 128, 1024], bf16, kind="ExternalOutput")

    with tile.TileContext(nc) as tc:
        pp = tc.alloc_tile_pool(name="pp", bufs=1)
        xt_sb = pp.tile([128, EC, S], f16)
        wq_sb = pp.tile([128, EC, 512], f16)
        wk_sb = pp.tile([128, EC, 512], f16)
        wv_sb = pp.tile([128, EC, 512], f16)
        wo_sb = pp.tile([128, EC, 1024], f16)
        qt_sb = [[pp.tile([128, 512], f16, name=f"qt_{d}_{s}")
                  for s in range(2)] for d in range(4)]
        kt_sb = [[pp.tile([128, 512], f16, name=f"kt_{d}_{s}")
                  for s in range(2)] for d in range(4)]
        vp_sb = pp.tile([128, TT, 1024], f16)
        x2t_sb = pp.tile([128, EC, 512], f16)
        bq_sb = pp.tile([128, 4], f32)
        bk_sb = pp.tile([128, 4], f32)
        tri_sb = pp.tile([128, 128], bf16)

        # ---- DMA preload: 2-ec batches (HWDGE/SP fixed costs are per-DMA),
        # ordered so the QK phase can start early ----
        groups = [(0, 1), (1, 3), (3, 5), (5, 7), (7, 8)]
        for a, b in groups:
            nc.sync.dma_start(out=wq_sb[:, a:b, :], in_=wq[:, a:b, :])
            nc.sync.dma_start(out=xt_sb[:, a:b, 0:512], in_=xt[:, a:b, 0:512])
            if a == 1:
                nc.sync.dma_start(out=bq_sb, in_=bq.ap())
        nc.sync.dma_start(out=bk_sb, in_=bk.ap())
        for k in range(0, EC, 2):
            nc.sync.dma_start(out=wk_sb[:, k:k + 2, :], in_=wk[:, k:k + 2, :])
        for k in range(0, EC, 2):
            nc.sync.dma_start(out=xt_sb[:, k:k + 2, 512:1024], in_=xt[:, k:k + 2, 512:1024])
        nc.sync.dma_start(out=tri_sb, in_=tri.ap())
        for k in range(0, EC, 2):
            nc.sync.dma_start(out=wv_sb[:, k:k + 2, :], in_=wv[:, k:k + 2, :])
        for k in range(0, EC, 2):
            nc.sync.dma_start(out=wo_sb[:, k:k + 2, :], in_=wo[:, k:k + 2, :])

        vview = vp_sb.rearrange("p t (h two d) -> p t h two d", two=2, d=DH)
        for tt in range(TT):
            nc.gpsimd.memset(vview[:, tt, :, 1, :], 1.0)

        # ---- Q^T/K^T projections: 4 PSUM accumulators x 4 subphases so
        # the score pool (4 banks) can be allocated upfront: no PSUM pool
        # boundary between projections and attention ----
        scp = tc.alloc_tile_pool(name="scp", bufs=2, space="PSUM")
        qkp = tc.alloc_tile_pool(name="qkp", bufs=4, space="PSUM")
        ep = tc.alloc_tile_pool(name="ep", bufs=32)
        osp = tc.alloc_tile_pool(name="osp", bufs=2)
        rcp = tc.alloc_tile_pool(name="rcp", bufs=3)
        for sh, wi, wsb, dest, bias in (
            (0, 0, wq_sb, qt_sb, bq_sb), (0, 1, wk_sb, kt_sb, bk_sb),
            (1, 0, wq_sb, qt_sb, bq_sb), (1, 1, wk_sb, kt_sb, bk_sb),
        ):
            sl = slice(512 * sh, 512 * sh + 512)
            pss = [qkp.tile([128, 512], f32, tag="qk", name=f"qk_{sh}_{wi}_{d}")
                   for d in range(4)]
            for ec in range(EC):
                for dt_ in range(4):
                    nc.tensor.matmul(
                        pss[dt_],
                        wsb[:, ec, 128 * dt_:128 * dt_ + 128],
                        xt_sb[:, ec, sl],
                        start=(ec == 0), stop=(ec == EC - 1),
                    )
            for dt_ in range(4):
                nc.vector.tensor_scalar_add(
                    out=dest[dt_][sh], in0=pss[dt_],
                    scalar1=bias[:, dt_:dt_ + 1],
                )

        pend = []
        held = []

        def drain(n):
            for _ in range(min(n, len(pend))):
                pend.pop(0)()

        def vjob(tt):
            ps = qkp.tile([128, 512], f32, tag="qk", name=f"v_{tt}")
            for ec in range(EC):
                nc.tensor.matmul(
                    ps, xt_sb[:, ec, 128 * tt:128 * tt + 128], wv_sb[:, ec, :],
                    start=(ec == 0), stop=(ec == EC - 1),
                )
            nc.vector.tensor_copy(
                vview[:, tt, :, 0, :], ps.rearrange("p (h d) -> p h d", d=DH)
            )

        for tt in range(TT):
            pend.append(lambda tt=tt: vjob(tt))

        ztp = opp = None
        for hp in range(4):
            pair = (2 * hp, 2 * hp + 1)
            et_ref = {}
            js = [1, 0] if hp in (1, 2) else [0, 1]
            blocks = []
            for j in js:
                cur, used = [], 0
                for tt in range(TT):
                    if 512 * j + 511 < 128 * tt:
                        continue
                    w = 512 * j + 512 - max(512 * j, 128 * tt)
                    if used + w > 512:
                        if cur:
                            blocks.append(cur)
                        cur, used = [], 0
                    cur.append((tt, j))
                    used += w
                if cur:
                    blocks.append(cur)

            def sblock(group, hp=hp, pair=pair, et_ref=et_ref):
                # group: list of (tt, j) segments packed into one score tile
                # (the small diagonal blocks share a tile so one activation
                # instruction exps them together)
                segs = []
                col = 0
                for (tt, j) in group:
                    c0 = 128 * tt
                    lo = max(512 * j, c0)
                    w = 512 * j + 512 - lo
                    segs.append((tt, j, c0, lo, w, col))
                    col += w
                total = col
                gname = f"{hp}_{group[0][0]}_{group[0][1]}"
                sc = scp.tile([128, 2, 512], f32, tag="sc", name=f"sc_{gname}")
                for (tt, j, c0, lo, w, col) in segs:
                    kc = c0 % 512
                    for hh, h in enumerate(pair):
                        pb = 64 * (h % 2)
                        nc.tensor.matmul(
                            sc[:, hh, col:col + w],
                            kt_sb[hp][tt // 4][pb:pb + 64, kc:kc + 128],
                            qt_sb[hp][j][pb:pb + 64, lo - 512 * j:512],
                            start=True, stop=True,
                        )
                et = ep.tile([128, 2, 512], bf16, tag="e", name=f"e_{gname}")
                nc.scalar.activation(
                    et[:, :, 0:total], sc[:, :, 0:total], Exp, scale=0.5
                )
                for (tt, j, c0, lo, w, col) in segs:
                    if lo == c0:
                        for hh in range(2):
                            nc.gpsimd.tensor_tensor(
                                et[:, hh, col:col + 128],
                                et[:, hh, col:col + 128], tri_sb, op=mult,
                            )
                    et_ref[(tt, j)] = (et, col, lo - 512 * j, w)

            def push_z(h, hh, j, et_ref=et_ref):
                """Append z-chain units (split mm halves + parallel divides)."""
                box = {}
                tts = [tt for tt in range(TT) if (tt, j) in et_ref]
                halves = ([tts[:4], tts[4:]] if len(tts) > 5 else [tts])

                def mm_unit(sub, first, last, h=h, hh=hh, j=j, box=box,
                            et_ref=et_ref, ntot=len(tts)):
                    if first:
                        box["zt"] = ztp.tile([128, 512], f32, tag="zt",
                                             name=f"zt_{h}_{j}")
                    zt = box["zt"]
                    for i, tt in enumerate(sub):
                        et, col, rel0, w = et_ref[(tt, j)]
                        nc.tensor.matmul(
                            zt[:, rel0:rel0 + w],
                            vp_sb[:, tt, 128 * h:128 * h + 128],
                            et[:, hh, col:col + w],
                            start=(first and i == 0),
                            stop=(last and i == len(sub) - 1),
                        )

                def div_unit(h=h, j=j, hp=hp, box=box):
                    # x2t = z / sums; both live in PSUM and an ALU op may
                    # read only one PSUM operand, so 1/sums goes via SBUF.
                    # For the final pair ACT is idle (no more exps) while DVE
                    # gates the endgame: compute 1/sums as exp(-ln(sums))
                    # there instead.
                    zt = box["zt"]
                    rec = rcp.tile([64, 512], f32, tag="rec",
                                   name=f"rec_{h}_{j}")
                    nc.vector.reciprocal(rec, zt[64:128, :])
                    zv = zt[0:64].rearrange("p (m c par) -> par p c m",
                                            m=32, c=8, par=2)
                    rv = rec.rearrange("p (m c par) -> par p c m",
                                       m=32, c=8, par=2)
                    o0 = 64 * h + 32 * j
                    nc.vector.tensor_tensor(
                        x2t_sb[0:64, :, o0:o0 + 32], zv[0], rv[0], op=mult)
                    nc.vector.tensor_tensor(
                        x2t_sb[64:128, :, o0:o0 + 32], zv[1], rv[1], op=mult)

                for si, sub in enumerate(halves):
                    pend.append(lambda s=sub, f=mm_unit, first=(si == 0),
                                last=(si == len(halves) - 1): f(s, first, last))
                pend.append(div_unit)

            osb_box = {}

            def o_unit(eh, half, hp=hp, osb_box=osb_box):
                if "t" not in osb_box:
                    osb_box["t"] = osp.tile([128, 1024], bf16, tag="osb",
                                            name=f"osb_{hp}")
                t = osb_box["t"]
                if half == 0:
                    osb_box[eh] = opp.tile([128, 512], f32, tag="op",
                                           name=f"op_{hp}_{eh}")
                op_t = osb_box[eh]
                cs = range(0, 4) if half == 0 else range(4, EC)
                for c in cs:
                    nc.tensor.matmul(
                        op_t, x2t_sb[:, c, 128 * hp:128 * hp + 128],
                        wo_sb[:, c, 512 * eh:512 * eh + 512],
                        start=(c == 0), stop=(c == EC - 1),
                    )
                if half == 1:
                    nc.vector.tensor_copy(t[:, 512 * eh:512 * eh + 512], op_t)
                    # final pair: issue eh0's transfer from the (idle)
                    # ACT queue so the last transfer isn't serialized
                    eng = nc.scalar if (eh == 0 and hp == 3) else nc.sync
                    eng.dma_start(out=out[hp, :, 512 * eh:512 * eh + 512],
                                  in_=t[:, 512 * eh:512 * eh + 512])

            nj0 = sum(1 for g in blocks if g[0][1] == js[0])
            for bi, group in enumerate(blocks):
                sblock(group)
                drain(1)
                if bi == nj0 - 1 and hp > 0:
                    # first j-group e-tiles complete: its z can drain now
                    for hh, h in enumerate(pair):
                        push_z(h, hh, js[0])
            if hp == 0:
                while pend:
                    pend.pop(0)()
                qkp.release()
                ztp = tc.alloc_tile_pool(name="ztp", bufs=2, space="PSUM")
                opp = tc.alloc_tile_pool(name="opp", bufs=2, space="PSUM")
                for hh, h in enumerate(pair):
                    push_z(h, hh, js[0])
            for hh, h in enumerate(pair):
                push_z(h, hh, js[1])
                if held:
                    # previous pair's output projection was held back: its
                    # matmuls fill the PE idle window while DVE normalizes
                    pend.append(held.pop(0))
                    pend.append(held.pop(0))
            for eh in range(2):
                for half in range(2):
                    u = (lambda f=o_unit, eh=eh, half=half: f(eh, half))
                    (held if hp < 3 else pend).append(u)

        while pend:
            pend.pop(0)()
        for p_ in (rcp, osp, ep, opp, ztp, scp):
            p_.release()
        pp.release()
    nc.compile()
    return nc


def _build_generic():
    """Fallback for a non-causal mask: the original fp32r kernel."""
    import concourse.bacc as bacc
    import concourse.tile as tile
    import concourse.mybir as mybir

    f32 = mybir.dt.float32
    f32r = mybir.dt.float32r
    bf16 = mybir.dt.bfloat16
    Exp = mybir.ActivationFunctionType.Exp
    mult = mybir.AluOpType.mult

    nc = bacc.Bacc("TRN2")
    xt = nc.dram_tensor("xt", [128, EC, S], f32r, kind="ExternalInput")
    wq = nc.dram_tensor("wq", [128, EC, 512], f32r, kind="ExternalInput")
    wk = nc.dram_tensor("wk", [128, EC, 512], f32r, kind="ExternalInput")
    wv = nc.dram_tensor("wv", [128, EC, 512], f32r, kind="ExternalInput")
    wo = nc.dram_tensor("wo", [128, EC, 1024], f32r, kind="ExternalInput")
    bq = nc.dram_tensor("bq", [128, 4], f32, kind="ExternalInput")
    bk = nc.dram_tensor("bk", [128, 4], f32, kind="ExternalInput")
    mkt = nc.dram_tensor("mkt", [128, TT, S], bf16, kind="ExternalInput")
    out = nc.dram_tensor("out", [4, 128, 1024], f32, kind="ExternalOutput")

    with tile.TileContext(nc) as tc:
        with (
            tc.tile_pool(name="persist", bufs=1) as pp,
            tc.tile_pool(name="mm", bufs=3, space="PSUM") as mm,
            tc.tile_pool(name="ztp", bufs=2, space="PSUM") as ztp,
        ):
            p1 = tc.alloc_tile_pool(name="p1", bufs=1)
            xt_sb = p1.tile([128, EC, S], f32r)
            wq_sb = p1.tile([128, EC, 512], f32r)
            wk_sb = p1.tile([128, EC, 512], f32r)
            wv_sb = p1.tile([128, EC, 512], f32r)
            for k in range(0, EC, 2):
                nc.sync.dma_start(out=xt_sb[:, k:k + 2, :], in_=xt[:, k:k + 2, :])
                nc.sync.dma_start(out=wq_sb[:, k:k + 2, :], in_=wq[:, k:k + 2, :])
                nc.sync.dma_start(out=wk_sb[:, k:k + 2, :], in_=wk[:, k:k + 2, :])
                nc.sync.dma_start(out=wv_sb[:, k:k + 2, :], in_=wv[:, k:k + 2, :])
            qt_sb = pp.tile([128, 4, S], f32r)
            kt_sb = pp.tile([128, 4, S], f32r)
            vp_sb = pp.tile([128, TT, 1024], bf16)
            x2t_sb = pp.tile([128, EC, 512], f32r)
            bq_sb = pp.tile([128, 4], f32)
            bk_sb = pp.tile([128, 4], f32)
            mkt_sb = pp.tile([128, TT, S], bf16)
            nc.sync.dma_start(out=mkt_sb, in_=mkt.ap())
            nc.sync.dma_start(out=bq_sb, in_=bq.ap())
            nc.sync.dma_start(out=bk_sb, in_=bk.ap())

            vview = vp_sb.rearrange("p t (h two d) -> p t h two d", two=2, d=DH)
            ones_sb = pp.tile([128, 512], f32)
            nc.vector.memset(ones_sb, 1.0)
            ones_v = ones_sb.rearrange("p (h d) -> p h d", d=DH)
            for tt in range(TT):
                nc.vector.tensor_copy(vview[:, tt, :, 1, :], ones_v)

            for wsb, dest, bias in ((wq_sb, qt_sb, bq_sb), (wk_sb, kt_sb, bk_sb)):
                for dt_ in range(4):
                    for sh in range(2):
                        ps = mm.tile([128, 512], f32, tag="mm")
                        for ec in range(EC):
                            nc.tensor.matmul(
                                ps, wsb[:, ec, 128 * dt_:128 * dt_ + 128],
                                xt_sb[:, ec, 512 * sh:512 * sh + 512],
                                start=(ec == 0), stop=(ec == EC - 1),
                            )
                        nc.vector.tensor_scalar_add(
                            out=dest[:, dt_, 512 * sh:512 * sh + 512],
                            in0=ps, scalar1=bias[:, dt_:dt_ + 1],
                        )
            for tt in range(TT):
                ps = mm.tile([128, 512], f32, tag="mm")
                for ec in range(EC):
                    nc.tensor.matmul(
                        ps, xt_sb[:, ec, 128 * tt:128 * tt + 128],
                        wv_sb[:, ec, :],
                        start=(ec == 0), stop=(ec == EC - 1),
                    )
                nc.vector.tensor_copy(
                    vview[:, tt, :, 0, :], ps.rearrange("p (h d) -> p h d", d=DH)
                )
            p1.release()
            late = tc.alloc_tile_pool(name="late", bufs=1)
            expa = tc.alloc_tile_pool(name="expa", bufs=16)
            small = tc.alloc_tile_pool(name="small", bufs=2)
            outp = tc.alloc_tile_pool(name="outp", bufs=2)
            wo_sb = late.tile([128, EC, 1024], f32r)
            nc.sync.dma_start(out=wo_sb, in_=wo.ap())

            for hp in range(HPC // 2):
                pair = (2 * hp, 2 * hp + 1)
                et = {}
                for tt in range(TT):
                    pss = {}
                    for h in pair:
                        dt_ = h // 2
                        pb = 64 * (h % 2)
                        ps = mm.tile([128, 1024], f32, tag="mm", name=f"ps_{h}_{tt}")
                        pss[h] = ps
                        for j in range(NJ):
                            nc.tensor.matmul(
                                ps[:, 512 * j:512 * j + 512],
                                kt_sb[pb:pb + 64, dt_, 128 * tt:128 * tt + 128],
                                qt_sb[pb:pb + 64, dt_, 512 * j:512 * j + 512],
                                start=True, stop=True,
                            )
                            nc.vector.tensor_add(
                                ps[:, 512 * j:512 * j + 512],
                                ps[:, 512 * j:512 * j + 512],
                                mkt_sb[:, tt, 512 * j:512 * j + 512],
                            )
                    for h in pair:
                        ps = pss[h]
                        e = expa.tile([128, 1024], bf16, tag="expa",
                                      name=f"e_{h}_{tt}")
                        nc.scalar.activation(e[:, :], ps[:, :], Exp, scale=0.5)
                        for j in range(NJ):
                            et[(h, tt, j)] = e[:, 512 * j:512 * j + 512]
                for h in pair:
                    zt_f = small.tile([64, S], f32, tag="ztf", name=f"ztf_{h}")
                    rec = small.tile([64, S], f32, tag="rec", name=f"rec_{h}")
                    for j in range(NJ):
                        zt = ztp.tile([128, 512], f32, tag="zt", name=f"zt_{h}_{j}")
                        for i, tt in enumerate(range(TT)):
                            nc.tensor.matmul(
                                zt, vp_sb[:, tt, 128 * h:128 * h + 128],
                                et[(h, tt, j)],
                                start=(i == 0), stop=(i == TT - 1),
                            )
                        nc.vector.reciprocal(rec[:, 512 * j:512 * j + 512],
                                             zt[64:128, :])
                        nc.vector.tensor_copy(zt_f[:, 512 * j:512 * j + 512],
                                              zt[0:64, :])
                    zv = zt_f.rearrange("p (m c par) -> par p c m", m=64, c=8, par=2)
                    rv = rec.rearrange("p (m c par) -> par p c m", m=64, c=8, par=2)
                    for P in range(2):
                        nc.vector.tensor_tensor(
                            x2t_sb[64 * P:64 * P + 64, :, 64 * h:64 * h + 64],
                            zv[P], rv[P], op=mult,
                        )

            for tp in range(4):
                osb = outp.tile([128, 1024], f32, tag="osb")
                for eh in range(2):
                    ps = mm.tile([128, 512], f32, tag="mm")
                    for c in range(EC):
                        nc.tensor.matmul(
                            ps, x2t_sb[:, c, 128 * tp:128 * tp + 128],
                            wo_sb[:, c, 512 * eh:512 * eh + 512],
                            start=(c == 0), stop=(c == EC - 1),
                        )
                    nc.vector.tensor_copy(osb[:, 512 * eh:512 * eh + 512], ps)
                nc.sync.dma_start(out=out[tp], in_=osb)
            for p in (outp, small, expa, late):
                p.release()
    nc.compile()
    return nc


def kernel(inputs, mask, wq, bq, wk, bk, wv, bv, wo, bo):
    import ml_dtypes
    from concourse.bass_utils import run_bass_kernel_spmd

    x = np.asarray(inputs, dtype=np.float32)
    wq = np.asarray(wq, dtype=np.float32)
    wk = np.asarray(wk, dtype=np.float32)
    wv = np.asarray(wv, dtype=np.float32)
    wo = np.asarray(wo, dtype=np.float32)
    bq = np.asarray(bq, dtype=np.float32)
    bk = np.asarray(bk, dtype=np.float32)
    mask2d = np.asarray(mask, dtype=np.float32).reshape(S, S)
    causal_ref = 1.0 - np.tril(np.ones((S, S), dtype=np.float32))
    causal = bool(np.array_equal(mask2d, causal_ref))
    variant = "causal" if causal else "generic"
    if variant not in _CACHE:
        _CACHE[variant] = _build_causal() if causal else _build_generic()
    nc = _CACHE[variant]

    in_maps = []
    for c in range(NCORES):
        b, hg = c // 2, c % 2
        sl = slice(512 * hg, 512 * hg + 512)
        if causal:
            f16 = np.float16
            m = {
                "xt": np.ascontiguousarray(
                    x[b].T.reshape(EC, 128, S).transpose(1, 0, 2)).astype(f16),
                "wq": np.ascontiguousarray(
                    wq[:, sl].reshape(EC, 128, 512).transpose(1, 0, 2)).astype(f16),
                "wk": np.ascontiguousarray(
                    wk[:, sl].reshape(EC, 128, 512).transpose(1, 0, 2)).astype(f16),
                "wv": np.ascontiguousarray(
                    wv[:, sl].reshape(EC, 128, 512).transpose(1, 0, 2)).astype(f16),
                "wo": np.ascontiguousarray(
                    wo.reshape(EC, 128, 1024).transpose(1, 0, 2)).astype(f16),
                "bq": np.ascontiguousarray(bq[sl].reshape(4, 128).T),
                "bk": np.ascontiguousarray(bk[sl].reshape(4, 128).T),
                "tri": np.triu(np.ones((128, 128))).astype(ml_dtypes.bfloat16),
            }
        else:
            m = {
                "xt": np.ascontiguousarray(x[b].T.reshape(EC, 128, S).transpose(1, 0, 2)),
                "wq": np.ascontiguousarray(wq[:, sl].reshape(EC, 128, 512).transpose(1, 0, 2)),
                "wk": np.ascontiguousarray(wk[:, sl].reshape(EC, 128, 512).transpose(1, 0, 2)),
                "wv": np.ascontiguousarray(wv[:, sl].reshape(EC, 128, 512).transpose(1, 0, 2)),
                "wo": np.ascontiguousarray(wo.reshape(EC, 128, 1024).transpose(1, 0, 2)),
                "bq": np.ascontiguousarray(bq[sl].reshape(4, 128).T),
                "bk": np.ascontiguousarray(bk[sl].reshape(4, 128).T),
                "mkt": np.ascontiguousarray(
                    (mask2d.T * np.float32(-2e9)).reshape(TT, 128, S).transpose(1, 0, 2)
                ).astype(ml_dtypes.bfloat16),
            }
        in_maps.append(m)

    global _last_in_maps
    _last_in_maps = in_maps
    res = run_bass_kernel_spmd(nc, in_maps, core_ids=list(range(NCORES)))
    full = np.empty((B, S, E), dtype=np.float32)
    for c in range(NCORES):
        b, hg = c // 2, c % 2
        full[b, 512 * hg:512 * hg + 512, :] = np.asarray(
            res.results[c]["out"], dtype=np.float32).reshape(512, 1024)

    # biases bv/bo are zero in this problem; fold in exactly if ever nonzero.
    bv = np.asarray(bv, dtype=np.float32)
    bo = np.asarray(bo, dtype=np.float32)
    if np.any(bv != 0):
        bmat = np.zeros((S, E), dtype=np.float64)
        tpr = np.arange(S)
        e = np.arange(E)
        bmat[:, :] = bv[(64 * (tpr[:, None] // 64) + e[None, :] % 64)]
        full += (bmat @ np.asarray(wo, dtype=np.float64)).astype(np.float32)[None]
    if np.any(bo != 0):
        full += bo[None, None, :]
    return full


# revision 88
# speedup vs baseline: 1.0147x; 1.0001x over previous
"""Trainium2 Bass kernel for MultiHeadAttention (B=4, S=1024, E=1024, H=16, Dh=64).

Sharding: 8 cores = (batch b in 0..3) x (head-group hg in 0..1, 8 heads each).
The reference reshapes [B,H,S,Dh] -> [B,S,E] WITHOUT transposing heads back, so
head h's attention output occupies output rows t' = h*64 + s//16 - the final
projection is row-parallel across head groups: no cross-core communication.

Per-core pipeline (PE-roofline-bound: every matmul at 1.0 cycles/row):
  - inputs host-cast to fp16 (2.4e-4 exactness, half DMA bytes), outputs bf16
    (cast back on host); 2-ec-batched DMA ordered for early PE start
  - QK projections: 4 PSUM accumulators x 4 subphases (wq/wk x s-halves) so
    the 4-bank score pool coexists - no PSUM pool boundary before attention;
    xt streamed chunk-by-chunk; bias+fp16 cast on DVE into per-(dt,sh) tiles
    (fine-grained deps so scores never wait on unrelated copies)
  - V projection (shares the QK PSUM ring) into fp16 V' with 64 ones cols per
    head (one gpsimd memset): the z matmul emits z^T AND softmax denominators
  - scores^T[t,s] = K @ Q^T per (head-pair, segment-packed col group) into
    [128,2,512] PSUM tiles (1 bank/head): ONE activation exps both heads;
    small diagonal blocks share a tile to cut activation count; e tiles bf16
    (range-safe for exp with no max-subtraction); causal triangle = 0/1
    multiply on gpsimd (SBUF-only engine) for diagonal segments only
  - z/normalize: PSUM z + sums; reciprocal(sums)->SBUF then two PSUM*SBUF
    multiplies write the scrambled-reshape layout directly in fp16
  - out = X2 @ wo (fp16), PSUM->SBUF copy on DVE, bf16 DMA out per half
  - software pipelining: z/normalize/output work is queued as fine-grained
    units and drained one per score block so the in-order PE queue always has
    ready work while ACT exps; V-projection fills pair 0; each pair's output
    projection is HELD one pair and released into the next pair's normalize
    window (fills the DVE-bound endgame).

TimelineSim: 97720 ns (baseline 136254), rel err 2.8e-3 (gate 2e-2).
"""
import numpy as np

B, S, E, H, DH = 4, 1024, 1024, 16, 64
NCORES = 8
HPC = 8          # heads per core
EC = 8           # 128-row chunks of E
TT = 8           # 128-row t-tiles of S
NJ = 2           # 512-col s-blocks

_CACHE = {}


def _build_causal():
    import concourse.bacc as bacc
    import concourse.tile as tile
    import concourse.mybir as mybir

    f32 = mybir.dt.float32
    f32r = mybir.dt.float32r
    f16 = mybir.dt.float16
    bf16 = mybir.dt.bfloat16
    Exp = mybir.ActivationFunctionType.Exp
    Copy = mybir.ActivationFunctionType.Copy
    Ident = mybir.ActivationFunctionType.Identity
    Ln = mybir.ActivationFunctionType.Ln
    mult = mybir.AluOpType.mult
    div = mybir.AluOpType.divide

    nc = bacc.Bacc("TRN2")
    xt = nc.dram_tensor("xt", [128, EC, S], f16, kind="ExternalInput")
    wq = nc.dram_tensor("wq", [128, EC, 512], f16, kind="ExternalInput")
    wk = nc.dram_tensor("wk", [128, EC, 512], f16, kind="ExternalInput")
    wv = nc.dram_tensor("wv", [128, EC, 512], f16, kind="ExternalInput")
    wo = nc.dram_tensor("wo", [128, EC, 1024], f16, kind="ExternalInput")
    bq = nc.dram_tensor("bq", [128, 4], f32, kind="ExternalInput")
    bk = nc.dram_tensor("bk", [128, 4], f32, kind="ExternalInput")
    tri = nc.dram_tensor("tri", [128, 128], bf16, kind="ExternalInput")
    out = nc.dram_tensor("out", [4, 128, 1024], bf16, kind="ExternalOutput")

    with tile.TileContext(nc) as tc:
        pp = tc.alloc_tile_pool(name="pp", bufs=1)
        xt_sb = pp.tile([128, EC, S], f16)
        wq_sb = pp.tile([128, EC, 512], f16)
        wk_sb = pp.tile([128, EC, 512], f16)
        wv_sb = pp.tile([128, EC, 512], f16)
        wo_sb = pp.tile([128, EC, 1024], f16)
        qt_sb = [[pp.tile([128, 512], f16, name=f"qt_{d}_{s}")
                  for s in range(2)] for d in range(4)]
        kt_sb = [[pp.tile([128, 512], f16, name=f"kt_{d}_{s}")
                  for s in range(2)] for d in range(4)]
        vp_sb = pp.tile([128, TT, 1024], f16)
        x2t_sb = pp.tile([128, EC, 512], f16)
        bq_sb = pp.tile([128, 4], f32)
        bk_sb = pp.tile([128, 4], f32)
        tri_sb = pp.tile([128, 128], bf16)

        # ---- DMA preload: 2-ec batches (HWDGE/SP fixed costs are per-DMA),
        # ordered so the QK phase can start early ----
        groups = [(0, 1), (1, 3), (3, 5), (5, 7), (7, 8)]
        for a, b in groups:
            nc.sync.dma_start(out=wq_sb[:, a:b, :], in_=wq[:, a:b, :])
            nc.sync.dma_start(out=xt_sb[:, a:b, 0:512], in_=xt[:, a:b, 0:512])
            if a == 1:
                nc.sync.dma_start(out=bq_sb, in_=bq.ap())
        nc.sync.dma_start(out=bk_sb, in_=bk.ap())
        for k in range(0, EC, 2):
            nc.sync.dma_start(out=wk_sb[:, k:k + 2, :], in_=wk[:, k:k + 2, :])
        for k in range(0, EC, 2):
            nc.sync.dma_start(out=xt_sb[:, k:k + 2, 512:1024], in_=xt[:, k:k + 2, 512:1024])
        nc.sync.dma_start(out=tri_sb, in_=tri.ap())
        for k in range(0, EC, 2):
            nc.sync.dma_start(out=wv_sb[:, k:k + 2, :], in_=wv[:, k:k + 2, :])
        for k in range(0, EC, 2):
            nc.sync.dma_start(out=wo_sb[:, k:k + 2, :], in_=wo[:, k:k + 2, :])

        vview = vp_sb.rearrange("p t (h two d) -> p t h two d", two=2, d=DH)
        for tt in range(TT):
            nc.gpsimd.memset(vview[:, tt, :, 1, :], 1.0)

        # ---- Q^T/K^T projections: 4 PSUM accumulators x 4 subphases so
        # the score pool (4 banks) can be allocated upfront: no PSUM pool
        # boundary between projections and attention ----
        scp = tc.alloc_tile_pool(name="scp", bufs=2, space="PSUM")
        qkp = tc.alloc_tile_pool(name="qkp", bufs=4, space="PSUM")
        ep = tc.alloc_tile_pool(name="ep", bufs=32)
        osp = tc.alloc_tile_pool(name="osp", bufs=2)
        rcp = tc.alloc_tile_pool(name="rcp", bufs=3)
        for sh, wi, wsb, dest, bias in (
            (0, 0, wq_sb, qt_sb, bq_sb), (0, 1, wk_sb, kt_sb, bk_sb),
            (1, 0, wq_sb, qt_sb, bq_sb), (1, 1, wk_sb, kt_sb, bk_sb),
        ):
            sl = slice(512 * sh, 512 * sh + 512)
            pss = [qkp.tile([128, 512], f32, tag="qk", name=f"qk_{sh}_{wi}_{d}")
                   for d in range(4)]
            for ec in range(EC):
                for dt_ in range(4):
                    nc.tensor.matmul(
                        pss[dt_],
                        wsb[:, ec, 128 * dt_:128 * dt_ + 128],
                        xt_sb[:, ec, sl],
                        start=(ec == 0), stop=(ec == EC - 1),
                    )
            for dt_ in range(4):
                nc.vector.tensor_scalar_add(
                    out=dest[dt_][sh], in0=pss[dt_],
                    scalar1=bias[:, dt_:dt_ + 1],
                )

        pend = []
        held = []

        def drain(n):
            for _ in range(min(n, len(pend))):
                pend.pop(0)()

        def vjob(tt):
            ps = qkp.tile([128, 512], f32, tag="qk", name=f"v_{tt}")
            for ec in range(EC):
                nc.tensor.matmul(
                    ps, xt_sb[:, ec, 128 * tt:128 * tt + 128], wv_sb[:, ec, :],
                    start=(ec == 0), stop=(ec == EC - 1),
                )
            nc.vector.tensor_copy(
                vview[:, tt, :, 0, :], ps.rearrange("p (h d) -> p h d", d=DH)
            )

        for tt in range(TT):
            pend.append(lambda tt=tt: vjob(tt))

        ztp = opp = None
        for hp in range(4):
            pair = (2 * hp, 2 * hp + 1)
            et_ref = {}
            js = [1, 0] if hp == 1 else [0, 1]
            blocks = []
            for j in js:
                cur, used = [], 0
                for tt in range(TT):
                    if 512 * j + 511 < 128 * tt:
                        continue
                    w = 512 * j + 512 - max(512 * j, 128 * tt)
                    if used + w > 512:
                        if cur:
                            blocks.append(cur)
                        cur, used = [], 0
                    cur.append((tt, j))
                    used += w
                if cur:
                    blocks.append(cur)

            def sblock(group, hp=hp, pair=pair, et_ref=et_ref):
                # group: list of (tt, j) segments packed into one score tile
                # (the small diagonal blocks share a tile so one activation
                # instruction exps them together)
                segs = []
                col = 0
                for (tt, j) in group:
                    c0 = 128 * tt
                    lo = max(512 * j, c0)
                    w = 512 * j + 512 - lo
                    segs.append((tt, j, c0, lo, w, col))
                    col += w
                total = col
                gname = f"{hp}_{group[0][0]}_{group[0][1]}"
                sc = scp.tile([128, 2, 512], f32, tag="sc", name=f"sc_{gname}")
                for (tt, j, c0, lo, w, col) in segs:
                    kc = c0 % 512
                    for hh, h in enumerate(pair):
                        pb = 64 * (h % 2)
                        nc.tensor.matmul(
                            sc[:, hh, col:col + w],
                            kt_sb[hp][tt // 4][pb:pb + 64, kc:kc + 128],
                            qt_sb[hp][j][pb:pb + 64, lo - 512 * j:512],
                            start=True, stop=True,
                        )
                et = ep.tile([128, 2, 512], bf16, tag="e", name=f"e_{gname}")
                nc.scalar.activation(
                    et[:, :, 0:total], sc[:, :, 0:total], Exp, scale=0.5
                )
                for (tt, j, c0, lo, w, col) in segs:
                    if lo == c0:
                        for hh in range(2):
                            nc.gpsimd.tensor_tensor(
                                et[:, hh, col:col + 128],
                                et[:, hh, col:col + 128], tri_sb, op=mult,
                            )
                    et_ref[(tt, j)] = (et, col, lo - 512 * j, w)

            def push_z(h, hh, j, et_ref=et_ref):
                """Append z-chain units (split mm halves + parallel divides)."""
                box = {}
                tts = [tt for tt in range(TT) if (tt, j) in et_ref]
                halves = ([tts[:4], tts[4:]] if len(tts) > 5 else [tts])

                def mm_unit(sub, first, last, h=h, hh=hh, j=j, box=box,
                            et_ref=et_ref, ntot=len(tts)):
                    if first:
                        box["zt"] = ztp.tile([128, 512], f32, tag="zt",
                                             name=f"zt_{h}_{j}")
                    zt = box["zt"]
                    for i, tt in enumerate(sub):
                        et, col, rel0, w = et_ref[(tt, j)]
                        nc.tensor.matmul(
                            zt[:, rel0:rel0 + w],
                            vp_sb[:, tt, 128 * h:128 * h + 128],
                            et[:, hh, col:col + w],
                            start=(first and i == 0),
                            stop=(last and i == len(sub) - 1),
                        )

                def div_unit(h=h, j=j, hp=hp, box=box):
                    # x2t = z / sums; both live in PSUM and an ALU op may
                    # read only one PSUM operand, so 1/sums goes via SBUF.
                    # For the final pair ACT is idle (no more exps) while DVE
                    # gates the endgame: compute 1/sums as exp(-ln(sums))
                    # there instead.
                    zt = box["zt"]
                    rec = rcp.tile([64, 512], f32, tag="rec",
                                   name=f"rec_{h}_{j}")
                    nc.vector.reciprocal(rec, zt[64:128, :])
                    zv = zt[0:64].rearrange("p (m c par) -> par p c m",
                                            m=32, c=8, par=2)
                    rv = rec.rearrange("p (m c par) -> par p c m",
                                       m=32, c=8, par=2)
                    o0 = 64 * h + 32 * j
                    nc.vector.tensor_tensor(
                        x2t_sb[0:64, :, o0:o0 + 32], zv[0], rv[0], op=mult)
                    nc.vector.tensor_tensor(
                        x2t_sb[64:128, :, o0:o0 + 32], zv[1], rv[1], op=mult)

                for si, sub in enumerate(halves):
                    pend.append(lambda s=sub, f=mm_unit, first=(si == 0),
                                last=(si == len(halves) - 1): f(s, first, last))
                pend.append(div_unit)

            osb_box = {}

            def o_unit(eh, half, hp=hp, osb_box=osb_box):
                if "t" not in osb_box:
                    osb_box["t"] = osp.tile([128, 1024], bf16, tag="osb",
                                            name=f"osb_{hp}")
                t = osb_box["t"]
                if half == 0:
                    osb_box[eh] = opp.tile([128, 512], f32, tag="op",
                                           name=f"op_{hp}_{eh}")
                op_t = osb_box[eh]
                cs = range(0, 4) if half == 0 else range(4, EC)
                for c in cs:
                    nc.tensor.matmul(
                        op_t, x2t_sb[:, c, 128 * hp:128 * hp + 128],
                        wo_sb[:, c, 512 * eh:512 * eh + 512],
                        start=(c == 0), stop=(c == EC - 1),
                    )
                if half == 1:
                    nc.vector.tensor_copy(t[:, 512 * eh:512 * eh + 512], op_t)
                    # final pair: issue eh0's transfer from the (idle)
                    # ACT queue so the last transfer isn't serialized
                    eng = nc.scalar if (eh == 0 and hp == 3) else nc.sync
                    eng.dma_start(out=out[hp, :, 512 * eh:512 * eh + 512],
                                  in_=t[:, 512 * eh:512 * eh + 512])

            nj0 = sum(1 for g in blocks if g[0][1] == js[0])
            for bi, group in enumerate(blocks):
                sblock(group)
                drain(1)
                if bi == nj0 - 1 and hp > 0:
                    # first j-group e-tiles complete: its z can drain now
                    for hh, h in enumerate(pair):
                        push_z(h, hh, js[0])
            if hp == 0:
                while pend:
                    pend.pop(0)()
                qkp.release()
                ztp = tc.alloc_tile_pool(name="ztp", bufs=2, space="PSUM")
                opp = tc.alloc_tile_pool(name="opp", bufs=2, space="PSUM")
                for hh, h in enumerate(pair):
                    push_z(h, hh, js[0])
            for hh, h in enumerate(pair):
                push_z(h, hh, js[1])
                if held:
                    # previous pair's output projection was held back: its
                    # matmuls fill the PE idle window while DVE normalizes
                    pend.append(held.pop(0))
                    pend.append(held.pop(0))
            for eh in range(2):
                for half in range(2):
                    u = (lambda f=o_unit, eh=eh, half=half: f(eh, half))
                    (held if hp < 3 else pend).append(u)

        while pend:
            pend.pop(0)()
        for p_ in (rcp, osp, ep, opp, ztp, scp):
            p_.release()
        pp.release()
    nc.compile()
    return nc


def _build_generic():
    """Fallback for a non-causal mask: the original fp32r kernel."""
    import concourse.bacc as bacc
    import concourse.tile as tile
    import concourse.mybir as mybir

    f32 = mybir.dt.float32
    f32r = mybir.dt.float32r
    bf16 = mybir.dt.bfloat16
    Exp = mybir.ActivationFunctionType.Exp
    mult = mybir.AluOpType.mult

    nc = bacc.Bacc("TRN2")
    xt = nc.dram_tensor("xt", [128, EC, S], f32r, kind="ExternalInput")
    wq = nc.dram_tensor("wq", [128, EC, 512], f32r, kind="ExternalInput")
    wk = nc.dram_tensor("wk", [128, EC, 512], f32r, kind="ExternalInput")
    wv = nc.dram_tensor("wv", [128, EC, 512], f32r, kind="ExternalInput")
    wo = nc.dram_tensor("wo", [128, EC, 1024], f32r, kind="ExternalInput")
    bq = nc.dram_tensor("bq", [128, 4], f32, kind="ExternalInput")
    bk = nc.dram_tensor("bk", [128, 4], f32, kind="ExternalInput")
    mkt = nc.dram_tensor("mkt", [128, TT, S], bf16, kind="ExternalInput")
    out = nc.dram_tensor("out", [4, 128, 1024], f32, kind="ExternalOutput")

    with tile.TileContext(nc) as tc:
        with (
            tc.tile_pool(name="persist", bufs=1) as pp,
            tc.tile_pool(name="mm", bufs=3, space="PSUM") as mm,
            tc.tile_pool(name="ztp", bufs=2, space="PSUM") as ztp,
        ):
            p1 = tc.alloc_tile_pool(name="p1", bufs=1)
            xt_sb = p1.tile([128, EC, S], f32r)
            wq_sb = p1.tile([128, EC, 512], f32r)
            wk_sb = p1.tile([128, EC, 512], f32r)
            wv_sb = p1.tile([128, EC, 512], f32r)
            for k in range(0, EC, 2):
                nc.sync.dma_start(out=xt_sb[:, k:k + 2, :], in_=xt[:, k:k + 2, :])
                nc.sync.dma_start(out=wq_sb[:, k:k + 2, :], in_=wq[:, k:k + 2, :])
                nc.sync.dma_start(out=wk_sb[:, k:k + 2, :], in_=wk[:, k:k + 2, :])
                nc.sync.dma_start(out=wv_sb[:, k:k + 2, :], in_=wv[:, k:k + 2, :])
            qt_sb = pp.tile([128, 4, S], f32r)
            kt_sb = pp.tile([128, 4, S], f32r)
            vp_sb = pp.tile([128, TT, 1024], bf16)
            x2t_sb = pp.tile([128, EC, 512], f32r)
            bq_sb = pp.tile([128, 4], f32)
            bk_sb = pp.tile([128, 4], f32)
            mkt_sb = pp.tile([128, TT, S], bf16)
            nc.sync.dma_start(out=mkt_sb, in_=mkt.ap())
            nc.sync.dma_start(out=bq_sb, in_=bq.ap())
            nc.sync.dma_start(out=bk_sb, in_=bk.ap())

            vview = vp_sb.rearrange("p t (h two d) -> p t h two d", two=2, d=DH)
            ones_sb = pp.tile([128, 512], f32)
            nc.vector.memset(ones_sb, 1.0)
            ones_v = ones_sb.rearrange("p (h d) -> p h d", d=DH)
            for tt in range(TT):
                nc.vector.tensor_copy(vview[:, tt, :, 1, :], ones_v)

            for wsb, dest, bias in ((wq_sb, qt_sb, bq_sb), (wk_sb, kt_sb, bk_sb)):
                for dt_ in range(4):
                    for sh in range(2):
                        ps = mm.tile([128, 512], f32, tag="mm")
                        for ec in range(EC):
                            nc.tensor.matmul(
                                ps, wsb[:, ec, 128 * dt_:128 * dt_ + 128],
                                xt_sb[:, ec, 512 * sh:512 * sh + 512],
                                start=(ec == 0), stop=(ec == EC - 1),
                            )
                        nc.vector.tensor_scalar_add(
                            out=dest[:, dt_, 512 * sh:512 * sh + 512],
                            in0=ps, scalar1=bias[:, dt_:dt_ + 1],
                        )
            for tt in range(TT):
                ps = mm.tile([128, 512], f32, tag="mm")
                for ec in range(EC):
                    nc.tensor.matmul(
                        ps, xt_sb[:, ec, 128 * tt:128 * tt + 128],
                        wv_sb[:, ec, :],
                        start=(ec == 0), stop=(ec == EC - 1),
                    )
                nc.vector.tensor_copy(
                    vview[:, tt, :, 0, :], ps.rearrange("p (h d) -> p h d", d=DH)
                )
            p1.release()
            late = tc.alloc_tile_pool(name="late", bufs=1)
            expa = tc.alloc_tile_pool(name="expa", bufs=16)
            small = tc.alloc_tile_pool(name="small", bufs=2)
            outp = tc.alloc_tile_pool(name="outp", bufs=2)
            wo_sb = late.tile([128, EC, 1024], f32r)
            nc.sync.dma_start(out=wo_sb, in_=wo.ap())

            for hp in range(HPC // 2):
                pair = (2 * hp, 2 * hp + 1)
                et = {}
                for tt in range(TT):
                    pss = {}
                    for h in pair:
                        dt_ = h // 2
                        pb = 64 * (h % 2)
                        ps = mm.tile([128, 1024], f32, tag="mm", name=f"ps_{h}_{tt}")
                        pss[h] = ps
                        for j in range(NJ):
                            nc.tensor.matmul(
                                ps[:, 512 * j:512 * j + 512],
                                kt_sb[pb:pb + 64, dt_, 128 * tt:128 * tt + 128],
                                qt_sb[pb:pb + 64, dt_, 512 * j:512 * j + 512],
                                start=True, stop=True,
                            )
                            nc.vector.tensor_add(
                                ps[:, 512 * j:512 * j + 512],
                                ps[:, 512 * j:512 * j + 512],
                                mkt_sb[:, tt, 512 * j:512 * j + 512],
                            )
                    for h in pair:
                        ps = pss[h]
                        e = expa.tile([128, 1024], bf16, tag="expa",
                                      name=f"e_{h}_{tt}")
                        nc.scalar.activation(e[:, :], ps[:, :], Exp, scale=0.5)
                        for j in range(NJ):
                            et[(h, tt, j)] = e[:, 512 * j:512 * j + 512]
                for h in pair:
                    zt_f = small.tile([64, S], f32, tag="ztf", name=f"ztf_{h}")
                    rec = small.tile([64, S], f32, tag="rec", name=f"rec_{h}")
                    for j in range(NJ):
                        zt = ztp.tile([128, 512], f32, tag="zt", name=f"zt_{h}_{j}")
                        for i, tt in enumerate(range(TT)):
                            nc.tensor.matmul(
                                zt, vp_sb[:, tt, 128 * h:128 * h + 128],
                                et[(h, tt, j)],
                                start=(i == 0), stop=(i == TT - 1),
                            )
                        nc.vector.reciprocal(rec[:, 512 * j:512 * j + 512],
                                             zt[64:128, :])
                        nc.vector.tensor_copy(zt_f[:, 512 * j:512 * j + 512],
                                              zt[0:64, :])
                    zv = zt_f.rearrange("p (m c par) -> par p c m", m=64, c=8, par=2)
                    rv = rec.rearrange("p (m c par) -> par p c m", m=64, c=8, par=2)
                    for P in range(2):
                        nc.vector.tensor_tensor(
                            x2t_sb[64 * P:64 * P + 64, :, 64 * h:64 * h + 64],
                            zv[P], rv[P], op=mult,
                        )

            for tp in range(4):
                osb = outp.tile([128, 1024], f32, tag="osb")
                for eh in range(2):
                    ps = mm.tile([128, 512], f32, tag="mm")
                    for c in range(EC):
                        nc.tensor.matmul(
                            ps, x2t_sb[:, c, 128 * tp:128 * tp + 128],
                            wo_sb[:, c, 512 * eh:512 * eh + 512],
                            start=(c == 0), stop=(c == EC - 1),
                        )
                    nc.vector.tensor_copy(osb[:, 512 * eh:512 * eh + 512], ps)
                nc.sync.dma_start(out=out[tp], in_=osb)
            for p in (outp, small, expa, late):
                p.release()
    nc.compile()
    return nc


def kernel(inputs, mask, wq, bq, wk, bk, wv, bv, wo, bo):
    import ml_dtypes
    from concourse.bass_utils import run_bass_kernel_spmd

    x = np.asarray(inputs, dtype=np.float32)
    wq = np.asarray(wq, dtype=np.float32)
    wk = np.asarray(wk, dtype=np.float32)
    wv = np.asarray(wv, dtype=np.float32)
    wo = np.asarray(wo, dtype=np.float32)
    bq = np.asarray(bq, dtype=np.float32)
    bk = np.asarray(bk, dtype=np.float32)
    mask2d = np.asarray(mask, dtype=np.float32).reshape(S, S)
    causal_ref = 1.0 - np.tril(np.ones((S, S), dtype=np.float32))
    causal = bool(np.array_equal(mask2d, causal_ref))
    variant = "causal" if causal else "generic"
    if variant not in _CACHE:
        _CACHE[variant] = _build_causal() if causal else _build_generic()
    nc = _CACHE[variant]

    in_maps = []
    for c in range(NCORES):
        b, hg = c // 2, c % 2
        sl = slice(512 * hg, 512 * hg + 512)
        if causal:
            f16 = np.float16
            m = {
                "xt": np.ascontiguousarray(
                    x[b].T.reshape(EC, 128, S).transpose(1, 0, 2)).astype(f16),
                "wq": np.ascontiguousarray(
                    wq[:, sl].reshape(EC, 128, 512).transpose(1, 0, 2)).astype(f16),
                "wk": np.ascontiguousarray(
                    wk[:, sl].reshape(EC, 128, 512).transpose(1, 0, 2)).astype(f16),
                "wv": np.ascontiguousarray(
                    wv[:, sl].reshape(EC, 128, 512).transpose(1, 0, 2)).astype(f16),
                "wo": np.ascontiguousarray(
                    wo.reshape(EC, 128, 1024).transpose(1, 0, 2)).astype(f16),
                "bq": np.ascontiguousarray(bq[sl].reshape(4, 128).T),
                "bk": np.ascontiguousarray(bk[sl].reshape(4, 128).T),
                "tri": np.triu(np.ones((128, 128))).astype(ml_dtypes.bfloat16),
            }
        else:
            m = {
                "xt": np.ascontiguousarray(x[b].T.reshape(EC, 128, S).transpose(1, 0, 2)),
                "wq": np.ascontiguousarray(wq[:, sl].reshape(EC, 128, 512).transpose(1, 0, 2)),
                "wk": np.ascontiguousarray(wk[:, sl].reshape(EC, 128, 512).transpose(1, 0, 2)),
                "wv": np.ascontiguousarray(wv[:, sl].reshape(EC, 128, 512).transpose(1, 0, 2)),
                "wo": np.ascontiguousarray(wo.reshape(EC, 128, 1024).transpose(1, 0, 2)),
                "bq": np.ascontiguousarray(bq[sl].reshape(4, 128).T),
                "bk": np.ascontiguousarray(bk[sl].reshape(4, 128).T),
                "mkt": np.ascontiguousarray(
                    (mask2d.T * np.float32(-2e9)).reshape(TT, 128, S).transpose(1, 0, 2)
                ).astype(ml_dtypes.bfloat16),
            }
        in_maps.append(m)

    global _last_in_maps
    _last_in_maps = in_maps
    res = run_bass_kernel_spmd(nc, in_maps, core_ids=list(range(NCORES)))
    full = np.empty((B, S, E), dtype=np.float32)
    for c in range(NCORES):
        b, hg = c // 2, c % 2
        full[b, 512 * hg:512 * hg + 512, :] = np.asarray(
            res.results[c]["out"], dtype=np.float32).reshape(512, 1024)

    # biases bv/bo are zero in this problem; fold in exactly if ever nonzero.
    bv = np.asarray(bv, dtype=np.float32)
    bo = np.asarray(bo, dtype=np.float32)
    if np.any(bv != 0):
        bmat = np.zeros((S, E), dtype=np.float64)
        tpr = np.arange(S)
        e = np.arange(E)
        bmat[:, :] = bv[(64 * (tpr[:, None] // 64) + e[None, :] % 64)]
        full += (bmat @ np.asarray(wo, dtype=np.float64)).astype(np.float32)[None]
    if np.any(bo != 0):
        full += bo[None, None, :]
    return full
